# revision 1
# baseline (speedup 1.0000x reference)
"""CoAttentionFusion Trainium2 kernel (8 NeuronCores, SPMD, no collectives).

Sharding: core c = (batch b = c//2, query-half h = c%2). Each core computes
the full module for its 1024 query rows of batch b; K/V projections over the
full T=2048 are recomputed by both cores of a batch pair (21% redundant
compute, zero communication).

On-chip strategy:
  - activations feature-major (x^T: [d, tokens]) so every linear layer is
    lhsT = W (as stored, [din, dout]), rhs = x^T -> y^T, no transposes.
  - attention computed with transposed scores S^T[k, q] = K^T_h . Q_h^T so the
    exp'd probabilities P^T are directly the moving operand of P@V.
  - V produced token-major with a ones-column appended; the P@V accumulation
    then yields O'^T = [rawO^T ; softmax-denominator] in one group.
  - normalization of O via DVE reciprocal + GPSIMD partition_broadcast.
  - LayerNorms run token-major (per-partition stats) on 128-token chunks,
    entering/leaving via PE transposes.
  - SBUF is tight: x / K / V / O are streamed through DRAM scratch in 512-token
    blocks; K/V projections for attention-2 are emitted interleaved with
    attention-1 (and O-proj/LN of stream t with attention-2) to keep PE busy
    while the ACT engine grinds through exp().
All matmuls bf16 with fp32 PSUM accumulation; softmax/LN math in fp32.
"""

import numpy as np

P = 128
D = 1024
T = 2048
TQ = 1024
NH = 16
HD = 64
DT = D // P          # 8 feature tiles
KT = T // P          # 16 key-token tiles
QC = TQ // P         # 8 query-token chunks
NQ = TQ // 512       # 2 query free-dim tiles
EPS = 1e-5

_WNAMES = ["qt", "kf", "vf", "qf", "kt", "vt", "ot", "of"]


def _build_nc():
    import concourse.bass as bass
    import concourse.tile as tile
    from concourse import bacc, mybir
    from concourse.masks import make_identity
    from contextlib import ExitStack

    f32 = mybir.dt.float32
    bf16 = mybir.dt.bfloat16
    AF = mybir.ActivationFunctionType
    ALU = mybir.AluOpType

    nc = bacc.Bacc("TRN2", target_bir_lowering=False, debug=False, num_devices=8)

    # ---------------- DRAM I/O ----------------
    # x arrives pre-blocked/partition-major: [block, p, dt, 512]
    xtT_d = nc.dram_tensor("xtT", [T // 512, P, DT, 512], bf16,
                           kind="ExternalInput")
    xfT_d = nc.dram_tensor("xfT", [T // 512, P, DT, 512], bf16,
                           kind="ExternalInput")
    xtq_d = nc.dram_tensor("xtq", [TQ, D], f32, kind="ExternalInput")
    xfq_d = nc.dram_tensor("xfq", [TQ, D], f32, kind="ExternalInput")
    # weights pre-shuffled partition-major: [p, kt, dout]
    w_d = {}
    b_d = {}
    for n in _WNAMES:
        w_d[n] = nc.dram_tensor(f"w_{n}", [P, DT, D], bf16, kind="ExternalInput")
        b_d[n] = nc.dram_tensor(f"b_{n}", [P, DT], f32, kind="ExternalInput")
    w_d["f1"] = nc.dram_tensor("w_f1", [P, 2 * DT, D], bf16, kind="ExternalInput")
    b_d["f1"] = nc.dram_tensor("b_f1", [P, DT], f32, kind="ExternalInput")
    w_d["f2"] = nc.dram_tensor("w_f2", [P, DT, D], bf16, kind="ExternalInput")
    b_d["f2"] = nc.dram_tensor("b_f2", [P, DT], f32, kind="ExternalInput")
    # vf/vt biases additionally as broadcast-ready rows
    br_d = {}
    for n in ["vf", "vt"]:
        br_d[n] = nc.dram_tensor(f"br_{n}", [1, D], f32, kind="ExternalInput")
    ln_d = {}
    for n in ["lnt_w", "lnt_b", "lnf_w", "lnf_b", "lnu_w", "lnu_b"]:
        ln_d[n] = nc.dram_tensor(n, [D], f32, kind="ExternalInput")
    out_d = nc.dram_tensor("out", [TQ, D], f32, kind="ExternalOutput")

    with tile.TileContext(nc) as tc, ExitStack() as ctx:
        const = ctx.enter_context(tc.tile_pool(name="const", bufs=1))
        wpool = ctx.enter_context(tc.tile_pool(name="wpool", bufs=2))
        res = ctx.enter_context(tc.tile_pool(name="res", bufs=1))
        xs = ctx.enter_context(tc.tile_pool(name="xs", bufs=3))
        kvs = ctx.enter_context(tc.tile_pool(name="kvs", bufs=3))
        ost = ctx.enter_context(tc.tile_pool(name="ost", bufs=2))
        stg = ctx.enter_context(tc.tile_pool(name="stg", bufs=4))
        ppool = ctx.enter_context(tc.tile_pool(name="ppool", bufs=4))
        spool = ctx.enter_context(tc.tile_pool(name="spool", bufs=2))
        lnp = ctx.enter_context(tc.tile_pool(name="lnp", bufs=2))
        rowp = ctx.enter_context(tc.tile_pool(name="rowp", bufs=1))
        dram = ctx.enter_context(tc.tile_pool(name="dram", bufs=1, space="DRAM"))
        ps_acc = ctx.enter_context(tc.tile_pool(name="ps_acc", bufs=2, space="PSUM"))
        ps_o = ctx.enter_context(tc.tile_pool(name="ps_o", bufs=3, space="PSUM"))
        ps_ln = ctx.enter_context(tc.tile_pool(name="ps_ln", bufs=1, space="PSUM"))

        ident = const.tile([P, P], bf16)
        make_identity(nc, ident[:])
        eps_t = const.tile([P, 1], f32, name="eps")
        nc.gpsimd.memset(eps_t[:], EPS)

        bias_col = {}

        def load_bias_cols():
            for n in ["qt", "kf", "qf", "kt", "ot", "of", "f1", "f2"]:
                t = const.tile([P, DT], f32, name=f"bias_{n}")
                nc.sync.dma_start(t[:], b_d[n][:, :])
                bias_col[n] = t

        def row_bcast(dram_t, tag):
            """[1, D] f32 dram row -> [128, D] bf16 broadcast tile."""
            r = rowp.tile([1, D], f32, tag="row")
            nc.sync.dma_start(r[:], dram_t)
            rb = rowp.tile([1, D], bf16, tag="rowb")
            nc.vector.tensor_copy(rb[:], r[:])
            b = rowp.tile([P, D], bf16, tag=tag)
            nc.gpsimd.partition_broadcast(b[:], rb[:])
            return b

        def load_weight(name, kts=None):
            dram_t = w_d[name]
            if kts is None:
                kts = (0, dram_t.shape[1])
            nkt = kts[1] - kts[0]
            t = wpool.tile([P, nkt, D], bf16, tag="w")
            # split per contraction tile so the first matmul can start early
            for kt in range(nkt):
                nc.sync.dma_start(t[:, kt, :], dram_t[:, kts[0] + kt, :])
            return t

        # DRAM scratch
        kf_dr = dram.tile([D, T], bf16, name="kf_dr")
        kt_dr = dram.tile([D, T], bf16, name="kt_dr")
        vf_dr = dram.tile([NH, P, KT, HD + 1], bf16, name="vf_dr")
        vt_dr = dram.tile([NH, P, KT, HD + 1], bf16, name="vt_dr")
        ot_dr = dram.tile([NQ, P, DT, 512], bf16, name="ot_dr")
        of_dr = dram.tile([NQ, P, DT, 512], bf16, name="of_dr")

        # ------------------------------------------------------------------
        # unit builders (each unit = one closure emitting one psum group)
        # ------------------------------------------------------------------
        def x_block_loader(x_dram, n0):
            blk = {}

            def get():
                if "xb" not in blk:
                    xb = xs.tile([P, DT, 512], bf16, tag="xs")
                    nc.sync.dma_start(xb[:], x_dram[n0 // 512])
                    blk["xb"] = xb
                return blk["xb"]

            return get

        def featmaj_units(w_sb, bname, get_rhs, n0, sink, act=None):
            """y^T[dout, n0:n0+512] units; sink(dt, psum_ap) consumes."""
            units = []
            nkt = w_sb.shape[1]
            for dt in range(DT):

                def u(dt=dt):
                    ps = ps_o.tile([P, 512], f32, tag="ops")
                    rhs = get_rhs()
                    for kt in range(nkt):
                        nc.tensor.matmul(
                            ps[:],
                            w_sb[:, kt, dt * P: (dt + 1) * P],
                            rhs[:, kt, :],
                            start=(kt == 0),
                            stop=(kt == nkt - 1),
                        )
                    sink(dt, ps[:])

                units.append(u)
            return units

        def proj_to_dram_sink(bname, k_dr, n0, eng="act"):
            def sink(dt, ps):
                s = stg.tile([P, 512], bf16, tag="stg")
                if eng == "dve":
                    nc.vector.tensor_scalar_add(
                        s[:], ps, bias_col[bname][:, dt: dt + 1]
                    )
                else:
                    nc.scalar.activation(
                        s[:], ps, AF.Identity,
                        bias=bias_col[bname][:, dt: dt + 1],
                    )
                nc.sync.dma_start(k_dr[dt * P: (dt + 1) * P, n0: n0 + 512], s[:])

            return sink

        def proj_to_sbuf_sink(bname, out_sb, n0, eng="act"):
            def sink(dt, ps):
                if eng == "dve":
                    nc.vector.tensor_scalar_add(
                        out_sb[:, dt, n0: n0 + 512], ps,
                        bias_col[bname][:, dt: dt + 1],
                    )
                else:
                    nc.scalar.activation(
                        out_sb[:, dt, n0: n0 + 512],
                        ps,
                        AF.Identity,
                        bias=bias_col[bname][:, dt: dt + 1],
                    )

            return sink

        def v_units(w_sb, vb_bc, get_x, n0, v_dr):
            """token-major V' units for token block n0 (4 chunks x 2 halves)."""
            units = []
            for tci in range(4):
                for no in range(2):

                    def u(tci=tci, no=no):
                        ps = ps_o.tile([P, 512], f32, tag="ops")
                        xb = get_x()
                        for kt in range(DT):
                            nc.tensor.matmul(
                                ps[:],
                                xb[:, kt, tci * P: (tci + 1) * P],
                                w_sb[:, kt, no * 512: (no + 1) * 512],
                                start=(kt == 0),
                                stop=(kt == DT - 1),
                            )
                        s = stg.tile([P, 8, HD + 1], bf16, tag="stg")
                        nc.vector.tensor_add(
                            s[:, :, 0:HD],
                            ps.rearrange("p (h e) -> p h e", h=8),
                            vb_bc[:, no * 512: (no + 1) * 512].rearrange(
                                "p (h e) -> p h e", h=8
                            ),
                        )
                        nc.gpsimd.memset(s[:, :, HD: HD + 1], 1.0)
                        kt_idx = (n0 + tci * P) // P
                        nc.sync.dma_start(
                            v_dr.rearrange("h p kt e -> p h kt e")[
                                :, no * 8: (no + 1) * 8, kt_idx, :
                            ],
                            s[:],
                        )

                    units.append(u)
            return units

        def attention_units(qT, k_dr, v_dr, o_dr):
            """One closure per (qt, head-pair). Streams K/V', writes O^T."""
            units = []
            for qt in range(NQ):
                for hp in range(NH // 2):

                    def u(qt=qt, hp=hp):
                        kS = kvs.tile([P, T], bf16, tag="kS")
                        for half in range(2):
                            nc.sync.dma_start(
                                kS[:, half * TQ: (half + 1) * TQ],
                                k_dr[hp * P: (hp + 1) * P,
                                     half * TQ: (half + 1) * TQ],
                            )
                        vS = []
                        for sub in range(2):
                            v = kvs.tile([P, KT, HD + 1], bf16, tag="vS")
                            src = v_dr[hp * 2 + sub]
                            for half in range(2):
                                nc.sync.dma_start(
                                    v[:, half * 8: (half + 1) * 8, :],
                                    src[:, half * 8: (half + 1) * 8, :],
                                )
                            vS.append(v)
                        o_ps = [
                            ps_o.tile([P, 512], f32, tag="ops", name=f"o{s}")
                            for s in range(2)
                        ]
                        prev = None
                        for pr in range(KT // 2):
                            cur = []
                            for sub in range(2):
                                lo, hi = sub * HD, (sub + 1) * HD
                                s = ps_acc.tile([P, 2, 512], f32, tag="acc")
                                for j in range(2):
                                    kt = 2 * pr + j
                                    nc.tensor.matmul(
                                        s[:, j, :],
                                        kS[lo:hi, kt * P: (kt + 1) * P],
                                        qT[lo:hi, hp, qt * 512: (qt + 1) * 512],
                                        start=True,
                                        stop=True,
                                        tile_position=(lo, 0),
                                    )
                                pT = ppool.tile([P, 2, 512], bf16, tag="pT")
                                nc.scalar.activation(
                                    pT[:], s[:], AF.Exp, scale=1.0 / 8.0
                                )
                                cur.append((sub, pT))
                            # PV for previous pair (skewed to hide exp latency)
                            if prev is not None:
                                for sub, pTp in prev:
                                    for j in range(2):
                                        kt = 2 * (pr - 1) + j
                                        nc.tensor.matmul(
                                            o_ps[sub][0: HD + 1, :],
                                            vS[sub][:, kt, :],
                                            pTp[:, j, :],
                                            start=(kt == 0),
                                            stop=False,
                                        )
                            prev = cur
                        for sub, pTp in prev:
                            for j in range(2):
                                kt = KT - 2 + j
                                nc.tensor.matmul(
                                    o_ps[sub][0: HD + 1, :],
                                    vS[sub][:, kt, :],
                                    pTp[:, j, :],
                                    start=False,
                                    stop=(j == 1),
                                )
                        for sub in range(2):
                            inv = spool.tile([1, 512], f32, tag="inv")
                            nc.vector.reciprocal(inv[:], o_ps[sub][HD: HD + 1, :])
                            bc = spool.tile([HD, 512], f32, tag="bc")
                            nc.gpsimd.partition_broadcast(bc[:], inv[:])
                            s = stg.tile([HD, 512], bf16, tag="stg")
                            nc.vector.tensor_mul(s[:], o_ps[sub][0:HD, :], bc[:])
                            nc.sync.dma_start(
                                o_dr[qt, sub * HD: (sub + 1) * HD, hp, :], s[:]
                            )

                    units.append(u)
            return units

        def oproj_units(w_sb, bname, o_dr, attnT, eng="act"):
            units = []
            loaders = []
            for n0 in range(0, TQ, 512):
                get = {}

                def get_ob(n0=n0, get=get):
                    if "ob" not in get:
                        ob = ost.tile([P, DT, 512], bf16, tag="os")
                        nc.sync.dma_start(ob[:], o_dr[n0 // 512])
                        get["ob"] = ob
                    return get["ob"]

                loaders.append(get_ob)

                for dt in range(DT):

                    def u(dt=dt, n0=n0, get_ob=get_ob):
                        ps = ps_o.tile([P, 512], f32, tag="ops")
                        ob = get_ob()
                        for kt in range(DT):
                            nc.tensor.matmul(
                                ps[:],
                                w_sb[:, kt, dt * P: (dt + 1) * P],
                                ob[:, kt, :],
                                start=(kt == 0),
                                stop=(kt == DT - 1),
                            )
                        if eng == "dve":
                            nc.vector.tensor_scalar_add(
                                attnT[:, dt, n0: n0 + 512], ps[:],
                                bias_col[bname][:, dt: dt + 1],
                            )
                        else:
                            nc.scalar.activation(
                                attnT[:, dt, n0: n0 + 512],
                                ps[:],
                                AF.Identity,
                                bias=bias_col[bname][:, dt: dt + 1],
                            )

                    units.append(u)
            return units, loaders

        def ln_units(inT, resid_dram, w_bc, b_bc, outT, out_dram=None):
            """Token-major LN, one unit per 128-token chunk."""
            units = []
            for qc in range(QC):

                def u(qc=qc):
                    tok = ps_ln.tile([P, D], bf16, tag="lntok")
                    for dt in range(DT):
                        nc.tensor.transpose(
                            tok[:, dt * P: (dt + 1) * P],
                            inT[:, dt, qc * P: (qc + 1) * P],
                            ident[:],
                        )
                    if resid_dram is not None:
                        s = lnp.tile([P, D], f32, tag="lnB")
                        xq = lnp.tile([P, D], f32, tag="lnA")
                        nc.sync.dma_start(
                            xq[:], resid_dram[qc * P: (qc + 1) * P, :]
                        )
                        nc.vector.tensor_add(s[:], xq[:], tok[:])
                    else:
                        s = tok  # stats/normalize read the PSUM tile directly
                    bns = spool.tile([P, 2, 6], f32, tag="bns")
                    nc.vector.bn_stats(bns[:, 0, :], s[:, 0:512])
                    nc.vector.bn_stats(bns[:, 1, :], s[:, 512:D])
                    mv = spool.tile([P, 2], f32, tag="mv")
                    nc.vector.bn_aggr(mv[:], bns[:])
                    std = spool.tile([P, 1], f32, tag="std")
                    nc.scalar.activation(std[:], mv[:, 1:2], AF.Sqrt, bias=eps_t[:])
                    rstd = spool.tile([P, 1], f32, tag="rstd")
                    nc.vector.reciprocal(rstd[:], std[:])
                    t1 = lnp.tile([P, D], f32, tag="lnA")
                    nc.vector.scalar_tensor_tensor(
                        t1[:], s[:], mv[:, 0:1], w_bc[:],
                        op0=ALU.subtract, op1=ALU.mult,
                    )
                    if out_dram is not None:
                        o = lnp.tile([P, D], f32, tag="lnB")
                        nc.vector.scalar_tensor_tensor(
                            o[:], t1[:], rstd[:], b_bc[:],
                            op0=ALU.mult, op1=ALU.add,
                        )
                        nc.sync.dma_start(out_dram[qc * P: (qc + 1) * P, :], o[:])
                    else:
                        nrm = lnp.tile([P, D], bf16, tag="lnnrm")
                        nc.vector.scalar_tensor_tensor(
                            nrm[:], t1[:], rstd[:], b_bc[:],
                            op0=ALU.mult, op1=ALU.add,
                        )
                        ft = ps_ln.tile([P, D], bf16, tag="lntok")
                        for dt in range(DT):
                            nc.tensor.transpose(
                                ft[:, dt * P: (dt + 1) * P],
                                nrm[:, dt * P: (dt + 1) * P],
                                ident[:],
                            )
                        nc.vector.tensor_copy(
                            outT[:, :, qc * P: (qc + 1) * P],
                            ft.rearrange("p (dt c) -> p dt c", dt=DT),
                        )

                units.append(u)
            return units

        def run_interleaved(primary, filler):
            k = 0
            for i, u in enumerate(primary):
                u()
                want = (i + 1) * len(filler) // len(primary)
                while k < want:
                    filler[k]()
                    k += 1
            while k < len(filler):
                filler[k]()
                k += 1

        # ------------------------------------------------------------------
        # program
        # ------------------------------------------------------------------
        # resident activation tiles (slot-shared by tag across phases)
        qT_t = res.tile([P, DT, TQ], bf16, name="qT_t", tag="qTt")
        qT_f = res.tile([P, DT, TQ], bf16, name="qT_f", tag="qTf")

        # Phase 1: Kf/Vf -> dram, Qt -> sbuf
        loaders = [x_block_loader(xfT_d, n0) for n0 in range(0, T, 512)]
        loaders[0]()  # x DMA issued before the weight DMAs (startup latency)
        w_kf = load_weight("kf")
        load_bias_cols()  # after the critical first x/w DMAs
        w_vf = load_weight("vf")
        vb_f = row_bcast(br_d["vf"][:, :], "vbc")
        for bi, n0 in enumerate(range(0, T, 512)):
            get_x = loaders[bi]
            ku = featmaj_units(
                w_kf, "kf", get_x, n0, proj_to_dram_sink("kf", kf_dr, n0)
            )
            vu = v_units(w_vf, vb_f, get_x, n0, vf_dr)
            run_interleaved(ku, vu)
        w_qt = load_weight("qt")
        for n0 in range(0, TQ, 512):
            get_x = x_block_loader(xtT_d, n0)
            for u in featmaj_units(
                w_qt, "qt", get_x, n0, proj_to_sbuf_sink("qt", qT_t, n0)
            ):
                u()
        # Phase 2: attention-1 (streams kf/vf) || first-half Kt/Vt/Qf units;
        # the second halves (head-pairs 4-7) become early attention-2 fillers.
        w_kt = load_weight("kt")
        w_vt = load_weight("vt")
        w_qf = load_weight("qf")
        vb_t = row_bcast(br_d["vt"][:, :], "vbc")
        fillers = []
        fillers2 = []
        for n0 in range(0, T, 512):
            get_x = x_block_loader(xtT_d, n0)
            fillers += featmaj_units(
                w_kt, "kt", get_x, n0,
                proj_to_dram_sink("kt", kt_dr, n0, eng="dve"),
            )
            fillers += v_units(w_vt, vb_t, get_x, n0, vt_dr)
        for n0 in range(0, TQ, 512):
            get_x = x_block_loader(xfT_d, n0)
            fillers += featmaj_units(
                w_qf, "qf", get_x, n0,
                proj_to_sbuf_sink("qf", qT_f, n0, eng="dve"),
            )
        run_interleaved(attention_units(qT_t, kf_dr, vf_dr, ot_dr), fillers)

        # Phase 3: attention-2 || O-proj(t) + LN(t)
        w_ot = load_weight("ot")
        attnT_t = res.tile([P, DT, TQ], bf16, name="attnT_t", tag="big")
        fusedT_t = res.tile([P, DT, TQ], bf16, name="fusedT_t", tag="qTt")
        lnt_wb = row_bcast(ln_d["lnt_w"].rearrange("(a d) -> a d", a=1), "lnw")
        lnt_bb = row_bcast(ln_d["lnt_b"].rearrange("(a d) -> a d", a=1), "lnb")
        oprojA, _ = oproj_units(w_ot, "ot", ot_dr, attnT_t, eng="dve")
        lnA = ln_units(attnT_t, xtq_d, lnt_wb, lnt_bb, fusedT_t)
        # spread PE-rich oproj units across attn-2's tail; LN chunks (PE-poor,
        # long vector chains) slot between them as soon as their deps allow
        fillers2 += oprojA[:8]
        for i in range(4):
            fillers2.append(oprojA[8 + 2 * i])
            fillers2.append(oprojA[9 + 2 * i])
            fillers2.append(lnA[i])
        fillers2 += lnA[4:]

        # O-proj(f) block 0 only needs the qt=0 half of attention-2's output
        # (and attnT_t to be fully consumed) -- run it as late attn-2 filler.
        w_of = load_weight("of")
        attnT_f = res.tile([P, DT, TQ], bf16, name="attnT_f", tag="big")
        fusedT_f = res.tile([P, DT, TQ], bf16, name="fusedT_f", tag="ff")
        lnf_wb = row_bcast(ln_d["lnf_w"].rearrange("(a d) -> a d", a=1), "lnw")
        lnf_bb = row_bcast(ln_d["lnf_b"].rearrange("(a d) -> a d", a=1), "lnb")
        oprojB, oprojB_ld = oproj_units(w_of, "of", of_dr, attnT_f)
        lnB = ln_units(attnT_f, xfq_d, lnf_wb, lnf_bb, fusedT_f)
        run_interleaved(attention_units(qT_f, kt_dr, vt_dr, of_dr), fillers2)

        # Phases 4+5 (zippered): O-proj(f), LN(f), fus1, fus2, LN(fus) are a
        # pipeline over 512-token blocks; interleave so LN vector math hides
        # under the next stage's matmuls.

        w_f1a = load_weight("f1", kts=(0, DT))
        w_f1b = load_weight("f1", kts=(DT, 2 * DT))
        hT = res.tile([P, DT, TQ], bf16, name="hT", tag="qTf")

        def fus1_units(n0):
            units = []
            for dt in range(DT):

                def u(dt=dt, n0=n0):
                    ps = ps_o.tile([P, 512], f32, tag="ops")
                    for kt in range(DT):
                        nc.tensor.matmul(
                            ps[:],
                            w_f1a[:, kt, dt * P: (dt + 1) * P],
                            fusedT_t[:, kt, n0: n0 + 512],
                            start=(kt == 0),
                            stop=False,
                        )
                    for kt in range(DT):
                        nc.tensor.matmul(
                            ps[:],
                            w_f1b[:, kt, dt * P: (dt + 1) * P],
                            fusedT_f[:, kt, n0: n0 + 512],
                            start=False,
                            stop=(kt == DT - 1),
                        )
                    nc.scalar.activation(
                        hT[:, dt, n0: n0 + 512],
                        ps[:],
                        AF.Gelu,
                        bias=bias_col["f1"][:, dt: dt + 1],
                    )

                units.append(u)
            return units

        w_f2 = load_weight("f2")
        o2T = res.tile([P, DT, TQ], bf16, name="o2T", tag="big")
        lnu_wb = row_bcast(ln_d["lnu_w"].rearrange("(a d) -> a d", a=1), "lnw")
        lnu_bb = row_bcast(ln_d["lnu_b"].rearrange("(a d) -> a d", a=1), "lnb")
        lnU = ln_units(o2T, None, lnu_wb, lnu_bb, None, out_dram=out_d)

        def fus2_units(n0):
            return featmaj_units(
                w_f2, "f2", lambda: hT[:, :, n0: n0 + 512], n0,
                proj_to_sbuf_sink("f2", o2T, n0),
            )

        for u in oprojB[:8]:
            u()
        run_interleaved(oprojB[8:], lnB[:4])
        run_interleaved(fus1_units(0), lnB[4:])
        for u in fus1_units(512):
            u()
        for u in fus2_units(0):
            u()
        run_interleaved(fus2_units(512), lnU[:4])
        for u in lnU[4:]:
            u()

    nc.compile()
    return nc


# ---------------------------------------------------------------------------
# host side
# ---------------------------------------------------------------------------
_CACHE = {}


def _get_nc():
    if "nc" not in _CACHE:
        _CACHE["nc"] = _build_nc()
    return _CACHE["nc"]


def _make_in_maps(inputs):
    import ml_dtypes

    bf16 = ml_dtypes.bfloat16

    def wshuf(w):
        # [din, dout] -> partition-major [128, din/128, dout]
        w = np.asarray(w)
        nkt = w.shape[0] // P
        return np.ascontiguousarray(
            w.reshape(nkt, P, w.shape[1]).transpose(1, 0, 2)
        ).astype(bf16)

    def xshuf(xT):
        # [D, T] -> [T/512 blocks, 128, DT, 512]
        return np.ascontiguousarray(
            xT.reshape(DT, P, T // 512, 512).transpose(2, 1, 0, 3)
        ).astype(bf16)

    t = np.asarray(inputs["temporal_tokens"], np.float32)
    f = np.asarray(inputs["feature_tokens"], np.float32)

    def bshuf(b):
        # [D] -> [128, DT] (partition-major, contiguous per partition)
        return np.ascontiguousarray(
            np.asarray(b, np.float32).reshape(DT, P).T)

    shared = {}
    for n in _WNAMES:
        shared[f"w_{n}"] = wshuf(inputs[f"{n}_w"])
        shared[f"b_{n}"] = bshuf(inputs[f"{n}_b"])
    shared["w_f1"] = wshuf(inputs["fus1_w"])
    shared["b_f1"] = bshuf(inputs["fus1_b"])
    shared["w_f2"] = wshuf(inputs["fus2_w"])
    shared["b_f2"] = bshuf(inputs["fus2_b"])
    for n in ["vf", "vt"]:
        shared[f"br_{n}"] = np.ascontiguousarray(
            np.asarray(inputs[f"{n}_b"], np.float32).reshape(1, D))
    for src, dst in [
        ("ln_t_w", "lnt_w"), ("ln_t_b", "lnt_b"),
        ("ln_f_w", "lnf_w"), ("ln_f_b", "lnf_b"),
        ("ln_fus_w", "lnu_w"), ("ln_fus_b", "lnu_b"),
    ]:
        shared[dst] = np.ascontiguousarray(inputs[src], dtype=np.float32)

    in_maps = []
    for c in range(8):
        b, half = divmod(c, 2)
        r0 = half * TQ
        xt = t[b]
        xf = f[b]
        # query rows first, remaining rows after (K/V order is irrelevant)
        perm = np.concatenate([np.arange(r0, T), np.arange(0, r0)])
        m = dict(shared)
        m["xtT"] = xshuf(xt[perm].T)
        m["xfT"] = xshuf(xf[perm].T)
        m["xtq"] = np.ascontiguousarray(xt[r0: r0 + TQ])
        m["xfq"] = np.ascontiguousarray(xf[r0: r0 + TQ])
        in_maps.append(m)
    return in_maps


def kernel(**inputs):
    try:
        import jax

        jax.config.update("jax_compilation_cache_dir", "/tmp/jaxcache")
        jax.config.update("jax_persistent_cache_min_entry_size_bytes", -1)
        jax.config.update("jax_persistent_cache_min_compile_time_secs", 0.0)
    except Exception:
        pass
    from concourse.bass_utils import run_bass_kernel_spmd

    nc = _get_nc()
    in_maps = _make_in_maps(inputs)
    res = run_bass_kernel_spmd(nc, in_maps, list(range(8)))
    B = 4
    out = np.empty((B, T, D), np.float32)
    for c in range(8):
        b, half = divmod(c, 2)
        out[b, half * TQ: (half + 1) * TQ] = res.results[c]["out"]
    return out



# revision 2
# speedup vs baseline: 1.0090x; 1.0090x over previous
"""CoAttentionFusion TRN2 kernel v2 (8 cores SPMD, fp8 DoubleRow + 2-engine exp).

Per core c: batch b=c//2, query-half h=c%2 (1024 q rows); K/V over full T=2048
recomputed per pair (collectives cost more than the 109us of PE they save).

Key techniques vs the bf16 baseline:
- All attention-path matmuls in fp8e4m3 with DoubleRow perf mode (2 k-tiles
  per instruction, 0.5 cycles/row): K/V/Q projections, QK^T (2x32 contraction
  pairs), P@V (P^T stationary -> token-major O), O-projection.
- Q/K weights column-permuted on host so each head's 64 dims land as
  [32 partitions x 2 halves] for the DoubleRow QK layout.
- Q/K biases removed from the matmuls: K-bias is softmax-invariant; Q-bias
  becomes a per-key factor g_k = exp((x_kv @ (Wk@bq))/8) folded into V' rows
  and the denominator column of V'.
- O-proj bias and V-bias@W_o folded into the f32 residual on host.
- exp split between ACT (true exp->fp8) and DVE (Schraudolph uint8 bit-trick
  -> fp8e4m3) per EXP_PATTERN; probabilities consumed as fp8.
- Token-major O-proj output feeds LayerNorm directly (no LN in-transpose);
  LN rstd via Newton rsqrt on DVE (avoids ACT table thrashing with exp).
- fusion MLP stays bf16 (fp8 there fails the tolerance).
"""

import numpy as np

P = 128
D = 1024
T = 2048
TQ = 1024
NH = 16
HD = 64
DT = 8
KT = 16
QC = 8
EPS = 1e-5
LOG2E = 1.4426950408889634
O_SCALE = 32.0
WO_SCALE = 16.0
VG_SCALE = 64.0

# exp engine per (g8, head) slot within a unit: 'A' = ACT true exp,
# 'D' = DVE Schraudolph. Alternating keeps both engines fed. attn-1 runs
# with the projection sinks on ACT (9A/7D); attn-2 has the LN work on DVE
# (11A/5D).
EXP_PATTERN1 = "ADADADAADADADADA"
EXP_PATTERN2 = "ADAADADAADAADAAA"

_WQK = ["qt", "kf", "qf", "kt"]


def _build_nc(ln_trivial):
    import concourse.bass as bass
    import concourse.tile as tile
    from concourse import bacc, mybir
    from concourse.masks import make_identity
    from contextlib import ExitStack

    f32 = mybir.dt.float32
    bf16 = mybir.dt.bfloat16
    fp8 = mybir.dt.float8e4
    u8 = mybir.dt.uint8
    i32 = mybir.dt.int32
    AF = mybir.ActivationFunctionType
    ALU = mybir.AluOpType
    DR = mybir.MatmulPerfMode.DoubleRow

    nc = bacc.Bacc("TRN2", target_bir_lowering=False, debug=False, num_devices=8)

    # ---------------- DRAM I/O ----------------
    xtT_d = nc.dram_tensor("xtT", [T // 512, P, DT, 512], fp8, kind="ExternalInput")
    xfT_d = nc.dram_tensor("xfT", [T // 512, P, DT, 512], fp8, kind="ExternalInput")
    xtq_d = nc.dram_tensor("xtq", [TQ, D], f32, kind="ExternalInput")
    xfq_d = nc.dram_tensor("xfq", [TQ, D], f32, kind="ExternalInput")
    w_d = {}
    for n in ["qt", "kf", "vf", "qf", "kt", "vt", "ot", "of"]:
        w_d[n] = nc.dram_tensor(f"w_{n}", [P, DT, D], fp8, kind="ExternalInput")
    w_d["f1"] = nc.dram_tensor("w_f1", [DT, P, 2 * DT, P], bf16, kind="ExternalInput")
    w_d["f2"] = nc.dram_tensor("w_f2", [P, DT, D], bf16, kind="ExternalInput")
    vg_d = {"f": nc.dram_tensor("vg_f", [P, DT, NH], fp8, kind="ExternalInput"),
            "t": nc.dram_tensor("vg_t", [P, DT, NH], fp8, kind="ExternalInput")}
    bf1_d = nc.dram_tensor("b_f1", [P, DT], f32, kind="ExternalInput")
    b2_d = nc.dram_tensor("b2row", [1, D], f32, kind="ExternalInput")
    ln_d = {}
    if not ln_trivial:
        for n in ["lnt_w", "lnt_b", "lnf_w", "lnf_b", "lnu_w", "lnu_b"]:
            ln_d[n] = nc.dram_tensor(n, [1, D], f32, kind="ExternalInput")
    out_d = nc.dram_tensor("out", [TQ, D], f32, kind="ExternalOutput")

    with tile.TileContext(nc) as tc, ExitStack() as ctx:
        const = ctx.enter_context(tc.tile_pool(name="const", bufs=1))
        res = ctx.enter_context(tc.tile_pool(name="res", bufs=1))
        wpool = ctx.enter_context(tc.tile_pool(name="wpool", bufs=2))
        w2pool = ctx.enter_context(tc.tile_pool(name="w2pool", bufs=1))
        f1pool = ctx.enter_context(tc.tile_pool(name="f1pool", bufs=3))
        xs = ctx.enter_context(tc.tile_pool(name="xs", bufs=3))
        kvp = ctx.enter_context(tc.tile_pool(name="kvp", bufs=2))
        vsp = ctx.enter_context(tc.tile_pool(name="vsp", bufs=2))
        ppool = ctx.enter_context(tc.tile_pool(name="ppool", bufs=2))
        otokp = ctx.enter_context(tc.tile_pool(name="otokp", bufs=2))
        gpool = ctx.enter_context(tc.tile_pool(name="gpool", bufs=1))
        stg = ctx.enter_context(tc.tile_pool(name="stg", bufs=4))
        lns = ctx.enter_context(tc.tile_pool(name="lns", bufs=3))
        lsc = ctx.enter_context(tc.tile_pool(name="lsc", bufs=4))
        rowp = ctx.enter_context(tc.tile_pool(name="rowp", bufs=1))
        rsd = ctx.enter_context(tc.tile_pool(name="rsd", bufs=2))
        outp = ctx.enter_context(tc.tile_pool(name="outp", bufs=1))
        dram = ctx.enter_context(tc.tile_pool(name="dram", bufs=1, space="DRAM"))
        ps_qk = ctx.enter_context(tc.tile_pool(name="ps_qk", bufs=2, space="PSUM"))
        ps_pv = ctx.enter_context(tc.tile_pool(name="ps_pv", bufs=2, space="PSUM"))
        ps_mm = ctx.enter_context(tc.tile_pool(name="ps_mm", bufs=2, space="PSUM"))

        ident8 = const.tile([P, P], fp8, name="ident8")
        make_identity(nc, ident8[:])
        ident16 = const.tile([P, P], bf16, name="ident16")
        make_identity(nc, ident16[:])
        eps_t = const.tile([P, 1], f32, name="eps")
        nc.gpsimd.memset(eps_t[:], EPS)
        magic = const.tile([P, 1], i32, name="magic")
        nc.gpsimd.memset(magic[:], 0x5F3759DF)
        one_i = const.tile([P, 1], i32, name="one_i")
        nc.gpsimd.memset(one_i[:], 1)

        def row_bcast(dram_t, tag, dt_=f32):
            r = rowp.tile([1, D], f32, tag="row")
            nc.sync.dma_start(r[:], dram_t)
            if dt_ is not f32:
                rr = rowp.tile([1, D], dt_, tag="rowc")
                nc.vector.tensor_copy(rr[:], r[:])
                r = rr
            b = const.tile([P, D], dt_, name=tag)
            nc.gpsimd.partition_broadcast(b[:], r[:])
            return b

        b2_bc = row_bcast(b2_d[:, :], "b2bc")
        ln_bc = {}
        if not ln_trivial:
            for n in ["lnt_w", "lnt_b", "lnf_w", "lnf_b", "lnu_w", "lnu_b"]:
                ln_bc[n] = row_bcast(ln_d[n][:, :], n)
        bf1_col = const.tile([P, DT], f32, name="bf1")
        nc.sync.dma_start(bf1_col[:], bf1_d[:, :])

        def lw8(name):
            # two DMAs: first half unblocks the first matmuls, and each DMA
            # costs ~625ns of HWDGE issue time so fewer is better
            t = wpool.tile([P, DT, D], fp8, tag="w8")
            nc.sync.dma_start(t[:, 0:4, :], w_d[name][:, 0:4, :])
            nc.sync.dma_start(t[:, 4:8, :], w_d[name][:, 4:8, :])
            return t

        # DRAM scratch
        k_dr = {"f": dram.tile([4, P, 2, T], fp8, name="kf_dr"),
                "t": dram.tile([4, P, 2, T], fp8, name="kt_dr")}
        v_dr = {"f": dram.tile([NH, P, KT, HD + 1], fp8, name="vf_dr"),
                "t": dram.tile([NH, P, KT, HD + 1], fp8, name="vt_dr")}

        # resident activations
        qT = {"t": res.tile([P, 4, 2, TQ], fp8, name="qT_t"),
              "f": res.tile([P, 4, 2, TQ], fp8, name="qT_f")}
        attnT = {"t": res.tile([P, DT, TQ], fp8, name="attnT_t"),
                 "f": res.tile([P, DT, TQ], fp8, name="attnT_f")}
        fusedT = {"t": res.tile([P, DT, TQ], bf16, name="fusedT_t"),
                  "f": res.tile([P, DT, TQ], bf16, name="fusedT_f")}
        hT = res.tile([P, DT, TQ], bf16, name="hT")
        g_sb = {"f": gpool.tile([P, KT, NH], f32, name="g_f"),
                "t": gpool.tile([P, KT, NH], f32, name="g_t")}

        def x_loader(x_dram, bi):
            blk = {}

            def get():
                if "x" not in blk:
                    xb = xs.tile([P, DT, 512], fp8, tag="xs")
                    nc.sync.dma_start(xb[:], x_dram[bi])
                    blk["x"] = xb
                return blk["x"]

            return get

        # ---------------- unit builders ----------------
        def qk_proj_units(w_sb, get_x, n0, sink):
            units = []
            for hg in range(4):
                for dh in range(2):

                    def u(hg=hg, dh=dh):
                        ps = ps_mm.tile([P, 512], f32, tag="mm")
                        xb = get_x()
                        cs = (2 * hg + dh) * P
                        for t4 in range(4):
                            nc.tensor.matmul(
                                ps[:],
                                w_sb[:, 2 * t4: 2 * t4 + 2, cs: cs + P],
                                xb[:, 2 * t4: 2 * t4 + 2, :],
                                start=(t4 == 0), stop=(t4 == 3), perf_mode=DR,
                            )
                        sink(hg, dh, ps)

                    units.append(u)
            return units

        def k_sink(kd, n0):
            def sink(hg, dh, ps):
                s = stg.tile([P, 512], fp8, tag="k8")
                nc.scalar.activation(s[:], ps[:], AF.Identity)
                nc.sync.dma_start(kd[hg][:, dh, n0: n0 + 512], s[:])

            return sink

        def q_sink(qdst, n0):
            def sink(hg, dh, ps):
                nc.scalar.activation(qdst[:, hg, dh, n0: n0 + 512], ps[:],
                                     AF.Identity)

            return sink

        def g_units(vg_sb, get_x, n0, g_t):
            units = []
            for tci in range(4):

                def u(tci=tci):
                    ps = ps_mm.tile([P, 512], f32, tag="mm")
                    xb = get_x()
                    for t4 in range(4):
                        nc.tensor.matmul(
                            ps[:, 0:NH],
                            xb[:, 2 * t4: 2 * t4 + 2, tci * P: (tci + 1) * P],
                            vg_sb[:, 2 * t4: 2 * t4 + 2, :],
                            start=(t4 == 0), stop=(t4 == 3), perf_mode=DR,
                        )
                    kti = (n0 + tci * P) // P
                    nc.scalar.activation(g_t[:, kti, :], ps[:, 0:NH], AF.Exp,
                                         scale=1.0 / (8.0 * VG_SCALE))

                units.append(u)
            return units

        def v_units(w_sb, get_x, n0, g_t, vd):
            units = []
            for tci in range(4):
                for half in range(2):

                    def u(tci=tci, half=half):
                        ps = ps_mm.tile([P, 512], f32, tag="mm")
                        xb = get_x()
                        for t4 in range(4):
                            nc.tensor.matmul(
                                ps[:],
                                xb[:, 2 * t4: 2 * t4 + 2, tci * P: (tci + 1) * P],
                                w_sb[:, 2 * t4: 2 * t4 + 2,
                                     half * 512: (half + 1) * 512],
                                start=(t4 == 0), stop=(t4 == 3), perf_mode=DR,
                            )
                        kti = (n0 + tci * P) // P
                        s = stg.tile([P, 8, HD + 1], fp8, tag="v8")
                        gb = g_t[:, kti, half * 8:(half + 1) * 8]
                        nc.vector.tensor_tensor(
                            s[:, :, 0:HD],
                            ps[:].rearrange("p (h e) -> p h e", h=8),
                            gb.unsqueeze(-1).broadcast_to([P, 8, HD]),
                            op=ALU.mult,
                        )
                        nc.vector.tensor_copy(s[:, :, HD: HD + 1],
                                              gb.unsqueeze(-1))
                        nc.sync.dma_start(
                            vd.rearrange("h p kt e -> p h kt e")[
                                :, half * 8: (half + 1) * 8, kti, :],
                            s[:],
                        )

                    units.append(u)
            return units

        # ---------------- attention ----------------
        exp_ctr = [0]

        def attn_units(qt_sb, kd, vd, att_dst, qi, hp, pat):
            """returns list of quanta closures for unit (qi, hp)."""
            state = {}
            h0, h1 = 2 * hp, 2 * hp + 1
            hg = hp // 2

            def get_ks():
                if "ks" not in state:
                    ks = kvp.tile([P, 2, T], fp8, tag="ks")
                    nc.sync.dma_start(ks[:], kd[hg])
                    state["ks"] = ks
                return state["ks"]

            def get_vs(hi):
                if "vs" not in state:
                    v = vsp.tile([P, 2, KT, HD + 1], fp8, tag="vs", name="vs")
                    nc.sync.dma_start(
                        v[:], vd.rearrange("h p kt e -> p h kt e")
                        [:, 2 * hp: 2 * hp + 2, :, :])
                    state["vs"] = v
                return state["vs"][:, hi]

            def get_pt(hi):
                key = f"pt{hi}"
                if key not in state:
                    state[key] = ppool.tile([P, 8, 2, 512], fp8, tag="pt",
                                            name=f"pt{hi}")
                return state[key]

            def qk_quantum(hi, g8lo, g8hi):
                def u():
                    ks = get_ks()
                    h = 2 * hp + hi
                    base = 32 * (h % 4)
                    for g8 in range(g8lo, g8hi):
                        ps = ps_qk.tile([P, 2, 512], f32, tag="qk")
                        for j in range(2):
                            kt = 2 * g8 + j
                            nc.tensor.matmul(
                                ps[:, j, :],
                                ks[base: base + 32, :, kt * P: (kt + 1) * P],
                                qt_sb[base: base + 32, h // 4, :,
                                      qi * 512: (qi + 1) * 512],
                                start=True, stop=True, perf_mode=DR,
                                tile_position=(base, 0),
                            )
                        pt = get_pt(hi)
                        eng = pat[exp_ctr[0] % len(pat)]
                        exp_ctr[0] += 1
                        if eng == "A":
                            nc.scalar.activation(pt[:, g8, :, :], ps[:],
                                                 AF.Exp, scale=0.125)
                        else:
                            nc.vector.tensor_scalar(
                                pt[:, g8, :, :].bitcast(u8), ps[:],
                                LOG2E, 56.0, op0=ALU.mult, op1=ALU.add)

                return u

            def pv_quantum(hi):
                def u():
                    vs = get_vs(hi)
                    pt = get_pt(hi)
                    otok = state["otok"]
                    for qc in range(4):
                        ps = ps_pv.tile([P, 512], f32, tag="pv")
                        for g8 in range(8):
                            nc.tensor.matmul(
                                ps[:, 0: HD + 1],
                                pt[:, g8, :, qc * P: (qc + 1) * P],
                                vs[:, 2 * g8: 2 * g8 + 2, :],
                                start=(g8 == 0), stop=(g8 == 7), perf_mode=DR,
                            )
                        inv = lsc.tile([P, 1], f32, tag="inv")
                        nc.vector.reciprocal(inv[:], ps[:, HD: HD + 1])
                        nc.vector.tensor_scalar(
                            otok[:, qc, hi * HD: (hi + 1) * HD],
                            ps[:, 0:HD], inv[:], O_SCALE,
                            op0=ALU.mult, op1=ALU.mult)

                return u

            def fin_quantum():
                def u():
                    otok = state["otok"]
                    tr = ps_mm.tile([P, 4, P], bf16, tag="mm", name="otr")
                    for qc in range(4):
                        nc.tensor.transpose(tr[:, qc, :], otok[:, qc, :],
                                            ident16[:])
                    nc.scalar.activation(
                        att_dst[:, hp, qi * 512: (qi + 1) * 512]
                        .rearrange("p (a b) -> p a b", a=4),
                        tr[:], AF.Identity,
                    )

                return u

            def start_quantum():
                def u():
                    state["otok"] = otokp.tile([P, 4, P], bf16, tag="otok",
                                               name="otok")
                    get_ks()
                    get_vs(0)
                    get_vs(1)

                return u

            # head-major: PV of head 0 overlaps head 1's exps, halving the
            # exp->PV convoy on the DVE queue and freeing pt slots earlier.
            # fin is returned separately so the caller can defer it one unit
            # (its deps are stale by then -> no ACT-queue stall).
            return ([start_quantum(), qk_quantum(0, 0, 4), qk_quantum(0, 4, 8),
                     pv_quantum(0), qk_quantum(1, 0, 4), qk_quantum(1, 4, 8),
                     pv_quantum(1)], fin_quantum())

        # ---------------- O-proj + LN ----------------
        def newton_rstd(var_ap):
            """rstd [P,1] f32 from var (+eps) via bit-trick + 2 Newton steps."""
            a = lsc.tile([P, 1], f32, tag="nva")
            nc.vector.tensor_scalar_add(a[:], var_ap, eps_t[:])
            y = lsc.tile([P, 1], f32, tag="nvy")
            nc.vector.tensor_scalar(y[:].bitcast(i32), a[:].bitcast(i32),
                                    one_i[:], None,
                                    op0=ALU.logical_shift_right)
            nc.vector.tensor_tensor(y[:].bitcast(i32), magic[:],
                                    y[:].bitcast(i32), op=ALU.subtract)
            uu = lsc.tile([P, 1], f32, tag="nvu")
            # one Newton step: ~0.17% max rel error on rstd, well inside
            # the tolerance; a second step would double the serial DVE chain
            nc.vector.tensor_tensor(uu[:], y[:], y[:], op=ALU.mult)
            nc.vector.tensor_tensor(uu[:], uu[:], a[:], op=ALU.mult)
            nc.vector.tensor_scalar(uu[:], uu[:], -0.5, 1.5,
                                    op0=ALU.mult, op1=ALU.add)
            nc.vector.tensor_tensor(y[:], y[:], uu[:], op=ALU.mult)
            return y

        def ln_chunk(s, wkey, outT=None, qc=None, out_dram=None):
            """stats+normalize s [P,D] bf16; write transposed to outT or
            f32 rows to out_dram."""
            bns = lsc.tile([P, 2, 6], f32, tag="bns")
            nc.vector.bn_stats(bns[:, 0, :], s[:, 0:512])
            nc.vector.bn_stats(bns[:, 1, :], s[:, 512:D])
            mv = lsc.tile([P, 2], f32, tag="mv")
            nc.vector.bn_aggr(mv[:], bns[:])
            rstd = newton_rstd(mv[:, 1:2])
            if out_dram is not None:
                o = outp.tile([P, D], f32, tag="out")
                nc.vector.tensor_scalar(o[:], s[:], mv[:, 0:1], rstd[:],
                                        op0=ALU.subtract, op1=ALU.mult)
                if not ln_trivial:
                    nc.vector.tensor_tensor(o[:], o[:], ln_bc[wkey + "_w"][:],
                                            op=ALU.mult)
                    nc.vector.tensor_tensor(o[:], o[:], ln_bc[wkey + "_b"][:],
                                            op=ALU.add)
                nc.sync.dma_start(out_dram[qc * P: (qc + 1) * P, :], o[:])
            else:
                nrm = lns.tile([P, D], bf16, tag="nrm")
                nc.vector.tensor_scalar(nrm[:], s[:], mv[:, 0:1], rstd[:],
                                        op0=ALU.subtract, op1=ALU.mult)
                if not ln_trivial:
                    nc.vector.tensor_tensor(nrm[:], nrm[:], ln_bc[wkey + "_w"][:],
                                            op=ALU.mult)
                    nc.vector.tensor_tensor(nrm[:], nrm[:], ln_bc[wkey + "_b"][:],
                                            op=ALU.add)
                tr = ps_mm.tile([P, D], bf16, tag="mm", name="lntr")
                for dt in range(DT):
                    nc.tensor.transpose(tr[:, dt * P: (dt + 1) * P],
                                        nrm[:, dt * P: (dt + 1) * P], ident16[:])
                nc.vector.tensor_copy(
                    outT[:, :, qc * P: (qc + 1) * P],
                    tr[:].rearrange("p (dt c) -> p dt c", dt=DT),
                )

        def oproj_ln_units(att_sb, w_sb, resid_dram, wkey, outT):
            units = []
            for qc in range(QC):

                def u(qc=qc):
                    xq = rsd.tile([P, D], f32, tag="xq")
                    nc.sync.dma_start(xq[:], resid_dram[qc * P: (qc + 1) * P, :])
                    s = lns.tile([P, D], bf16, tag="lns")
                    for half in range(2):
                        ps = ps_mm.tile([P, 512], f32, tag="mm")
                        for t4 in range(4):
                            nc.tensor.matmul(
                                ps[:],
                                att_sb[:, 2 * t4: 2 * t4 + 2, qc * P: (qc + 1) * P],
                                w_sb[:, 2 * t4: 2 * t4 + 2,
                                     half * 512: (half + 1) * 512],
                                start=(t4 == 0), stop=(t4 == 3), perf_mode=DR,
                            )
                        nc.vector.scalar_tensor_tensor(
                            s[:, half * 512: (half + 1) * 512], ps[:],
                            1.0 / (O_SCALE * WO_SCALE),
                            xq[:, half * 512: (half + 1) * 512],
                            op0=ALU.mult, op1=ALU.add)
                    ln_chunk(s, wkey, outT=outT, qc=qc)

                units.append(u)
            return units

        # ---------------- fusion MLP ----------------
        def fus1_units(n0):
            units = []
            for dt in range(DT):

                def u(dt=dt, n0=n0):
                    wt = f1pool.tile([P, 2 * DT, P], bf16, tag="f1")
                    nc.sync.dma_start(wt[:], w_d["f1"][dt])
                    ps = ps_mm.tile([P, 512], f32, tag="mm")
                    for kt in range(DT):
                        nc.tensor.matmul(
                            ps[:], wt[:, kt, :], fusedT["t"][:, kt, n0: n0 + 512],
                            start=(kt == 0), stop=False,
                        )
                    for kt in range(DT):
                        nc.tensor.matmul(
                            ps[:], wt[:, DT + kt, :],
                            fusedT["f"][:, kt, n0: n0 + 512],
                            start=False, stop=(kt == DT - 1),
                        )
                    nc.scalar.activation(
                        hT[:, dt, n0: n0 + 512], ps[:], AF.Gelu,
                        bias=bf1_col[:, dt: dt + 1],
                    )

                units.append(u)
            return units

        def fus2_ln_units(w2_sb):
            units = []
            for qc in range(QC):

                def u(qc=qc):
                    s = lns.tile([P, D], bf16, tag="lns")
                    for half in range(2):
                        ps = ps_mm.tile([P, 512], f32, tag="mm")
                        for dt in range(DT):
                            nc.tensor.matmul(
                                ps[:],
                                hT[:, dt, qc * P: (qc + 1) * P],
                                w2_sb[:, dt, half * 512: (half + 1) * 512],
                                start=(dt == 0), stop=(dt == DT - 1),
                            )
                        nc.vector.tensor_tensor(
                            s[:, half * 512: (half + 1) * 512], ps[:],
                            b2_bc[:, half * 512: (half + 1) * 512], op=ALU.add)
                    ln_chunk(s, "lnu", qc=qc, out_dram=out_d)

                units.append(u)
            return units

        def run_interleaved(primary, filler):
            k = 0
            for i, u in enumerate(primary):
                u()
                want = (i + 1) * len(filler) // len(primary)
                while k < want:
                    filler[k]()
                    k += 1
            while k < len(filler):
                filler[k]()
                k += 1

        def attn_stream(qt_sb, kd, vd, att_dst, order, pat):
            """flat quanta stream over units with fin deferred one unit."""
            stream = []
            prev_fin = None
            for qi, hp in order:
                qs, fin = attn_units(qt_sb, kd, vd, att_dst, qi, hp, pat)
                stream += qs[:2]
                if prev_fin is not None:
                    stream.append(prev_fin)
                stream += qs[2:]
                prev_fin = fin
            stream.append(prev_fin)
            return stream

        # ------------------------------------------------------------------
        # program
        # ------------------------------------------------------------------
        # Phase 1: Kf/Vf/g_f (full T from xfT) then Qt (xtT blocks 0-1)
        xf_load = [x_loader(xfT_d, bi) for bi in range(4)]
        xf_load[0]()
        w_kf = lw8("kf")
        w_vf = lw8("vf")
        vg_f = const.tile([P, DT, NH], fp8, name="vgf")
        nc.sync.dma_start(vg_f[:], vg_d["f"][:, :, :])
        vg_t = const.tile([P, DT, NH], fp8, name="vgt")
        nc.sync.dma_start(vg_t[:], vg_d["t"][:, :, :])
        for bi in range(4):
            n0 = bi * 512
            gx = xf_load[bi]
            ku = qk_proj_units(w_kf, gx, n0, k_sink(k_dr["f"], n0))
            gu = g_units(vg_f, gx, n0, g_sb["f"])
            vu = v_units(w_vf, gx, n0, g_sb["f"], v_dr["f"])
            run_interleaved(ku, gu + vu)
        w_qt = lw8("qt")
        xt_load = [x_loader(xtT_d, bi) for bi in range(4)]
        # hg-major Qt emission: heads 0-3 (hg0) complete after 4 units, at
        # which point attention-1's first units can start
        qt_byblk = [qk_proj_units(w_qt, xt_load[bi], bi * 512,
                                  q_sink(qT["t"], bi * 512)) for bi in range(2)]
        qt_units = []
        for j in range(8):
            qt_units += [qt_byblk[0][j], qt_byblk[1][j]]

        # Phase 2: attn-1 || Kt/Vt/g_t + Qf
        # blocks 0-1 of xtT reuse phase-1 cached tiles (their readers are all
        # emitted before the xs slot cycles again); blocks 2-3 and the Qf xf
        # re-reads get fresh loaders.
        w_kt = lw8("kt")
        w_vt = lw8("vt")
        xt_load2 = [xt_load[0], xt_load[1],
                    x_loader(xtT_d, 2), x_loader(xtT_d, 3)]
        xf_load2 = [x_loader(xfT_d, 0), x_loader(xfT_d, 1)]
        fillers = []
        for bi in range(4):
            n0 = bi * 512
            gx = xt_load2[bi]
            fillers += qk_proj_units(w_kt, gx, n0, k_sink(k_dr["t"], n0))
            fillers += g_units(vg_t, gx, n0, g_sb["t"])
            fillers += v_units(w_vt, gx, n0, g_sb["t"], v_dr["t"])
        w_qf = lw8("qf")
        for bi in range(2):
            n0 = bi * 512
            fillers += qk_proj_units(w_qf, xf_load2[bi], n0, q_sink(qT["f"], n0))
        for u in qt_units[:4]:
            u()
        prim1 = attn_stream(qT["t"], k_dr["f"], v_dr["f"], attnT["t"],
                            [(qi, hp) for qi in range(2) for hp in range(8)],
                            EXP_PATTERN1)
        # issue unit 0's kS/vS DMAs before the interleave so the first QK
        # isn't waiting on the load latency
        prim1[0]()
        run_interleaved(prim1[1:], qt_units[4:] + fillers)

        # Phase 3: attn-2 || O-proj(t)+LN_t, then late: oproj_f qt0 + fus blk0
        w_ot = lw8("ot")
        w_of = lw8("of")
        w_f2 = w2pool.tile([P, DT, D], bf16, tag="w16")
        nc.sync.dma_start(w_f2[:], w_d["f2"][:, :, :])
        oln_t = oproj_ln_units(attnT["t"], w_ot, xtq_d, "lnt", fusedT["t"])
        oln_f = oproj_ln_units(attnT["f"], w_of, xfq_d, "lnf", fusedT["f"])
        f1_0 = fus1_units(0)
        f1_1 = fus1_units(512)
        f2u = fus2_ln_units(w_f2)

        def blob(us):
            def u():
                for x in us:
                    x()

            return u

        # attn-2 processes qt1 FIRST so the qt1 half of the fusion pipeline
        # (oproj_f qc4-7, fus1 blk1, fus2 qc4-7) overlaps the qt0 attention
        # units; only qt0's short chain remains as the tail. f1 gelu blobs
        # keep the ACT table set from thrashing mid-attention.
        noop = lambda: None
        prim2 = attn_stream(qT["f"], k_dr["t"], v_dr["t"], attnT["f"],
                            [(qi, hp) for qi in (1, 0) for hp in range(8)],
                            EXP_PATTERN2)
        half = len(prim2) // 2
        run_interleaved(prim2[:half], list(oln_t))
        run_interleaved(prim2[half:],
                        list(oln_f[4:]) + [noop, blob(f1_1), noop,
                                           blob(f2u[4:6]), blob(f2u[6:8]),
                                           noop])

        # Phase 4 tail: qt0's chain
        for u in oln_f[:4]:
            u()
        for u in f1_0:
            u()
        for u in f2u[:4]:
            u()

    nc.compile()
    return nc


# ---------------------------------------------------------------------------
# host side
# ---------------------------------------------------------------------------
_CACHE = {}


def _get_nc(ln_trivial=True):
    key = f"nc{ln_trivial}"
    if key not in _CACHE:
        _CACHE[key] = _build_nc(ln_trivial)
    return _CACHE[key]


def _qk_perm():
    idx = np.empty(D, np.int64)
    for tile in range(DT):
        hg, dh = tile // 2, tile % 2
        p = np.arange(P)
        head = 4 * hg + p // 32
        d = 32 * dh + p % 32
        idx[tile * P: (tile + 1) * P] = 64 * head + d
    return idx


def _make_in_maps(inputs):
    import ml_dtypes

    F8 = ml_dtypes.float8_e4m3fn

    def wshuf(w, dt_):
        w = np.asarray(w, np.float32)
        nkt = w.shape[0] // P
        return np.ascontiguousarray(
            w.reshape(nkt, P, w.shape[1]).transpose(1, 0, 2)).astype(dt_)

    t = np.asarray(inputs["temporal_tokens"], np.float32)
    f = np.asarray(inputs["feature_tokens"], np.float32)
    perm = _qk_perm()

    shared = {}
    for n in ["qt", "kf", "qf", "kt"]:
        shared[f"w_{n}"] = wshuf(np.asarray(inputs[f"{n}_w"], np.float32)[:, perm], F8)
    for n in ["vf", "vt"]:
        shared[f"w_{n}"] = wshuf(inputs[f"{n}_w"], F8)
    for n in ["ot", "of"]:
        shared[f"w_{n}"] = wshuf(np.asarray(inputs[f"{n}_w"], np.float32) * WO_SCALE, F8)
    f1 = np.asarray(inputs["fus1_w"], np.float32)  # [2D, D]
    # [dt, 128(din-part), 2DT(kt), 128(dout)] per dout-tile
    f1r = f1.reshape(2 * DT, P, DT, P).transpose(2, 1, 0, 3)
    shared["w_f1"] = np.ascontiguousarray(f1r).astype(ml_dtypes.bfloat16)
    shared["w_f2"] = wshuf(inputs["fus2_w"], ml_dtypes.bfloat16)
    kfw = np.asarray(inputs["kf_w"], np.float32)
    ktw = np.asarray(inputs["kt_w"], np.float32)
    qtb = np.asarray(inputs["qt_b"], np.float32)
    qfb = np.asarray(inputs["qf_b"], np.float32)
    vgf = np.stack([kfw[:, 64 * h: 64 * h + 64] @ qtb[64 * h: 64 * h + 64]
                    for h in range(NH)], axis=1) * VG_SCALE
    vgt = np.stack([ktw[:, 64 * h: 64 * h + 64] @ qfb[64 * h: 64 * h + 64]
                    for h in range(NH)], axis=1) * VG_SCALE
    shared["vg_f"] = wshuf(vgf, F8)
    shared["vg_t"] = wshuf(vgt, F8)
    shared["b_f1"] = np.ascontiguousarray(
        np.asarray(inputs["fus1_b"], np.float32).reshape(DT, P).T)
    shared["b2row"] = np.ascontiguousarray(
        np.asarray(inputs["fus2_b"], np.float32).reshape(1, D))

    ln_trivial = all(
        np.all(np.asarray(inputs[k + "_w"]) == 1) and
        np.all(np.asarray(inputs[k + "_b"]) == 0)
        for k in ["ln_t", "ln_f", "ln_fus"])
    if not ln_trivial:
        for src, dst in [("ln_t_w", "lnt_w"), ("ln_t_b", "lnt_b"),
                         ("ln_f_w", "lnf_w"), ("ln_f_b", "lnf_b"),
                         ("ln_fus_w", "lnu_w"), ("ln_fus_b", "lnu_b")]:
            shared[dst] = np.ascontiguousarray(
                np.asarray(inputs[src], np.float32).reshape(1, D))

    rt = (np.asarray(inputs["ot_b"], np.float32)
          + np.asarray(inputs["vf_b"], np.float32) @ np.asarray(inputs["ot_w"], np.float32))
    rf = (np.asarray(inputs["of_b"], np.float32)
          + np.asarray(inputs["vt_b"], np.float32) @ np.asarray(inputs["of_w"], np.float32))

    def xshuf(xT):
        return np.ascontiguousarray(
            xT.reshape(DT, P, T // 512, 512).transpose(2, 1, 0, 3)).astype(F8)

    in_maps = []
    for c in range(8):
        b, half = divmod(c, 2)
        r0 = half * TQ
        xt, xf = t[b], f[b]
        pr = np.concatenate([np.arange(r0, T), np.arange(0, r0)])
        m = dict(shared)
        m["xtT"] = xshuf(xt[pr].T)
        m["xfT"] = xshuf(xf[pr].T)
        m["xtq"] = np.ascontiguousarray(xt[r0: r0 + TQ] + rt)
        m["xfq"] = np.ascontiguousarray(xf[r0: r0 + TQ] + rf)
        in_maps.append(m)
    return in_maps, ln_trivial


def kernel(**inputs):
    try:
        import jax

        jax.config.update("jax_compilation_cache_dir", "/tmp/jaxcache")
        jax.config.update("jax_persistent_cache_min_entry_size_bytes", -1)
        jax.config.update("jax_persistent_cache_min_compile_time_secs", 0.0)
    except Exception:
        pass
    from concourse.bass_utils import run_bass_kernel_spmd

    in_maps, ln_trivial = _make_in_maps(inputs)
    nc = _get_nc(ln_trivial)
    res = run_bass_kernel_spmd(nc, in_maps, list(range(8)))
    out = np.empty((4, T, D), np.float32)
    for c in range(8):
        b, half = divmod(c, 2)
        out[b, half * TQ: (half + 1) * TQ] = res.results[c]["out"]
    return out


# revision 3
# speedup vs baseline: 1.0115x; 1.0025x over previous
"""CoAttentionFusion TRN2 kernel v2 (8 cores SPMD, fp8 DoubleRow + 2-engine exp).

Per core c: batch b=c//2, query-half h=c%2 (1024 q rows); K/V over full T=2048
recomputed per pair (collectives cost more than the 109us of PE they save).

Key techniques vs the bf16 baseline:
- All attention-path matmuls in fp8e4m3 with DoubleRow perf mode (2 k-tiles
  per instruction, 0.5 cycles/row): K/V/Q projections, QK^T (2x32 contraction
  pairs), P@V (P^T stationary -> token-major O), O-projection.
- Q/K weights column-permuted on host so each head's 64 dims land as
  [32 partitions x 2 halves] for the DoubleRow QK layout.
- Q/K biases removed from the matmuls: K-bias is softmax-invariant; Q-bias
  becomes a per-key factor g_k = exp((x_kv @ (Wk@bq))/8) folded into V' rows
  and the denominator column of V'.
- O-proj bias and V-bias@W_o folded into the f32 residual on host.
- exp split between ACT (true exp->fp8) and DVE (Schraudolph uint8 bit-trick
  -> fp8e4m3) per EXP_PATTERN; probabilities consumed as fp8.
- Token-major O-proj output feeds LayerNorm directly (no LN in-transpose);
  LN rstd via Newton rsqrt on DVE (avoids ACT table thrashing with exp).
- fusion MLP stays bf16 (fp8 there fails the tolerance).
"""

import numpy as np

P = 128
D = 1024
T = 2048
TQ = 1024
NH = 16
HD = 64
DT = 8
KT = 16
QC = 8
EPS = 1e-5
LOG2E = 1.4426950408889634
O_SCALE = 32.0
WO_SCALE = 16.0
VG_SCALE = 64.0

# exp engine per (g8, head) slot within a unit: 'A' = ACT true exp,
# 'D' = DVE Schraudolph. Alternating keeps both engines fed. attn-1 runs
# with the projection sinks on ACT (9A/7D); attn-2 has the LN work on DVE
# (11A/5D).
EXP_PATTERN1 = "ADADADAADADADADA"
EXP_PATTERN2 = "ADAADADAADAADAAA" "ADAADADAADAADAAD"

_WQK = ["qt", "kf", "qf", "kt"]


def _build_nc(ln_trivial):
    import concourse.bass as bass
    import concourse.tile as tile
    from concourse import bacc, mybir
    from concourse.masks import make_identity
    from contextlib import ExitStack

    f32 = mybir.dt.float32
    bf16 = mybir.dt.bfloat16
    fp8 = mybir.dt.float8e4
    u8 = mybir.dt.uint8
    i32 = mybir.dt.int32
    AF = mybir.ActivationFunctionType
    ALU = mybir.AluOpType
    DR = mybir.MatmulPerfMode.DoubleRow

    nc = bacc.Bacc("TRN2", target_bir_lowering=False, debug=False, num_devices=8)

    # ---------------- DRAM I/O ----------------
    xtT_d = nc.dram_tensor("xtT", [T // 512, P, DT, 512], fp8, kind="ExternalInput")
    xfT_d = nc.dram_tensor("xfT", [T // 512, P, DT, 512], fp8, kind="ExternalInput")
    xtq_d = nc.dram_tensor("xtq", [TQ, D], f32, kind="ExternalInput")
    xfq_d = nc.dram_tensor("xfq", [TQ, D], f32, kind="ExternalInput")
    w_d = {}
    for n in ["qt", "kf", "vf", "qf", "kt", "vt", "ot", "of"]:
        w_d[n] = nc.dram_tensor(f"w_{n}", [P, DT, D], fp8, kind="ExternalInput")
    w_d["f1"] = nc.dram_tensor("w_f1", [DT, P, 2 * DT, P], bf16, kind="ExternalInput")
    w_d["f2"] = nc.dram_tensor("w_f2", [P, DT, D], bf16, kind="ExternalInput")
    vg_d = {"f": nc.dram_tensor("vg_f", [P, DT, NH], fp8, kind="ExternalInput"),
            "t": nc.dram_tensor("vg_t", [P, DT, NH], fp8, kind="ExternalInput")}
    bf1_d = nc.dram_tensor("b_f1", [P, DT], f32, kind="ExternalInput")
    b2_d = nc.dram_tensor("b2row", [1, D], f32, kind="ExternalInput")
    ln_d = {}
    if not ln_trivial:
        for n in ["lnt_w", "lnt_b", "lnf_w", "lnf_b", "lnu_w", "lnu_b"]:
            ln_d[n] = nc.dram_tensor(n, [1, D], f32, kind="ExternalInput")
    out_d = nc.dram_tensor("out", [TQ, D], f32, kind="ExternalOutput")

    with tile.TileContext(nc) as tc, ExitStack() as ctx:
        const = ctx.enter_context(tc.tile_pool(name="const", bufs=1))
        res = ctx.enter_context(tc.tile_pool(name="res", bufs=1))
        wpool = ctx.enter_context(tc.tile_pool(name="wpool", bufs=2))
        w2pool = ctx.enter_context(tc.tile_pool(name="w2pool", bufs=1))
        f1pool = ctx.enter_context(tc.tile_pool(name="f1pool", bufs=2))
        xs = ctx.enter_context(tc.tile_pool(name="xs", bufs=3))
        kvp = ctx.enter_context(tc.tile_pool(name="kvp", bufs=2))
        vsp = ctx.enter_context(tc.tile_pool(name="vsp", bufs=2))
        ppool = ctx.enter_context(tc.tile_pool(name="ppool", bufs=2))
        otokp = ctx.enter_context(tc.tile_pool(name="otokp", bufs=2))
        gpool = ctx.enter_context(tc.tile_pool(name="gpool", bufs=1))
        stg = ctx.enter_context(tc.tile_pool(name="stg", bufs=4))
        lns = ctx.enter_context(tc.tile_pool(name="lns", bufs=3))
        lsc = ctx.enter_context(tc.tile_pool(name="lsc", bufs=4))
        rowp = ctx.enter_context(tc.tile_pool(name="rowp", bufs=1))
        rsd = ctx.enter_context(tc.tile_pool(name="rsd", bufs=2))
        outp = ctx.enter_context(tc.tile_pool(name="outp", bufs=1))
        dram = ctx.enter_context(tc.tile_pool(name="dram", bufs=1, space="DRAM"))
        ps_qk = ctx.enter_context(tc.tile_pool(name="ps_qk", bufs=2, space="PSUM"))
        ps_pv = ctx.enter_context(tc.tile_pool(name="ps_pv", bufs=2, space="PSUM"))
        ps_mm = ctx.enter_context(tc.tile_pool(name="ps_mm", bufs=2, space="PSUM"))

        ident16 = const.tile([P, P], bf16, name="ident16")
        make_identity(nc, ident16[:])
        eps_t = const.tile([P, 1], f32, name="eps")
        nc.gpsimd.memset(eps_t[:], EPS)
        magic = const.tile([P, 1], i32, name="magic")
        nc.gpsimd.memset(magic[:], 0x5F3759DF)
        one_i = const.tile([P, 1], i32, name="one_i")
        nc.gpsimd.memset(one_i[:], 1)

        def row_bcast(dram_t, tag, dt_=f32):
            r = rowp.tile([1, D], f32, tag="row")
            nc.sync.dma_start(r[:], dram_t)
            if dt_ is not f32:
                rr = rowp.tile([1, D], dt_, tag="rowc")
                nc.vector.tensor_copy(rr[:], r[:])
                r = rr
            b = const.tile([P, D], dt_, name=tag)
            nc.gpsimd.partition_broadcast(b[:], r[:])
            return b

        b2_bc = row_bcast(b2_d[:, :], "b2bc")
        ln_bc = {}
        if not ln_trivial:
            for n in ["lnt_w", "lnt_b", "lnf_w", "lnf_b", "lnu_w", "lnu_b"]:
                ln_bc[n] = row_bcast(ln_d[n][:, :], n)
        bf1_col = const.tile([P, DT], f32, name="bf1")
        nc.sync.dma_start(bf1_col[:], bf1_d[:, :])

        def lw8(name):
            # two DMAs: first half unblocks the first matmuls, and each DMA
            # costs ~625ns of HWDGE issue time so fewer is better
            t = wpool.tile([P, DT, D], fp8, tag="w8")
            nc.sync.dma_start(t[:, 0:4, :], w_d[name][:, 0:4, :])
            nc.sync.dma_start(t[:, 4:8, :], w_d[name][:, 4:8, :])
            return t

        # DRAM scratch
        k_dr = {"f": dram.tile([4, P, 2, T], fp8, name="kf_dr"),
                "t": dram.tile([4, P, 2, T], fp8, name="kt_dr")}
        v_dr = {"f": dram.tile([NH, P, KT, HD + 1], fp8, name="vf_dr"),
                "t": dram.tile([NH, P, KT, HD + 1], fp8, name="vt_dr")}

        # resident activations
        qT = {"t": res.tile([P, 4, 2, TQ], fp8, name="qT_t"),
              "f": res.tile([P, 4, 2, TQ], fp8, name="qT_f")}
        attnT = {"t": res.tile([P, DT, TQ], fp8, name="attnT_t"),
                 "f": res.tile([P, DT, TQ], fp8, name="attnT_f")}
        fusedT = {"t": res.tile([P, DT, TQ], bf16, name="fusedT_t"),
                  "f": res.tile([P, DT, TQ], bf16, name="fusedT_f")}
        hT = res.tile([P, DT, TQ], bf16, name="hT")
        g_sb = {"f": gpool.tile([P, KT, NH], f32, name="g_f"),
                "t": gpool.tile([P, KT, NH], f32, name="g_t")}

        def x_loader(x_dram, bi):
            blk = {}

            def get():
                if "x" not in blk:
                    xb = xs.tile([P, DT, 512], fp8, tag="xs")
                    nc.sync.dma_start(xb[:], x_dram[bi])
                    blk["x"] = xb
                return blk["x"]

            return get

        # ---------------- unit builders ----------------
        def qk_proj_units(w_sb, get_x, n0, sink):
            units = []
            for hg in range(4):
                for dh in range(2):

                    def u(hg=hg, dh=dh):
                        ps = ps_mm.tile([P, 512], f32, tag="mm")
                        xb = get_x()
                        cs = (2 * hg + dh) * P
                        for t4 in range(4):
                            nc.tensor.matmul(
                                ps[:],
                                w_sb[:, 2 * t4: 2 * t4 + 2, cs: cs + P],
                                xb[:, 2 * t4: 2 * t4 + 2, :],
                                start=(t4 == 0), stop=(t4 == 3), perf_mode=DR,
                            )
                        sink(hg, dh, ps)

                    units.append(u)
            return units

        def k_sink(kd, n0):
            # batch the two dh halves of one hg into a single DMA (each DMA
            # costs ~625ns of HWDGE issue time)
            stage = {}

            def sink(hg, dh, ps):
                if hg not in stage:
                    stage[hg] = stg.tile([P, 2, 512], fp8, tag="k8", name="k8")
                nc.scalar.activation(stage[hg][:, dh, :], ps[:], AF.Identity)
                if dh == 1:
                    nc.sync.dma_start(kd[hg][:, :, n0: n0 + 512], stage[hg][:])

            return sink

        def q_sink(qdst, n0):
            # DVE copy: attention phases are ACT-walled, startup is balanced
            def sink(hg, dh, ps):
                nc.vector.tensor_copy(qdst[:, hg, dh, n0: n0 + 512], ps[:])

            return sink

        def g_units(vg_sb, get_x, n0, g_t):
            units = []
            for tci in range(4):

                def u(tci=tci):
                    ps = ps_mm.tile([P, 512], f32, tag="mm")
                    xb = get_x()
                    for t4 in range(4):
                        nc.tensor.matmul(
                            ps[:, 0:NH],
                            xb[:, 2 * t4: 2 * t4 + 2, tci * P: (tci + 1) * P],
                            vg_sb[:, 2 * t4: 2 * t4 + 2, :],
                            start=(t4 == 0), stop=(t4 == 3), perf_mode=DR,
                        )
                    kti = (n0 + tci * P) // P
                    nc.scalar.activation(g_t[:, kti, :], ps[:, 0:NH], AF.Exp,
                                         scale=1.0 / (8.0 * VG_SCALE))

                units.append(u)
            return units

        def v_units(w_sb, get_x, n0, g_t, vd):
            units = []
            vstage = {}
            for tci in range(4):
                for half in range(2):

                    def u(tci=tci, half=half):
                        ps = ps_mm.tile([P, 512], f32, tag="mm")
                        xb = get_x()
                        for t4 in range(4):
                            nc.tensor.matmul(
                                ps[:],
                                xb[:, 2 * t4: 2 * t4 + 2, tci * P: (tci + 1) * P],
                                w_sb[:, 2 * t4: 2 * t4 + 2,
                                     half * 512: (half + 1) * 512],
                                start=(t4 == 0), stop=(t4 == 3), perf_mode=DR,
                            )
                        kti = (n0 + tci * P) // P
                        if tci not in vstage:
                            vstage[tci] = stg.tile([P, NH, HD + 1], fp8,
                                                   tag="v8", name="v8")
                        s = vstage[tci]
                        gb = g_t[:, kti, half * 8:(half + 1) * 8]
                        nc.vector.tensor_tensor(
                            s[:, half * 8: (half + 1) * 8, 0:HD],
                            ps[:].rearrange("p (h e) -> p h e", h=8),
                            gb.unsqueeze(-1).broadcast_to([P, 8, HD]),
                            op=ALU.mult,
                        )
                        nc.vector.tensor_copy(
                            s[:, half * 8: (half + 1) * 8, HD: HD + 1],
                            gb.unsqueeze(-1))
                        if half == 1:
                            nc.sync.dma_start(
                                vd.rearrange("h p kt e -> p h kt e")[:, :, kti, :],
                                s[:],
                            )

                    units.append(u)
            return units

        # ---------------- attention ----------------
        exp_ctr = [0]

        def attn_units(qt_sb, kd, vd, att_dst, qi, hp, pat):
            """returns list of quanta closures for unit (qi, hp)."""
            state = {}
            h0, h1 = 2 * hp, 2 * hp + 1
            hg = hp // 2

            def get_ks():
                if "ks" not in state:
                    ks = kvp.tile([P, 2, T], fp8, tag="ks")
                    nc.sync.dma_start(ks[:], kd[hg])
                    state["ks"] = ks
                return state["ks"]

            def get_vs(hi):
                if "vs" not in state:
                    v = vsp.tile([P, 2, KT, HD + 1], fp8, tag="vs", name="vs")
                    nc.sync.dma_start(
                        v[:], vd.rearrange("h p kt e -> p h kt e")
                        [:, 2 * hp: 2 * hp + 2, :, :])
                    state["vs"] = v
                return state["vs"][:, hi]

            def get_pt(hi):
                key = f"pt{hi}"
                if key not in state:
                    state[key] = ppool.tile([P, 8, 2, 512], fp8, tag="pt",
                                            name=f"pt{hi}")
                return state[key]

            def qk_quantum(hi, g8lo, g8hi):
                def u():
                    ks = get_ks()
                    h = 2 * hp + hi
                    base = 32 * (h % 4)
                    for g8 in range(g8lo, g8hi):
                        ps = ps_qk.tile([P, 2, 512], f32, tag="qk")
                        for j in range(2):
                            kt = 2 * g8 + j
                            nc.tensor.matmul(
                                ps[:, j, :],
                                ks[base: base + 32, :, kt * P: (kt + 1) * P],
                                qt_sb[base: base + 32, h // 4, :,
                                      qi * 512: (qi + 1) * 512],
                                start=True, stop=True, perf_mode=DR,
                                tile_position=(base, 0),
                            )
                        pt = get_pt(hi)
                        eng = pat[exp_ctr[0] % len(pat)]
                        exp_ctr[0] += 1
                        if eng == "A":
                            nc.scalar.activation(pt[:, g8, :, :], ps[:],
                                                 AF.Exp, scale=0.125)
                        else:
                            nc.vector.tensor_scalar(
                                pt[:, g8, :, :].bitcast(u8), ps[:],
                                LOG2E, 56.0, op0=ALU.mult, op1=ALU.add)

                return u

            def pv_quantum(hi):
                def u():
                    vs = get_vs(hi)
                    pt = get_pt(hi)
                    otok = state["otok"]
                    for qc in range(4):
                        ps = ps_pv.tile([P, 512], f32, tag="pv")
                        for g8 in range(8):
                            nc.tensor.matmul(
                                ps[:, 0: HD + 1],
                                pt[:, g8, :, qc * P: (qc + 1) * P],
                                vs[:, 2 * g8: 2 * g8 + 2, :],
                                start=(g8 == 0), stop=(g8 == 7), perf_mode=DR,
                            )
                        inv = lsc.tile([P, 1], f32, tag="inv")
                        nc.vector.reciprocal(inv[:], ps[:, HD: HD + 1])
                        nc.vector.tensor_scalar(
                            otok[:, qc, hi * HD: (hi + 1) * HD],
                            ps[:, 0:HD], inv[:], O_SCALE,
                            op0=ALU.mult, op1=ALU.mult)

                return u

            def fin_quantum():
                def u():
                    otok = state["otok"]
                    tr = ps_mm.tile([P, 4, P], bf16, tag="mm", name="otr")
                    for qc in range(4):
                        nc.tensor.transpose(tr[:, qc, :], otok[:, qc, :],
                                            ident16[:])
                    nc.scalar.activation(
                        att_dst[:, hp, qi * 512: (qi + 1) * 512]
                        .rearrange("p (a b) -> p a b", a=4),
                        tr[:], AF.Identity,
                    )

                return u

            def start_quantum():
                def u():
                    state["otok"] = otokp.tile([P, 4, P], bf16, tag="otok",
                                               name="otok")
                    get_ks()
                    get_vs(0)
                    get_vs(1)

                return u

            # head-major: PV of head 0 overlaps head 1's exps, halving the
            # exp->PV convoy on the DVE queue and freeing pt slots earlier.
            # fin is returned separately so the caller can defer it one unit
            # (its deps are stale by then -> no ACT-queue stall).
            return ([start_quantum(), qk_quantum(0, 0, 4), qk_quantum(0, 4, 8),
                     pv_quantum(0), qk_quantum(1, 0, 4), qk_quantum(1, 4, 8),
                     pv_quantum(1)], fin_quantum())

        # ---------------- O-proj + LN ----------------
        def newton_rstd(var_ap):
            """rstd [P,1] f32 from var (+eps) via bit-trick + 2 Newton steps."""
            a = lsc.tile([P, 1], f32, tag="nva")
            nc.vector.tensor_scalar_add(a[:], var_ap, eps_t[:])
            y = lsc.tile([P, 1], f32, tag="nvy")
            nc.vector.tensor_scalar(y[:].bitcast(i32), a[:].bitcast(i32),
                                    one_i[:], None,
                                    op0=ALU.logical_shift_right)
            nc.vector.tensor_tensor(y[:].bitcast(i32), magic[:],
                                    y[:].bitcast(i32), op=ALU.subtract)
            uu = lsc.tile([P, 1], f32, tag="nvu")
            # one Newton step: ~0.17% max rel error on rstd, well inside
            # the tolerance; a second step would double the serial DVE chain
            nc.vector.tensor_tensor(uu[:], y[:], y[:], op=ALU.mult)
            nc.vector.tensor_tensor(uu[:], uu[:], a[:], op=ALU.mult)
            nc.vector.tensor_scalar(uu[:], uu[:], -0.5, 1.5,
                                    op0=ALU.mult, op1=ALU.add)
            nc.vector.tensor_tensor(y[:], y[:], uu[:], op=ALU.mult)
            return y

        def ln_chunk(s, wkey, outT=None, qc=None, out_dram=None):
            """stats+normalize s [P,D] bf16; write transposed to outT or
            f32 rows to out_dram."""
            bns = lsc.tile([P, 2, 6], f32, tag="bns")
            nc.vector.bn_stats(bns[:, 0, :], s[:, 0:512])
            nc.vector.bn_stats(bns[:, 1, :], s[:, 512:D])
            mv = lsc.tile([P, 2], f32, tag="mv")
            nc.vector.bn_aggr(mv[:], bns[:])
            rstd = newton_rstd(mv[:, 1:2])
            if out_dram is not None:
                o = outp.tile([P, D], f32, tag="out")
                nc.vector.tensor_scalar(o[:], s[:], mv[:, 0:1], rstd[:],
                                        op0=ALU.subtract, op1=ALU.mult)
                if not ln_trivial:
                    nc.vector.tensor_tensor(o[:], o[:], ln_bc[wkey + "_w"][:],
                                            op=ALU.mult)
                    nc.vector.tensor_tensor(o[:], o[:], ln_bc[wkey + "_b"][:],
                                            op=ALU.add)
                nc.sync.dma_start(out_dram[qc * P: (qc + 1) * P, :], o[:])
            else:
                nrm = lns.tile([P, D], bf16, tag="nrm")
                nc.vector.tensor_scalar(nrm[:], s[:], mv[:, 0:1], rstd[:],
                                        op0=ALU.subtract, op1=ALU.mult)
                if not ln_trivial:
                    nc.vector.tensor_tensor(nrm[:], nrm[:], ln_bc[wkey + "_w"][:],
                                            op=ALU.mult)
                    nc.vector.tensor_tensor(nrm[:], nrm[:], ln_bc[wkey + "_b"][:],
                                            op=ALU.add)
                tr = ps_mm.tile([P, D], bf16, tag="mm", name="lntr")
                for dt in range(DT):
                    nc.tensor.transpose(tr[:, dt * P: (dt + 1) * P],
                                        nrm[:, dt * P: (dt + 1) * P], ident16[:])
                nc.vector.tensor_copy(
                    outT[:, :, qc * P: (qc + 1) * P],
                    tr[:].rearrange("p (dt c) -> p dt c", dt=DT),
                )

        def oproj_ln_units(att_sb, w_sb, resid_dram, wkey, outT):
            units = []
            for qc in range(QC):

                def u(qc=qc):
                    xq = rsd.tile([P, D], f32, tag="xq")
                    nc.sync.dma_start(xq[:], resid_dram[qc * P: (qc + 1) * P, :])
                    s = lns.tile([P, D], bf16, tag="lns")
                    for half in range(2):
                        ps = ps_mm.tile([P, 512], f32, tag="mm")
                        for t4 in range(4):
                            nc.tensor.matmul(
                                ps[:],
                                att_sb[:, 2 * t4: 2 * t4 + 2, qc * P: (qc + 1) * P],
                                w_sb[:, 2 * t4: 2 * t4 + 2,
                                     half * 512: (half + 1) * 512],
                                start=(t4 == 0), stop=(t4 == 3), perf_mode=DR,
                            )
                        nc.vector.scalar_tensor_tensor(
                            s[:, half * 512: (half + 1) * 512], ps[:],
                            1.0 / (O_SCALE * WO_SCALE),
                            xq[:, half * 512: (half + 1) * 512],
                            op0=ALU.mult, op1=ALU.add)
                    ln_chunk(s, wkey, outT=outT, qc=qc)

                units.append(u)
            return units

        # ---------------- fusion MLP ----------------
        def fus1_units(n0):
            units = []
            for dt in range(DT):

                def u(dt=dt, n0=n0):
                    wt = f1pool.tile([P, 2 * DT, P], bf16, tag="f1")
                    nc.sync.dma_start(wt[:], w_d["f1"][dt])
                    ps = ps_mm.tile([P, 512], f32, tag="mm")
                    for kt in range(DT):
                        nc.tensor.matmul(
                            ps[:], wt[:, kt, :], fusedT["t"][:, kt, n0: n0 + 512],
                            start=(kt == 0), stop=False,
                        )
                    for kt in range(DT):
                        nc.tensor.matmul(
                            ps[:], wt[:, DT + kt, :],
                            fusedT["f"][:, kt, n0: n0 + 512],
                            start=False, stop=(kt == DT - 1),
                        )
                    nc.scalar.activation(
                        hT[:, dt, n0: n0 + 512], ps[:], AF.Gelu,
                        bias=bf1_col[:, dt: dt + 1],
                    )

                units.append(u)
            return units

        def fus2_ln_units(w2_sb):
            units = []
            for qc in range(QC):

                def u(qc=qc):
                    s = lns.tile([P, D], bf16, tag="lns")
                    for half in range(2):
                        ps = ps_mm.tile([P, 512], f32, tag="mm")
                        for dt in range(DT):
                            nc.tensor.matmul(
                                ps[:],
                                hT[:, dt, qc * P: (qc + 1) * P],
                                w2_sb[:, dt, half * 512: (half + 1) * 512],
                                start=(dt == 0), stop=(dt == DT - 1),
                            )
                        nc.vector.tensor_tensor(
                            s[:, half * 512: (half + 1) * 512], ps[:],
                            b2_bc[:, half * 512: (half + 1) * 512], op=ALU.add)
                    ln_chunk(s, "lnu", qc=qc, out_dram=out_d)

                units.append(u)
            return units

        def run_interleaved(primary, filler):
            k = 0
            for i, u in enumerate(primary):
                u()
                want = (i + 1) * len(filler) // len(primary)
                while k < want:
                    filler[k]()
                    k += 1
            while k < len(filler):
                filler[k]()
                k += 1

        def attn_stream(qt_sb, kd, vd, att_dst, order, pat):
            """flat quanta stream over units with fin deferred one unit."""
            stream = []
            prev_fin = None
            for qi, hp in order:
                qs, fin = attn_units(qt_sb, kd, vd, att_dst, qi, hp, pat)
                stream += qs[:2]
                if prev_fin is not None:
                    stream.append(prev_fin)
                stream += qs[2:]
                prev_fin = fin
            stream.append(prev_fin)
            return stream

        # ------------------------------------------------------------------
        # program
        # ------------------------------------------------------------------
        # Phase 1: Kf/Vf/g_f (full T from xfT) then Qt (xtT blocks 0-1)
        xf_load = [x_loader(xfT_d, bi) for bi in range(4)]
        xf_load[0]()
        w_kf = lw8("kf")
        w_vf = lw8("vf")
        vg_f = const.tile([P, DT, NH], fp8, name="vgf")
        nc.sync.dma_start(vg_f[:], vg_d["f"][:, :, :])
        vg_t = const.tile([P, DT, NH], fp8, name="vgt")
        nc.sync.dma_start(vg_t[:], vg_d["t"][:, :, :])
        for bi in range(4):
            n0 = bi * 512
            gx = xf_load[bi]
            ku = qk_proj_units(w_kf, gx, n0, k_sink(k_dr["f"], n0))
            gu = g_units(vg_f, gx, n0, g_sb["f"])
            vu = v_units(w_vf, gx, n0, g_sb["f"], v_dr["f"])
            run_interleaved(ku, gu + vu)
        w_qt = lw8("qt")
        xt_load = [x_loader(xtT_d, bi) for bi in range(4)]
        # hg-major Qt emission: heads 0-3 (hg0) complete after 4 units, at
        # which point attention-1's first units can start
        qt_byblk = [qk_proj_units(w_qt, xt_load[bi], bi * 512,
                                  q_sink(qT["t"], bi * 512)) for bi in range(2)]
        qt_units = []
        for j in range(8):
            qt_units += [qt_byblk[0][j], qt_byblk[1][j]]
        for u in qt_units[:4]:
            u()

        # Phase 2: attn-1 || Kt/Vt/g_t + Qf
        # blocks 0-1 of xtT reuse phase-1 cached tiles (their readers are all
        # emitted before the xs slot cycles again); blocks 2-3 and the Qf xf
        # re-reads get fresh loaders.
        w_kt = lw8("kt")
        w_vt = lw8("vt")
        xt_load2 = [xt_load[0], xt_load[1],
                    x_loader(xtT_d, 2), x_loader(xtT_d, 3)]
        xf_load2 = [x_loader(xfT_d, 0), x_loader(xfT_d, 1)]
        fillers = []
        for bi in range(4):
            n0 = bi * 512
            gx = xt_load2[bi]
            fillers += qk_proj_units(w_kt, gx, n0, k_sink(k_dr["t"], n0))
            fillers += g_units(vg_t, gx, n0, g_sb["t"])
            fillers += v_units(w_vt, gx, n0, g_sb["t"], v_dr["t"])
        w_qf = lw8("qf")
        for bi in range(2):
            n0 = bi * 512
            fillers += qk_proj_units(w_qf, xf_load2[bi], n0, q_sink(qT["f"], n0))
        prim1 = attn_stream(qT["t"], k_dr["f"], v_dr["f"], attnT["t"],
                            [(qi, hp) for qi in range(2) for hp in range(8)],
                            EXP_PATTERN1)
        # issue unit 0's kS/vS DMAs before the interleave so the first QK
        # isn't waiting on the load latency
        prim1[0]()
        run_interleaved(prim1[1:], qt_units[4:] + fillers)

        # Phase 3: attn-2 || O-proj(t)+LN_t, then late: oproj_f qt0 + fus blk0
        w_ot = lw8("ot")
        w_of = lw8("of")
        w_f2 = w2pool.tile([P, DT, D], bf16, tag="w16")
        nc.sync.dma_start(w_f2[:], w_d["f2"][:, :, :])
        oln_t = oproj_ln_units(attnT["t"], w_ot, xtq_d, "lnt", fusedT["t"])
        oln_f = oproj_ln_units(attnT["f"], w_of, xfq_d, "lnf", fusedT["f"])
        f1_0 = fus1_units(0)
        f1_1 = fus1_units(512)
        f2u = fus2_ln_units(w_f2)

        def blob(us):
            def u():
                for x in us:
                    x()

            return u

        # attn-2 processes qt1 FIRST so the qt1 half of the fusion pipeline
        # (oproj_f qc4-7, fus1 blk1, fus2 qc4-7) overlaps the qt0 attention
        # units; only qt0's short chain remains as the tail. f1 gelu blobs
        # keep the ACT table set from thrashing mid-attention.
        noop = lambda: None
        prim2 = attn_stream(qT["f"], k_dr["t"], v_dr["t"], attnT["f"],
                            [(qi, hp) for qi in (1, 0) for hp in range(8)],
                            EXP_PATTERN2)
        half = len(prim2) // 2
        run_interleaved(prim2[:half], list(oln_t))
        run_interleaved(prim2[half:],
                        list(oln_f[4:]) + [noop, blob(f1_1), noop,
                                           blob(f2u[4:6]), blob(f2u[6:8]),
                                           noop])

        # Phase 4 tail: qt0's chain
        for u in oln_f[:4]:
            u()
        for u in f1_0:
            u()
        for u in f2u[:4]:
            u()

    nc.compile()
    return nc


# ---------------------------------------------------------------------------
# host side
# ---------------------------------------------------------------------------
_CACHE = {}


def _get_nc(ln_trivial=True):
    key = f"nc{ln_trivial}"
    if key not in _CACHE:
        _CACHE[key] = _build_nc(ln_trivial)
    return _CACHE[key]


def _qk_perm():
    idx = np.empty(D, np.int64)
    for tile in range(DT):
        hg, dh = tile // 2, tile % 2
        p = np.arange(P)
        head = 4 * hg + p // 32
        d = 32 * dh + p % 32
        idx[tile * P: (tile + 1) * P] = 64 * head + d
    return idx


def _make_in_maps(inputs):
    import ml_dtypes

    F8 = ml_dtypes.float8_e4m3fn

    def wshuf(w, dt_):
        w = np.asarray(w, np.float32)
        nkt = w.shape[0] // P
        return np.ascontiguousarray(
            w.reshape(nkt, P, w.shape[1]).transpose(1, 0, 2)).astype(dt_)

    t = np.asarray(inputs["temporal_tokens"], np.float32)
    f = np.asarray(inputs["feature_tokens"], np.float32)
    perm = _qk_perm()

    shared = {}
    for n in ["qt", "kf", "qf", "kt"]:
        shared[f"w_{n}"] = wshuf(np.asarray(inputs[f"{n}_w"], np.float32)[:, perm], F8)
    for n in ["vf", "vt"]:
        shared[f"w_{n}"] = wshuf(inputs[f"{n}_w"], F8)
    for n in ["ot", "of"]:
        shared[f"w_{n}"] = wshuf(np.asarray(inputs[f"{n}_w"], np.float32) * WO_SCALE, F8)
    f1 = np.asarray(inputs["fus1_w"], np.float32)  # [2D, D]
    # [dt, 128(din-part), 2DT(kt), 128(dout)] per dout-tile
    f1r = f1.reshape(2 * DT, P, DT, P).transpose(2, 1, 0, 3)
    shared["w_f1"] = np.ascontiguousarray(f1r).astype(ml_dtypes.bfloat16)
    shared["w_f2"] = wshuf(inputs["fus2_w"], ml_dtypes.bfloat16)
    kfw = np.asarray(inputs["kf_w"], np.float32)
    ktw = np.asarray(inputs["kt_w"], np.float32)
    qtb = np.asarray(inputs["qt_b"], np.float32)
    qfb = np.asarray(inputs["qf_b"], np.float32)
    vgf = np.stack([kfw[:, 64 * h: 64 * h + 64] @ qtb[64 * h: 64 * h + 64]
                    for h in range(NH)], axis=1) * VG_SCALE
    vgt = np.stack([ktw[:, 64 * h: 64 * h + 64] @ qfb[64 * h: 64 * h + 64]
                    for h in range(NH)], axis=1) * VG_SCALE
    shared["vg_f"] = wshuf(vgf, F8)
    shared["vg_t"] = wshuf(vgt, F8)
    shared["b_f1"] = np.ascontiguousarray(
        np.asarray(inputs["fus1_b"], np.float32).reshape(DT, P).T)
    shared["b2row"] = np.ascontiguousarray(
        np.asarray(inputs["fus2_b"], np.float32).reshape(1, D))

    ln_trivial = all(
        np.all(np.asarray(inputs[k + "_w"]) == 1) and
        np.all(np.asarray(inputs[k + "_b"]) == 0)
        for k in ["ln_t", "ln_f", "ln_fus"])
    if not ln_trivial:
        for src, dst in [("ln_t_w", "lnt_w"), ("ln_t_b", "lnt_b"),
                         ("ln_f_w", "lnf_w"), ("ln_f_b", "lnf_b"),
                         ("ln_fus_w", "lnu_w"), ("ln_fus_b", "lnu_b")]:
            shared[dst] = np.ascontiguousarray(
                np.asarray(inputs[src], np.float32).reshape(1, D))

    rt = (np.asarray(inputs["ot_b"], np.float32)
          + np.asarray(inputs["vf_b"], np.float32) @ np.asarray(inputs["ot_w"], np.float32))
    rf = (np.asarray(inputs["of_b"], np.float32)
          + np.asarray(inputs["vt_b"], np.float32) @ np.asarray(inputs["of_w"], np.float32))

    def xshuf(xT):
        return np.ascontiguousarray(
            xT.reshape(DT, P, T // 512, 512).transpose(2, 1, 0, 3)).astype(F8)

    in_maps = []
    for c in range(8):
        b, half = divmod(c, 2)
        r0 = half * TQ
        xt, xf = t[b], f[b]
        pr = np.concatenate([np.arange(r0, T), np.arange(0, r0)])
        m = dict(shared)
        m["xtT"] = xshuf(xt[pr].T)
        m["xfT"] = xshuf(xf[pr].T)
        m["xtq"] = np.ascontiguousarray(xt[r0: r0 + TQ] + rt)
        m["xfq"] = np.ascontiguousarray(xf[r0: r0 + TQ] + rf)
        in_maps.append(m)
    return in_maps, ln_trivial


def kernel(**inputs):
    try:
        import jax

        jax.config.update("jax_compilation_cache_dir", "/tmp/jaxcache")
        jax.config.update("jax_persistent_cache_min_entry_size_bytes", -1)
        jax.config.update("jax_persistent_cache_min_compile_time_secs", 0.0)
    except Exception:
        pass
    from concourse.bass_utils import run_bass_kernel_spmd

    in_maps, ln_trivial = _make_in_maps(inputs)
    nc = _get_nc(ln_trivial)
    res = run_bass_kernel_spmd(nc, in_maps, list(range(8)))
    out = np.empty((4, T, D), np.float32)
    for c in range(8):
        b, half = divmod(c, 2)
        out[b, half * TQ: (half + 1) * TQ] = res.results[c]["out"]
    return out


# revision 4
# speedup vs baseline: 1.0118x; 1.0003x over previous
"""CoAttentionFusion TRN2 kernel v2 (8 cores SPMD, fp8 DoubleRow + 2-engine exp).

Per core c: batch b=c//2, query-half h=c%2 (1024 q rows); K/V over full T=2048
recomputed per pair (collectives cost more than the 109us of PE they save).

Key techniques vs the bf16 baseline:
- All attention-path matmuls in fp8e4m3 with DoubleRow perf mode (2 k-tiles
  per instruction, 0.5 cycles/row): K/V/Q projections, QK^T (2x32 contraction
  pairs), P@V (P^T stationary -> token-major O), O-projection.
- Q/K weights column-permuted on host so each head's 64 dims land as
  [32 partitions x 2 halves] for the DoubleRow QK layout.
- Q/K biases removed from the matmuls: K-bias is softmax-invariant; Q-bias
  becomes a per-key factor g_k = exp((x_kv @ (Wk@bq))/8) folded into V' rows
  and the denominator column of V'.
- O-proj bias and V-bias@W_o folded into the f32 residual on host.
- exp split between ACT (true exp->fp8) and DVE (Schraudolph uint8 bit-trick
  -> fp8e4m3) per EXP_PATTERN; probabilities consumed as fp8.
- Token-major O-proj output feeds LayerNorm directly (no LN in-transpose);
  LN rstd via Newton rsqrt on DVE (avoids ACT table thrashing with exp).
- fusion MLP stays bf16 (fp8 there fails the tolerance).
"""

import numpy as np

P = 128
D = 1024
T = 2048
TQ = 1024
NH = 16
HD = 64
DT = 8
KT = 16
QC = 8
EPS = 1e-5
LOG2E = 1.4426950408889634
O_SCALE = 32.0
WO_SCALE = 16.0
VG_SCALE = 64.0

# exp engine per (g8, head) slot within a unit: 'A' = ACT true exp,
# 'D' = DVE Schraudolph. Alternating keeps both engines fed. attn-1 runs
# with the projection sinks on ACT (9A/7D); attn-2 has the LN work on DVE
# (11A/5D).
EXP_PATTERN1 = "ADADADAADADADADA"
EXP_PATTERN2 = "ADAADADAADAADAAA" "ADAADADAADAADAAD"

_WQK = ["qt", "kf", "qf", "kt"]


def _build_nc(ln_trivial):
    import concourse.bass as bass
    import concourse.tile as tile
    from concourse import bacc, mybir
    from concourse.masks import make_identity
    from contextlib import ExitStack

    f32 = mybir.dt.float32
    bf16 = mybir.dt.bfloat16
    fp8 = mybir.dt.float8e4
    u8 = mybir.dt.uint8
    i32 = mybir.dt.int32
    AF = mybir.ActivationFunctionType
    ALU = mybir.AluOpType
    DR = mybir.MatmulPerfMode.DoubleRow

    nc = bacc.Bacc("TRN2", target_bir_lowering=False, debug=False, num_devices=8)

    # ---------------- DRAM I/O ----------------
    xtT_d = nc.dram_tensor("xtT", [T // 512, P, DT, 512], fp8, kind="ExternalInput")
    xfT_d = nc.dram_tensor("xfT", [T // 512, P, DT, 512], fp8, kind="ExternalInput")
    xtq_d = nc.dram_tensor("xtq", [TQ, D], f32, kind="ExternalInput")
    xfq_d = nc.dram_tensor("xfq", [TQ, D], f32, kind="ExternalInput")
    w_d = {}
    for n in ["qt", "kf", "vf", "qf", "kt", "vt", "ot", "of"]:
        w_d[n] = nc.dram_tensor(f"w_{n}", [P, DT, D], fp8, kind="ExternalInput")
    w_d["f1"] = nc.dram_tensor("w_f1", [DT, P, 2 * DT, P], bf16, kind="ExternalInput")
    w_d["f2"] = nc.dram_tensor("w_f2", [P, DT, D], bf16, kind="ExternalInput")
    vg_d = {"f": nc.dram_tensor("vg_f", [P, DT, NH], fp8, kind="ExternalInput"),
            "t": nc.dram_tensor("vg_t", [P, DT, NH], fp8, kind="ExternalInput")}
    bf1_d = nc.dram_tensor("b_f1", [P, DT], f32, kind="ExternalInput")
    b2_d = nc.dram_tensor("b2row", [1, D], f32, kind="ExternalInput")
    ln_d = {}
    if not ln_trivial:
        for n in ["lnt_w", "lnt_b", "lnf_w", "lnf_b", "lnu_w", "lnu_b"]:
            ln_d[n] = nc.dram_tensor(n, [1, D], f32, kind="ExternalInput")
    out_d = nc.dram_tensor("out", [TQ, D], f32, kind="ExternalOutput")

    with tile.TileContext(nc) as tc, ExitStack() as ctx:
        const = ctx.enter_context(tc.tile_pool(name="const", bufs=1))
        res = ctx.enter_context(tc.tile_pool(name="res", bufs=1))
        wpool = ctx.enter_context(tc.tile_pool(name="wpool", bufs=2))
        w2pool = ctx.enter_context(tc.tile_pool(name="w2pool", bufs=1))
        f1pool = ctx.enter_context(tc.tile_pool(name="f1pool", bufs=2))
        xs = ctx.enter_context(tc.tile_pool(name="xs", bufs=3))
        kvp = ctx.enter_context(tc.tile_pool(name="kvp", bufs=2))
        vsp = ctx.enter_context(tc.tile_pool(name="vsp", bufs=2))
        ppool = ctx.enter_context(tc.tile_pool(name="ppool", bufs=2))
        otokp = ctx.enter_context(tc.tile_pool(name="otokp", bufs=2))
        gpool = ctx.enter_context(tc.tile_pool(name="gpool", bufs=1))
        stg = ctx.enter_context(tc.tile_pool(name="stg", bufs=4))
        lns = ctx.enter_context(tc.tile_pool(name="lns", bufs=3))
        lsc = ctx.enter_context(tc.tile_pool(name="lsc", bufs=4))
        rowp = ctx.enter_context(tc.tile_pool(name="rowp", bufs=1))
        rsd = ctx.enter_context(tc.tile_pool(name="rsd", bufs=2))
        outp = ctx.enter_context(tc.tile_pool(name="outp", bufs=1))
        dram = ctx.enter_context(tc.tile_pool(name="dram", bufs=1, space="DRAM"))
        ps_qk = ctx.enter_context(tc.tile_pool(name="ps_qk", bufs=2, space="PSUM"))
        ps_pv = ctx.enter_context(tc.tile_pool(name="ps_pv", bufs=2, space="PSUM"))
        ps_mm = ctx.enter_context(tc.tile_pool(name="ps_mm", bufs=2, space="PSUM"))

        ident16 = const.tile([P, P], bf16, name="ident16")
        make_identity(nc, ident16[:])
        eps_t = const.tile([P, 1], f32, name="eps")
        nc.gpsimd.memset(eps_t[:], EPS)
        magic = const.tile([P, 1], i32, name="magic")
        nc.gpsimd.memset(magic[:], 0x5F3759DF)
        one_i = const.tile([P, 1], i32, name="one_i")
        nc.gpsimd.memset(one_i[:], 1)

        def row_bcast(dram_t, tag, dt_=f32):
            r = rowp.tile([1, D], f32, tag="row")
            nc.sync.dma_start(r[:], dram_t)
            if dt_ is not f32:
                rr = rowp.tile([1, D], dt_, tag="rowc")
                nc.vector.tensor_copy(rr[:], r[:])
                r = rr
            b = const.tile([P, D], dt_, name=tag)
            nc.gpsimd.partition_broadcast(b[:], r[:])
            return b

        b2_bc = row_bcast(b2_d[:, :], "b2bc")
        ln_bc = {}
        if not ln_trivial:
            for n in ["lnt_w", "lnt_b", "lnf_w", "lnf_b", "lnu_w", "lnu_b"]:
                ln_bc[n] = row_bcast(ln_d[n][:, :], n)
        bf1_col = const.tile([P, DT], f32, name="bf1")
        nc.sync.dma_start(bf1_col[:], bf1_d[:, :])

        def lw8(name):
            # two DMAs: first half unblocks the first matmuls, and each DMA
            # costs ~625ns of HWDGE issue time so fewer is better
            t = wpool.tile([P, DT, D], fp8, tag="w8")
            nc.sync.dma_start(t[:, 0:4, :], w_d[name][:, 0:4, :])
            nc.sync.dma_start(t[:, 4:8, :], w_d[name][:, 4:8, :])
            return t

        # DRAM scratch
        k_dr = {"f": dram.tile([4, P, 2, T], fp8, name="kf_dr"),
                "t": dram.tile([4, P, 2, T], fp8, name="kt_dr")}
        v_dr = {"f": dram.tile([NH, P, KT, HD + 1], fp8, name="vf_dr"),
                "t": dram.tile([NH, P, KT, HD + 1], fp8, name="vt_dr")}

        # resident activations
        qT = {"t": res.tile([P, 4, 2, TQ], fp8, name="qT_t"),
              "f": res.tile([P, 4, 2, TQ], fp8, name="qT_f")}
        attnT = {"t": res.tile([P, DT, TQ], fp8, name="attnT_t"),
                 "f": res.tile([P, DT, TQ], fp8, name="attnT_f")}
        fusedT = {"t": res.tile([P, DT, TQ], bf16, name="fusedT_t"),
                  "f": res.tile([P, DT, TQ], bf16, name="fusedT_f")}
        hT = res.tile([P, DT, TQ], bf16, name="hT")
        g_sb = {"f": gpool.tile([P, KT, NH], f32, name="g_f"),
                "t": gpool.tile([P, KT, NH], f32, name="g_t")}

        def x_loader(x_dram, bi):
            blk = {}

            def get():
                if "x" not in blk:
                    xb = xs.tile([P, DT, 512], fp8, tag="xs")
                    nc.sync.dma_start(xb[:], x_dram[bi])
                    blk["x"] = xb
                return blk["x"]

            return get

        # ---------------- unit builders ----------------
        def qk_proj_units(w_sb, get_x, n0, sink):
            units = []
            for hg in range(4):
                for dh in range(2):

                    def u(hg=hg, dh=dh):
                        ps = ps_mm.tile([P, 512], f32, tag="mm")
                        xb = get_x()
                        cs = (2 * hg + dh) * P
                        for t4 in range(4):
                            nc.tensor.matmul(
                                ps[:],
                                w_sb[:, 2 * t4: 2 * t4 + 2, cs: cs + P],
                                xb[:, 2 * t4: 2 * t4 + 2, :],
                                start=(t4 == 0), stop=(t4 == 3), perf_mode=DR,
                            )
                        sink(hg, dh, ps)

                    units.append(u)
            return units

        def k_sink(kd, n0):
            # batch the two dh halves of one hg into a single DMA (each DMA
            # costs ~625ns of HWDGE issue time)
            stage = {}

            def sink(hg, dh, ps):
                if hg not in stage:
                    stage[hg] = stg.tile([P, 2, 512], fp8, tag="k8", name="k8")
                nc.scalar.activation(stage[hg][:, dh, :], ps[:], AF.Identity)
                if dh == 1:
                    nc.sync.dma_start(kd[hg][:, :, n0: n0 + 512], stage[hg][:])

            return sink

        def q_sink(qdst, n0):
            # DVE copy: attention phases are ACT-walled, startup is balanced
            def sink(hg, dh, ps):
                nc.vector.tensor_copy(qdst[:, hg, dh, n0: n0 + 512], ps[:])

            return sink

        def g_units(vg_sb, get_x, n0, g_t):
            units = []
            for tci in range(4):

                def u(tci=tci):
                    ps = ps_mm.tile([P, 512], f32, tag="mm")
                    xb = get_x()
                    for t4 in range(4):
                        nc.tensor.matmul(
                            ps[:, 0:NH],
                            xb[:, 2 * t4: 2 * t4 + 2, tci * P: (tci + 1) * P],
                            vg_sb[:, 2 * t4: 2 * t4 + 2, :],
                            start=(t4 == 0), stop=(t4 == 3), perf_mode=DR,
                        )
                    kti = (n0 + tci * P) // P
                    nc.scalar.activation(g_t[:, kti, :], ps[:, 0:NH], AF.Exp,
                                         scale=1.0 / (8.0 * VG_SCALE))

                units.append(u)
            return units

        def v_units(w_sb, get_x, n0, g_t, vd):
            units = []
            vstage = {}
            for tci in range(4):
                for half in range(2):

                    def u(tci=tci, half=half):
                        ps = ps_mm.tile([P, 512], f32, tag="mm")
                        xb = get_x()
                        for t4 in range(4):
                            nc.tensor.matmul(
                                ps[:],
                                xb[:, 2 * t4: 2 * t4 + 2, tci * P: (tci + 1) * P],
                                w_sb[:, 2 * t4: 2 * t4 + 2,
                                     half * 512: (half + 1) * 512],
                                start=(t4 == 0), stop=(t4 == 3), perf_mode=DR,
                            )
                        kti = (n0 + tci * P) // P
                        if tci not in vstage:
                            vstage[tci] = stg.tile([P, NH, HD + 1], fp8,
                                                   tag="v8", name="v8")
                        s = vstage[tci]
                        gb = g_t[:, kti, half * 8:(half + 1) * 8]
                        nc.vector.tensor_tensor(
                            s[:, half * 8: (half + 1) * 8, 0:HD],
                            ps[:].rearrange("p (h e) -> p h e", h=8),
                            gb.unsqueeze(-1).broadcast_to([P, 8, HD]),
                            op=ALU.mult,
                        )
                        nc.vector.tensor_copy(
                            s[:, half * 8: (half + 1) * 8, HD: HD + 1],
                            gb.unsqueeze(-1))
                        if half == 1:
                            nc.sync.dma_start(
                                vd.rearrange("h p kt e -> p h kt e")[:, :, kti, :],
                                s[:],
                            )

                    units.append(u)
            return units

        # ---------------- attention ----------------
        exp_ctr = [0]

        def attn_units(qt_sb, kd, vd, att_dst, qi, hp, pat):
            """returns list of quanta closures for unit (qi, hp)."""
            state = {}
            h0, h1 = 2 * hp, 2 * hp + 1
            hg = hp // 2

            def get_ks():
                if "ks" not in state:
                    ks = kvp.tile([P, 2, T], fp8, tag="ks")
                    # halves: QK groups 0-3 only need tokens 0-1023, so the
                    # first half unblocks as soon as x-blocks 0-1 are sunk
                    nc.sync.dma_start(ks[:, :, 0:TQ], kd[hg][:, :, 0:TQ])
                    nc.sync.dma_start(ks[:, :, TQ:T], kd[hg][:, :, TQ:T])
                    state["ks"] = ks
                return state["ks"]

            def get_vs(hi):
                if "vs" not in state:
                    v = vsp.tile([P, 2, KT, HD + 1], fp8, tag="vs", name="vs")
                    nc.sync.dma_start(
                        v[:], vd.rearrange("h p kt e -> p h kt e")
                        [:, 2 * hp: 2 * hp + 2, :, :])
                    state["vs"] = v
                return state["vs"][:, hi]

            def get_pt(hi):
                key = f"pt{hi}"
                if key not in state:
                    state[key] = ppool.tile([P, 8, 2, 512], fp8, tag="pt",
                                            name=f"pt{hi}")
                return state[key]

            def qk_quantum(hi, g8lo, g8hi):
                def u():
                    ks = get_ks()
                    h = 2 * hp + hi
                    base = 32 * (h % 4)
                    for g8 in range(g8lo, g8hi):
                        ps = ps_qk.tile([P, 2, 512], f32, tag="qk")
                        for j in range(2):
                            kt = 2 * g8 + j
                            nc.tensor.matmul(
                                ps[:, j, :],
                                ks[base: base + 32, :, kt * P: (kt + 1) * P],
                                qt_sb[base: base + 32, h // 4, :,
                                      qi * 512: (qi + 1) * 512],
                                start=True, stop=True, perf_mode=DR,
                                tile_position=(base, 0),
                            )
                        pt = get_pt(hi)
                        eng = pat[exp_ctr[0] % len(pat)]
                        exp_ctr[0] += 1
                        if eng == "A":
                            nc.scalar.activation(pt[:, g8, :, :], ps[:],
                                                 AF.Exp, scale=0.125)
                        else:
                            nc.vector.tensor_scalar(
                                pt[:, g8, :, :].bitcast(u8), ps[:],
                                LOG2E, 56.0, op0=ALU.mult, op1=ALU.add)

                return u

            def pv_quantum(hi):
                def u():
                    vs = get_vs(hi)
                    pt = get_pt(hi)
                    otok = state["otok"]
                    for qc in range(4):
                        ps = ps_pv.tile([P, 512], f32, tag="pv")
                        for g8 in range(8):
                            nc.tensor.matmul(
                                ps[:, 0: HD + 1],
                                pt[:, g8, :, qc * P: (qc + 1) * P],
                                vs[:, 2 * g8: 2 * g8 + 2, :],
                                start=(g8 == 0), stop=(g8 == 7), perf_mode=DR,
                            )
                        inv = lsc.tile([P, 1], f32, tag="inv")
                        nc.vector.reciprocal(inv[:], ps[:, HD: HD + 1])
                        nc.vector.tensor_scalar(
                            otok[:, qc, hi * HD: (hi + 1) * HD],
                            ps[:, 0:HD], inv[:], O_SCALE,
                            op0=ALU.mult, op1=ALU.mult)

                return u

            def fin_quantum():
                def u():
                    otok = state["otok"]
                    tr = ps_mm.tile([P, 4, P], bf16, tag="mm", name="otr")
                    for qc in range(4):
                        nc.tensor.transpose(tr[:, qc, :], otok[:, qc, :],
                                            ident16[:])
                    nc.scalar.activation(
                        att_dst[:, hp, qi * 512: (qi + 1) * 512]
                        .rearrange("p (a b) -> p a b", a=4),
                        tr[:], AF.Identity,
                    )

                return u

            def start_quantum():
                def u():
                    state["otok"] = otokp.tile([P, 4, P], bf16, tag="otok",
                                               name="otok")
                    get_ks()
                    get_vs(0)
                    get_vs(1)

                return u

            # head-major: PV of head 0 overlaps head 1's exps, halving the
            # exp->PV convoy on the DVE queue and freeing pt slots earlier.
            # fin is returned separately so the caller can defer it one unit
            # (its deps are stale by then -> no ACT-queue stall).
            return ([start_quantum(), qk_quantum(0, 0, 4), qk_quantum(0, 4, 8),
                     pv_quantum(0), qk_quantum(1, 0, 4), qk_quantum(1, 4, 8),
                     pv_quantum(1)], fin_quantum())

        # ---------------- O-proj + LN ----------------
        def newton_rstd(var_ap):
            """rstd [P,1] f32 from var (+eps) via bit-trick + 2 Newton steps."""
            a = lsc.tile([P, 1], f32, tag="nva")
            nc.vector.tensor_scalar_add(a[:], var_ap, eps_t[:])
            y = lsc.tile([P, 1], f32, tag="nvy")
            nc.vector.tensor_scalar(y[:].bitcast(i32), a[:].bitcast(i32),
                                    one_i[:], None,
                                    op0=ALU.logical_shift_right)
            nc.vector.tensor_tensor(y[:].bitcast(i32), magic[:],
                                    y[:].bitcast(i32), op=ALU.subtract)
            uu = lsc.tile([P, 1], f32, tag="nvu")
            # one Newton step: ~0.17% max rel error on rstd, well inside
            # the tolerance; a second step would double the serial DVE chain
            nc.vector.tensor_tensor(uu[:], y[:], y[:], op=ALU.mult)
            nc.vector.tensor_tensor(uu[:], uu[:], a[:], op=ALU.mult)
            nc.vector.tensor_scalar(uu[:], uu[:], -0.5, 1.5,
                                    op0=ALU.mult, op1=ALU.add)
            nc.vector.tensor_tensor(y[:], y[:], uu[:], op=ALU.mult)
            return y

        def ln_chunk(s, wkey, outT=None, qc=None, out_dram=None):
            """stats+normalize s [P,D] bf16; write transposed to outT or
            f32 rows to out_dram."""
            bns = lsc.tile([P, 2, 6], f32, tag="bns")
            nc.vector.bn_stats(bns[:, 0, :], s[:, 0:512])
            nc.vector.bn_stats(bns[:, 1, :], s[:, 512:D])
            mv = lsc.tile([P, 2], f32, tag="mv")
            nc.vector.bn_aggr(mv[:], bns[:])
            rstd = newton_rstd(mv[:, 1:2])
            if out_dram is not None:
                o = outp.tile([P, D], f32, tag="out")
                nc.vector.tensor_scalar(o[:], s[:], mv[:, 0:1], rstd[:],
                                        op0=ALU.subtract, op1=ALU.mult)
                if not ln_trivial:
                    nc.vector.tensor_tensor(o[:], o[:], ln_bc[wkey + "_w"][:],
                                            op=ALU.mult)
                    nc.vector.tensor_tensor(o[:], o[:], ln_bc[wkey + "_b"][:],
                                            op=ALU.add)
                nc.sync.dma_start(out_dram[qc * P: (qc + 1) * P, :], o[:])
            else:
                nrm = lns.tile([P, D], bf16, tag="nrm")
                nc.vector.tensor_scalar(nrm[:], s[:], mv[:, 0:1], rstd[:],
                                        op0=ALU.subtract, op1=ALU.mult)
                if not ln_trivial:
                    nc.vector.tensor_tensor(nrm[:], nrm[:], ln_bc[wkey + "_w"][:],
                                            op=ALU.mult)
                    nc.vector.tensor_tensor(nrm[:], nrm[:], ln_bc[wkey + "_b"][:],
                                            op=ALU.add)
                tr = ps_mm.tile([P, D], bf16, tag="mm", name="lntr")
                for dt in range(DT):
                    nc.tensor.transpose(tr[:, dt * P: (dt + 1) * P],
                                        nrm[:, dt * P: (dt + 1) * P], ident16[:])
                nc.vector.tensor_copy(
                    outT[:, :, qc * P: (qc + 1) * P],
                    tr[:].rearrange("p (dt c) -> p dt c", dt=DT),
                )

        def oproj_ln_units(att_sb, w_sb, resid_dram, wkey, outT):
            units = []
            for qc in range(QC):

                def u(qc=qc):
                    xq = rsd.tile([P, D], f32, tag="xq")
                    nc.sync.dma_start(xq[:], resid_dram[qc * P: (qc + 1) * P, :])
                    s = lns.tile([P, D], bf16, tag="lns")
                    for half in range(2):
                        ps = ps_mm.tile([P, 512], f32, tag="mm")
                        for t4 in range(4):
                            nc.tensor.matmul(
                                ps[:],
                                att_sb[:, 2 * t4: 2 * t4 + 2, qc * P: (qc + 1) * P],
                                w_sb[:, 2 * t4: 2 * t4 + 2,
                                     half * 512: (half + 1) * 512],
                                start=(t4 == 0), stop=(t4 == 3), perf_mode=DR,
                            )
                        nc.vector.scalar_tensor_tensor(
                            s[:, half * 512: (half + 1) * 512], ps[:],
                            1.0 / (O_SCALE * WO_SCALE),
                            xq[:, half * 512: (half + 1) * 512],
                            op0=ALU.mult, op1=ALU.add)
                    ln_chunk(s, wkey, outT=outT, qc=qc)

                units.append(u)
            return units

        # ---------------- fusion MLP ----------------
        def fus1_units(n0):
            units = []
            for dt in range(DT):

                def u(dt=dt, n0=n0):
                    wt = f1pool.tile([P, 2 * DT, P], bf16, tag="f1")
                    nc.sync.dma_start(wt[:], w_d["f1"][dt])
                    ps = ps_mm.tile([P, 512], f32, tag="mm")
                    for kt in range(DT):
                        nc.tensor.matmul(
                            ps[:], wt[:, kt, :], fusedT["t"][:, kt, n0: n0 + 512],
                            start=(kt == 0), stop=False,
                        )
                    for kt in range(DT):
                        nc.tensor.matmul(
                            ps[:], wt[:, DT + kt, :],
                            fusedT["f"][:, kt, n0: n0 + 512],
                            start=False, stop=(kt == DT - 1),
                        )
                    nc.scalar.activation(
                        hT[:, dt, n0: n0 + 512], ps[:], AF.Gelu,
                        bias=bf1_col[:, dt: dt + 1],
                    )

                units.append(u)
            return units

        def fus2_ln_units(w2_sb):
            units = []
            for qc in range(QC):

                def u(qc=qc):
                    s = lns.tile([P, D], bf16, tag="lns")
                    for half in range(2):
                        ps = ps_mm.tile([P, 512], f32, tag="mm")
                        for dt in range(DT):
                            nc.tensor.matmul(
                                ps[:],
                                hT[:, dt, qc * P: (qc + 1) * P],
                                w2_sb[:, dt, half * 512: (half + 1) * 512],
                                start=(dt == 0), stop=(dt == DT - 1),
                            )
                        nc.vector.tensor_tensor(
                            s[:, half * 512: (half + 1) * 512], ps[:],
                            b2_bc[:, half * 512: (half + 1) * 512], op=ALU.add)
                    ln_chunk(s, "lnu", qc=qc, out_dram=out_d)

                units.append(u)
            return units

        def run_interleaved(primary, filler):
            k = 0
            for i, u in enumerate(primary):
                u()
                want = (i + 1) * len(filler) // len(primary)
                while k < want:
                    filler[k]()
                    k += 1
            while k < len(filler):
                filler[k]()
                k += 1

        def attn_stream(qt_sb, kd, vd, att_dst, order, pat):
            """flat quanta stream over units with fin deferred one unit."""
            stream = []
            prev_fin = None
            for qi, hp in order:
                qs, fin = attn_units(qt_sb, kd, vd, att_dst, qi, hp, pat)
                stream += qs[:2]
                if prev_fin is not None:
                    stream.append(prev_fin)
                stream += qs[2:]
                prev_fin = fin
            stream.append(prev_fin)
            return stream

        # ------------------------------------------------------------------
        # program
        # ------------------------------------------------------------------
        # Phase 1: Kf/Vf/g_f (full T from xfT) then Qt (xtT blocks 0-1)
        xf_load = [x_loader(xfT_d, bi) for bi in range(4)]
        xf_load[0]()
        w_kf = lw8("kf")
        w_vf = lw8("vf")
        vg_f = const.tile([P, DT, NH], fp8, name="vgf")
        nc.sync.dma_start(vg_f[:], vg_d["f"][:, :, :])
        vg_t = const.tile([P, DT, NH], fp8, name="vgt")
        nc.sync.dma_start(vg_t[:], vg_d["t"][:, :, :])
        # queue the remaining x-block DMAs before the w_qt load so the
        # blocks aren't stuck behind its 1MB transfer
        for bi in range(1, 4):
            xf_load[bi]()
        # w_qt rides in the (phase-3) w2pool slot: it doesn't have to wait
        # for a wpool slot, so Qt-proj (and then attention-1) start earlier
        w_qt = w2pool.tile([P, DT, D], fp8, tag="w16", name="w_qt")
        nc.sync.dma_start(w_qt[:, 0:4, :], w_d["qt"][:, 0:4, :])
        nc.sync.dma_start(w_qt[:, 4:8, :], w_d["qt"][:, 4:8, :])
        xt_load = [x_loader(xtT_d, bi) for bi in range(4)]
        xt_load[0]()
        xt_load[1]()
        qt_byblk = [qk_proj_units(w_qt, xt_load[bi], bi * 512,
                                  q_sink(qT["t"], bi * 512)) for bi in range(2)]
        for bi in range(4):
            n0 = bi * 512
            gx = xf_load[bi]
            ku = qk_proj_units(w_kf, gx, n0, k_sink(k_dr["f"], n0))
            gu = g_units(vg_f, gx, n0, g_sb["f"])
            vu = v_units(w_vf, gx, n0, g_sb["f"], v_dr["f"])
            run_interleaved(ku, gu + vu)
        # hg-major Qt emission right after the blocks: w_qt is already
        # resident (w2pool), so these only wait on their x tiles
        for j in range(8):
            qt_byblk[0][j]()
            qt_byblk[1][j]()

        # Phase 2: attn-1 || Kt/Vt/g_t + Qf
        # blocks 0-1 of xtT reuse phase-1 cached tiles (their readers are all
        # emitted before the xs slot cycles again); blocks 2-3 and the Qf xf
        # re-reads get fresh loaders.
        w_kt = lw8("kt")
        w_vt = lw8("vt")
        xt_load2 = [xt_load[0], xt_load[1],
                    x_loader(xtT_d, 2), x_loader(xtT_d, 3)]
        xf_load2 = [x_loader(xfT_d, 0), x_loader(xfT_d, 1)]
        fillers = []
        for bi in range(4):
            n0 = bi * 512
            gx = xt_load2[bi]
            fillers += qk_proj_units(w_kt, gx, n0, k_sink(k_dr["t"], n0))
            fillers += g_units(vg_t, gx, n0, g_sb["t"])
            fillers += v_units(w_vt, gx, n0, g_sb["t"], v_dr["t"])
        w_qf = lw8("qf")
        for bi in range(2):
            n0 = bi * 512
            fillers += qk_proj_units(w_qf, xf_load2[bi], n0, q_sink(qT["f"], n0))
        prim1 = attn_stream(qT["t"], k_dr["f"], v_dr["f"], attnT["t"],
                            [(qi, hp) for qi in range(2) for hp in range(8)],
                            EXP_PATTERN1)
        # issue unit 0's kS/vS DMAs before the interleave so the first QK
        # isn't waiting on the load latency
        prim1[0]()
        run_interleaved(prim1[1:], fillers)

        # Phase 3: attn-2 || O-proj(t)+LN_t, then late: oproj_f qt0 + fus blk0
        w_ot = lw8("ot")
        w_of = lw8("of")
        w_f2 = w2pool.tile([P, DT, D], bf16, tag="w16")
        nc.sync.dma_start(w_f2[:], w_d["f2"][:, :, :])
        oln_t = oproj_ln_units(attnT["t"], w_ot, xtq_d, "lnt", fusedT["t"])
        oln_f = oproj_ln_units(attnT["f"], w_of, xfq_d, "lnf", fusedT["f"])
        f1_0 = fus1_units(0)
        f1_1 = fus1_units(512)
        f2u = fus2_ln_units(w_f2)

        def blob(us):
            def u():
                for x in us:
                    x()

            return u

        # attn-2 processes qt1 FIRST so the qt1 half of the fusion pipeline
        # (oproj_f qc4-7, fus1 blk1, fus2 qc4-7) overlaps the qt0 attention
        # units; only qt0's short chain remains as the tail. f1 gelu blobs
        # keep the ACT table set from thrashing mid-attention.
        noop = lambda: None
        prim2 = attn_stream(qT["f"], k_dr["t"], v_dr["t"], attnT["f"],
                            [(qi, hp) for qi in (1, 0) for hp in range(8)],
                            EXP_PATTERN2)
        half = len(prim2) // 2
        run_interleaved(prim2[:half], list(oln_t))
        run_interleaved(prim2[half:],
                        list(oln_f[4:]) + [noop, blob(f1_1), noop,
                                           blob(f2u[4:6]), blob(f2u[6:8]),
                                           noop])

        # Phase 4 tail: qt0's chain
        for u in oln_f[:4]:
            u()
        for u in f1_0:
            u()
        for u in f2u[:4]:
            u()

    nc.compile()
    return nc


# ---------------------------------------------------------------------------
# host side
# ---------------------------------------------------------------------------
_CACHE = {}


def _get_nc(ln_trivial=True):
    key = f"nc{ln_trivial}"
    if key not in _CACHE:
        _CACHE[key] = _build_nc(ln_trivial)
    return _CACHE[key]


def _qk_perm():
    idx = np.empty(D, np.int64)
    for tile in range(DT):
        hg, dh = tile // 2, tile % 2
        p = np.arange(P)
        head = 4 * hg + p // 32
        d = 32 * dh + p % 32
        idx[tile * P: (tile + 1) * P] = 64 * head + d
    return idx


def _make_in_maps(inputs):
    import ml_dtypes

    F8 = ml_dtypes.float8_e4m3fn

    def wshuf(w, dt_):
        w = np.asarray(w, np.float32)
        nkt = w.shape[0] // P
        return np.ascontiguousarray(
            w.reshape(nkt, P, w.shape[1]).transpose(1, 0, 2)).astype(dt_)

    t = np.asarray(inputs["temporal_tokens"], np.float32)
    f = np.asarray(inputs["feature_tokens"], np.float32)
    perm = _qk_perm()

    shared = {}
    for n in ["qt", "kf", "qf", "kt"]:
        shared[f"w_{n}"] = wshuf(np.asarray(inputs[f"{n}_w"], np.float32)[:, perm], F8)
    for n in ["vf", "vt"]:
        shared[f"w_{n}"] = wshuf(inputs[f"{n}_w"], F8)
    for n in ["ot", "of"]:
        shared[f"w_{n}"] = wshuf(np.asarray(inputs[f"{n}_w"], np.float32) * WO_SCALE, F8)
    f1 = np.asarray(inputs["fus1_w"], np.float32)  # [2D, D]
    # [dt, 128(din-part), 2DT(kt), 128(dout)] per dout-tile
    f1r = f1.reshape(2 * DT, P, DT, P).transpose(2, 1, 0, 3)
    shared["w_f1"] = np.ascontiguousarray(f1r).astype(ml_dtypes.bfloat16)
    shared["w_f2"] = wshuf(inputs["fus2_w"], ml_dtypes.bfloat16)
    kfw = np.asarray(inputs["kf_w"], np.float32)
    ktw = np.asarray(inputs["kt_w"], np.float32)
    qtb = np.asarray(inputs["qt_b"], np.float32)
    qfb = np.asarray(inputs["qf_b"], np.float32)
    vgf = np.stack([kfw[:, 64 * h: 64 * h + 64] @ qtb[64 * h: 64 * h + 64]
                    for h in range(NH)], axis=1) * VG_SCALE
    vgt = np.stack([ktw[:, 64 * h: 64 * h + 64] @ qfb[64 * h: 64 * h + 64]
                    for h in range(NH)], axis=1) * VG_SCALE
    shared["vg_f"] = wshuf(vgf, F8)
    shared["vg_t"] = wshuf(vgt, F8)
    shared["b_f1"] = np.ascontiguousarray(
        np.asarray(inputs["fus1_b"], np.float32).reshape(DT, P).T)
    shared["b2row"] = np.ascontiguousarray(
        np.asarray(inputs["fus2_b"], np.float32).reshape(1, D))

    ln_trivial = all(
        np.all(np.asarray(inputs[k + "_w"]) == 1) and
        np.all(np.asarray(inputs[k + "_b"]) == 0)
        for k in ["ln_t", "ln_f", "ln_fus"])
    if not ln_trivial:
        for src, dst in [("ln_t_w", "lnt_w"), ("ln_t_b", "lnt_b"),
                         ("ln_f_w", "lnf_w"), ("ln_f_b", "lnf_b"),
                         ("ln_fus_w", "lnu_w"), ("ln_fus_b", "lnu_b")]:
            shared[dst] = np.ascontiguousarray(
                np.asarray(inputs[src], np.float32).reshape(1, D))

    rt = (np.asarray(inputs["ot_b"], np.float32)
          + np.asarray(inputs["vf_b"], np.float32) @ np.asarray(inputs["ot_w"], np.float32))
    rf = (np.asarray(inputs["of_b"], np.float32)
          + np.asarray(inputs["vt_b"], np.float32) @ np.asarray(inputs["of_w"], np.float32))

    def xshuf(xT):
        return np.ascontiguousarray(
            xT.reshape(DT, P, T // 512, 512).transpose(2, 1, 0, 3)).astype(F8)

    in_maps = []
    for c in range(8):
        b, half = divmod(c, 2)
        r0 = half * TQ
        xt, xf = t[b], f[b]
        pr = np.concatenate([np.arange(r0, T), np.arange(0, r0)])
        m = dict(shared)
        m["xtT"] = xshuf(xt[pr].T)
        m["xfT"] = xshuf(xf[pr].T)
        m["xtq"] = np.ascontiguousarray(xt[r0: r0 + TQ] + rt)
        m["xfq"] = np.ascontiguousarray(xf[r0: r0 + TQ] + rf)
        in_maps.append(m)
    return in_maps, ln_trivial


def kernel(**inputs):
    try:
        import jax

        jax.config.update("jax_compilation_cache_dir", "/tmp/jaxcache")
        jax.config.update("jax_persistent_cache_min_entry_size_bytes", -1)
        jax.config.update("jax_persistent_cache_min_compile_time_secs", 0.0)
    except Exception:
        pass
    from concourse.bass_utils import run_bass_kernel_spmd

    in_maps, ln_trivial = _make_in_maps(inputs)
    nc = _get_nc(ln_trivial)
    res = run_bass_kernel_spmd(nc, in_maps, list(range(8)))
    out = np.empty((4, T, D), np.float32)
    for c in range(8):
        b, half = divmod(c, 2)
        out[b, half * TQ: (half + 1) * TQ] = res.results[c]["out"]
    return out


# revision 5
# speedup vs baseline: 1.0200x; 1.0081x over previous
"""CoAttentionFusion TRN2 kernel v2 (8 cores SPMD, fp8 DoubleRow + 2-engine exp).

Per core c: batch b=c//2, query-half h=c%2 (1024 q rows); K/V over full T=2048
recomputed per pair (collectives cost more than the 109us of PE they save).

Key techniques vs the bf16 baseline:
- All attention-path matmuls in fp8e4m3 with DoubleRow perf mode (2 k-tiles
  per instruction, 0.5 cycles/row): K/V/Q projections, QK^T (2x32 contraction
  pairs), P@V (P^T stationary -> token-major O), O-projection.
- Q/K weights column-permuted on host so each head's 64 dims land as
  [32 partitions x 2 halves] for the DoubleRow QK layout.
- Q/K biases removed from the matmuls: K-bias is softmax-invariant; Q-bias
  becomes a per-key factor g_k = exp((x_kv @ (Wk@bq))/8) folded into V' rows
  and the denominator column of V'.
- O-proj bias and V-bias@W_o folded into the f32 residual on host.
- exp split between ACT (true exp->fp8) and DVE (Schraudolph uint8 bit-trick
  -> fp8e4m3) per EXP_PATTERN; probabilities consumed as fp8.
- Token-major O-proj output feeds LayerNorm directly (no LN in-transpose);
  LN rstd via Newton rsqrt on DVE (avoids ACT table thrashing with exp).
- fusion MLP stays bf16 (fp8 there fails the tolerance).
"""

import numpy as np

P = 128
D = 1024
T = 2048
TQ = 1024
NH = 16
HD = 64
DT = 8
KT = 16
QC = 8
EPS = 1e-5
LOG2E = 1.4426950408889634
O_SCALE = 32.0
WO_SCALE = 16.0
VG_SCALE = 64.0

# exp engine per (g8, head) slot within a unit: 'A' = ACT true exp,
# 'D' = DVE Schraudolph. Alternating keeps both engines fed. attn-1 runs
# with the projection sinks on ACT (9A/7D); attn-2 has the LN work on DVE
# (11A/5D).
EXP_PATTERN1 = "ADADADAADADADADA"
EXP_PATTERN2 = "ADAADADAADAADAAA" "ADAADADAADAADAAD"

_WQK = ["qt", "kf", "qf", "kt"]


def _build_nc(ln_trivial):
    import concourse.bass as bass
    import concourse.tile as tile
    from concourse import bacc, mybir
    from concourse.masks import make_identity
    from contextlib import ExitStack

    f32 = mybir.dt.float32
    bf16 = mybir.dt.bfloat16
    fp8 = mybir.dt.float8e4
    u8 = mybir.dt.uint8
    i32 = mybir.dt.int32
    AF = mybir.ActivationFunctionType
    ALU = mybir.AluOpType
    DR = mybir.MatmulPerfMode.DoubleRow

    nc = bacc.Bacc("TRN2", target_bir_lowering=False, debug=False, num_devices=8)

    # ---------------- DRAM I/O ----------------
    xtT_d = nc.dram_tensor("xtT", [T // 512, P, DT, 512], fp8, kind="ExternalInput")
    xfT_d = nc.dram_tensor("xfT", [T // 512, P, DT, 512], fp8, kind="ExternalInput")
    xtq_d = nc.dram_tensor("xtq", [TQ, D], bf16, kind="ExternalInput")
    xfq_d = nc.dram_tensor("xfq", [TQ, D], bf16, kind="ExternalInput")
    w_d = {}
    for n in ["qt", "kf", "vf", "qf", "kt", "vt", "ot", "of"]:
        w_d[n] = nc.dram_tensor(f"w_{n}", [P, DT, D], fp8, kind="ExternalInput")
    w_d["f1"] = nc.dram_tensor("w_f1", [DT, P, 2 * DT, P], bf16, kind="ExternalInput")
    w_d["f2"] = nc.dram_tensor("w_f2", [P, DT, D], bf16, kind="ExternalInput")
    vg_d = {"f": nc.dram_tensor("vg_f", [P, DT, NH], fp8, kind="ExternalInput"),
            "t": nc.dram_tensor("vg_t", [P, DT, NH], fp8, kind="ExternalInput")}
    bf1_d = nc.dram_tensor("b_f1", [P, DT], f32, kind="ExternalInput")
    b2_d = nc.dram_tensor("b2row", [1, D], f32, kind="ExternalInput")
    ln_d = {}
    if not ln_trivial:
        for n in ["lnt_w", "lnt_b", "lnf_w", "lnf_b", "lnu_w", "lnu_b"]:
            ln_d[n] = nc.dram_tensor(n, [1, D], f32, kind="ExternalInput")
    out_d = nc.dram_tensor("out", [TQ, D], f32, kind="ExternalOutput")

    with tile.TileContext(nc) as tc, ExitStack() as ctx:
        const = ctx.enter_context(tc.tile_pool(name="const", bufs=1))
        res = ctx.enter_context(tc.tile_pool(name="res", bufs=1))
        wpool = ctx.enter_context(tc.tile_pool(name="wpool", bufs=2))
        w2pool = ctx.enter_context(tc.tile_pool(name="w2pool", bufs=1))
        f1pool = ctx.enter_context(tc.tile_pool(name="f1pool", bufs=2))
        xs = ctx.enter_context(tc.tile_pool(name="xs", bufs=3))
        kvp = ctx.enter_context(tc.tile_pool(name="kvp", bufs=2))
        vsp = ctx.enter_context(tc.tile_pool(name="vsp", bufs=2))
        ppool = ctx.enter_context(tc.tile_pool(name="ppool", bufs=3))
        otokp = ctx.enter_context(tc.tile_pool(name="otokp", bufs=2))
        gpool = ctx.enter_context(tc.tile_pool(name="gpool", bufs=1))
        stg = ctx.enter_context(tc.tile_pool(name="stg", bufs=3))
        lns = ctx.enter_context(tc.tile_pool(name="lns", bufs=3))
        lsc = ctx.enter_context(tc.tile_pool(name="lsc", bufs=4))
        rowp = ctx.enter_context(tc.tile_pool(name="rowp", bufs=1))
        rsd = ctx.enter_context(tc.tile_pool(name="rsd", bufs=2))
        outp = ctx.enter_context(tc.tile_pool(name="outp", bufs=1))
        dram = ctx.enter_context(tc.tile_pool(name="dram", bufs=1, space="DRAM"))
        ps_qk = ctx.enter_context(tc.tile_pool(name="ps_qk", bufs=2, space="PSUM"))
        ps_pv = ctx.enter_context(tc.tile_pool(name="ps_pv", bufs=2, space="PSUM"))
        ps_mm = ctx.enter_context(tc.tile_pool(name="ps_mm", bufs=2, space="PSUM"))

        ident16 = const.tile([P, P], bf16, name="ident16")
        make_identity(nc, ident16[:])
        eps_t = const.tile([P, 1], f32, name="eps")
        nc.gpsimd.memset(eps_t[:], EPS)
        magic = const.tile([P, 1], i32, name="magic")
        nc.gpsimd.memset(magic[:], 0x5F3759DF)
        one_i = const.tile([P, 1], i32, name="one_i")
        nc.gpsimd.memset(one_i[:], 1)

        def row_bcast(dram_t, tag, dt_=f32):
            r = rowp.tile([1, D], f32, tag="row")
            nc.sync.dma_start(r[:], dram_t)
            if dt_ is not f32:
                rr = rowp.tile([1, D], dt_, tag="rowc")
                nc.vector.tensor_copy(rr[:], r[:])
                r = rr
            b = const.tile([P, D], dt_, name=tag)
            nc.gpsimd.partition_broadcast(b[:], r[:])
            return b

        b2_bc = row_bcast(b2_d[:, :], "b2bc")
        ln_bc = {}
        if not ln_trivial:
            for n in ["lnt_w", "lnt_b", "lnf_w", "lnf_b", "lnu_w", "lnu_b"]:
                ln_bc[n] = row_bcast(ln_d[n][:, :], n)
        bf1_col = const.tile([P, DT], f32, name="bf1")
        nc.sync.dma_start(bf1_col[:], bf1_d[:, :])

        def lw8(name):
            # two DMAs: first half unblocks the first matmuls, and each DMA
            # costs ~625ns of HWDGE issue time so fewer is better
            t = wpool.tile([P, DT, D], fp8, tag="w8")
            nc.sync.dma_start(t[:, 0:4, :], w_d[name][:, 0:4, :])
            nc.sync.dma_start(t[:, 4:8, :], w_d[name][:, 4:8, :])
            return t

        # DRAM scratch
        k_dr = {"f": dram.tile([4, P, 2, T], fp8, name="kf_dr"),
                "t": dram.tile([4, P, 2, T], fp8, name="kt_dr")}
        v_dr = {"f": dram.tile([NH, P, KT, HD + 1], fp8, name="vf_dr"),
                "t": dram.tile([NH, P, KT, HD + 1], fp8, name="vt_dr")}

        # resident activations
        qT = {"t": res.tile([P, 4, 2, TQ], fp8, name="qT_t"),
              "f": res.tile([P, 4, 2, TQ], fp8, name="qT_f")}
        attnT = {"t": res.tile([P, DT, TQ], fp8, name="attnT_t"),
                 "f": res.tile([P, DT, TQ], fp8, name="attnT_f")}
        fusedT = {"t": res.tile([P, DT, TQ], bf16, name="fusedT_t"),
                  "f": res.tile([P, DT, TQ], bf16, name="fusedT_f")}
        hT = res.tile([P, DT, TQ], bf16, name="hT")
        g_sb = {"f": gpool.tile([P, KT, NH], f32, name="g_f"),
                "t": gpool.tile([P, KT, NH], f32, name="g_t")}

        def x_loader(x_dram, bi):
            blk = {}

            def get():
                if "x" not in blk:
                    xb = xs.tile([P, DT, 512], fp8, tag="xs")
                    nc.sync.dma_start(xb[:], x_dram[bi])
                    blk["x"] = xb
                return blk["x"]

            return get

        # ---------------- unit builders ----------------
        def qk_proj_units(w_sb, get_x, n0, sink):
            units = []
            for hg in range(4):
                for dh in range(2):

                    def u(hg=hg, dh=dh):
                        ps = ps_mm.tile([P, 512], f32, tag="mm")
                        xb = get_x()
                        cs = (2 * hg + dh) * P
                        for t4 in range(4):
                            nc.tensor.matmul(
                                ps[:],
                                w_sb[:, 2 * t4: 2 * t4 + 2, cs: cs + P],
                                xb[:, 2 * t4: 2 * t4 + 2, :],
                                start=(t4 == 0), stop=(t4 == 3), perf_mode=DR,
                            )
                        sink(hg, dh, ps)

                    units.append(u)
            return units

        def k_sink(kd, n0):
            # batch the two dh halves of one hg into a single DMA (each DMA
            # costs ~625ns of HWDGE issue time)
            stage = {}

            def sink(hg, dh, ps):
                if hg not in stage:
                    stage[hg] = stg.tile([P, 2, 512], fp8, tag="k8", name="k8")
                nc.scalar.activation(stage[hg][:, dh, :], ps[:], AF.Identity)
                if dh == 1:
                    nc.sync.dma_start(kd[hg][:, :, n0: n0 + 512], stage[hg][:])

            return sink

        def q_sink(qdst, n0):
            # DVE copy: attention phases are ACT-walled, startup is balanced
            def sink(hg, dh, ps):
                nc.vector.tensor_copy(qdst[:, hg, dh, n0: n0 + 512], ps[:])

            return sink

        def g_units(vg_sb, get_x, n0, g_t):
            units = []
            for tci in range(4):

                def u(tci=tci):
                    ps = ps_mm.tile([P, 512], f32, tag="mm")
                    xb = get_x()
                    for t4 in range(4):
                        nc.tensor.matmul(
                            ps[:, 0:NH],
                            xb[:, 2 * t4: 2 * t4 + 2, tci * P: (tci + 1) * P],
                            vg_sb[:, 2 * t4: 2 * t4 + 2, :],
                            start=(t4 == 0), stop=(t4 == 3), perf_mode=DR,
                        )
                    kti = (n0 + tci * P) // P
                    nc.scalar.activation(g_t[:, kti, :], ps[:, 0:NH], AF.Exp,
                                         scale=1.0 / (8.0 * VG_SCALE))

                units.append(u)
            return units

        def v_units(w_sb, get_x, n0, g_t, vd):
            units = []
            vstage = {}
            for tci in range(4):
                for half in range(2):

                    def u(tci=tci, half=half):
                        ps = ps_mm.tile([P, 512], f32, tag="mm")
                        xb = get_x()
                        for t4 in range(4):
                            nc.tensor.matmul(
                                ps[:],
                                xb[:, 2 * t4: 2 * t4 + 2, tci * P: (tci + 1) * P],
                                w_sb[:, 2 * t4: 2 * t4 + 2,
                                     half * 512: (half + 1) * 512],
                                start=(t4 == 0), stop=(t4 == 3), perf_mode=DR,
                            )
                        kti = (n0 + tci * P) // P
                        if tci not in vstage:
                            vstage[tci] = stg.tile([P, NH, HD + 1], fp8,
                                                   tag="v8", name="v8")
                        s = vstage[tci]
                        gb = g_t[:, kti, half * 8:(half + 1) * 8]
                        nc.vector.tensor_tensor(
                            s[:, half * 8: (half + 1) * 8, 0:HD],
                            ps[:].rearrange("p (h e) -> p h e", h=8),
                            gb.unsqueeze(-1).broadcast_to([P, 8, HD]),
                            op=ALU.mult,
                        )
                        nc.vector.tensor_copy(
                            s[:, half * 8: (half + 1) * 8, HD: HD + 1],
                            gb.unsqueeze(-1))
                        if half == 1:
                            nc.sync.dma_start(
                                vd.rearrange("h p kt e -> p h kt e")[:, :, kti, :],
                                s[:],
                            )

                    units.append(u)
            return units

        # ---------------- attention ----------------
        exp_ctr = [0]

        def attn_units(qt_sb, kd, vd, att_dst, qi, hp, pat):
            """returns list of quanta closures for unit (qi, hp)."""
            state = {}
            h0, h1 = 2 * hp, 2 * hp + 1
            hg = hp // 2

            def get_ks():
                if "ks" not in state:
                    ks = kvp.tile([P, 2, T], fp8, tag="ks")
                    # halves: QK groups 0-3 only need tokens 0-1023, so the
                    # first half unblocks as soon as x-blocks 0-1 are sunk
                    nc.sync.dma_start(ks[:, :, 0:TQ], kd[hg][:, :, 0:TQ])
                    nc.sync.dma_start(ks[:, :, TQ:T], kd[hg][:, :, TQ:T])
                    state["ks"] = ks
                return state["ks"]

            def get_vs(hi):
                if "vs" not in state:
                    v = vsp.tile([P, 2, KT, HD + 1], fp8, tag="vs", name="vs")
                    nc.sync.dma_start(
                        v[:], vd.rearrange("h p kt e -> p h kt e")
                        [:, 2 * hp: 2 * hp + 2, :, :])
                    state["vs"] = v
                return state["vs"][:, hi]

            def get_pt(hi):
                key = f"pt{hi}"
                if key not in state:
                    state[key] = ppool.tile([P, 8, 2, 512], fp8, tag="pt",
                                            name=f"pt{hi}")
                return state[key]

            def qk_quantum(hi, g8lo, g8hi):
                def u():
                    ks = get_ks()
                    h = 2 * hp + hi
                    base = 32 * (h % 4)
                    for g8 in range(g8lo, g8hi):
                        ps = ps_qk.tile([P, 2, 512], f32, tag="qk")
                        for j in range(2):
                            kt = 2 * g8 + j
                            nc.tensor.matmul(
                                ps[:, j, :],
                                ks[base: base + 32, :, kt * P: (kt + 1) * P],
                                qt_sb[base: base + 32, h // 4, :,
                                      qi * 512: (qi + 1) * 512],
                                start=True, stop=True, perf_mode=DR,
                                tile_position=(base, 0),
                            )
                        pt = get_pt(hi)
                        eng = pat[exp_ctr[0] % len(pat)]
                        exp_ctr[0] += 1
                        if eng == "A":
                            nc.scalar.activation(pt[:, g8, :, :], ps[:],
                                                 AF.Exp, scale=0.125)
                        else:
                            nc.vector.tensor_scalar(
                                pt[:, g8, :, :].bitcast(u8), ps[:],
                                LOG2E, 56.0, op0=ALU.mult, op1=ALU.add)

                return u

            def pv_quantum(hi):
                def u():
                    vs = get_vs(hi)
                    pt = get_pt(hi)
                    otok = state["otok"]
                    for qc in range(4):
                        ps = ps_pv.tile([P, 512], f32, tag="pv")
                        for g8 in range(8):
                            nc.tensor.matmul(
                                ps[:, 0: HD + 1],
                                pt[:, g8, :, qc * P: (qc + 1) * P],
                                vs[:, 2 * g8: 2 * g8 + 2, :],
                                start=(g8 == 0), stop=(g8 == 7), perf_mode=DR,
                            )
                        inv = lsc.tile([P, 1], f32, tag="inv")
                        nc.vector.reciprocal(inv[:], ps[:, HD: HD + 1])
                        nc.vector.tensor_scalar(
                            otok[:, qc, hi * HD: (hi + 1) * HD],
                            ps[:, 0:HD], inv[:], O_SCALE,
                            op0=ALU.mult, op1=ALU.mult)

                return u

            def fin_quantum():
                def u():
                    otok = state["otok"]
                    tr = ps_mm.tile([P, 4, P], bf16, tag="mm", name="otr")
                    for qc in range(4):
                        nc.tensor.transpose(tr[:, qc, :], otok[:, qc, :],
                                            ident16[:])
                    nc.scalar.activation(
                        att_dst[:, hp, qi * 512: (qi + 1) * 512]
                        .rearrange("p (a b) -> p a b", a=4),
                        tr[:], AF.Identity,
                    )

                return u

            def start_quantum():
                def u():
                    state["otok"] = otokp.tile([P, 4, P], bf16, tag="otok",
                                               name="otok")
                    get_ks()
                    get_vs(0)
                    get_vs(1)

                return u

            # head-major: PV of head 0 overlaps head 1's exps, halving the
            # exp->PV convoy on the DVE queue and freeing pt slots earlier.
            # fin is returned separately so the caller can defer it one unit
            # (its deps are stale by then -> no ACT-queue stall).
            return ([start_quantum(), qk_quantum(0, 0, 4), qk_quantum(0, 4, 8),
                     pv_quantum(0), qk_quantum(1, 0, 4), qk_quantum(1, 4, 8),
                     pv_quantum(1)], fin_quantum())

        # ---------------- O-proj + LN ----------------
        def newton_rstd(var_ap):
            """rstd [P,1] f32 from var (+eps) via bit-trick + 2 Newton steps."""
            a = lsc.tile([P, 1], f32, tag="nva")
            nc.vector.tensor_scalar_add(a[:], var_ap, eps_t[:])
            y = lsc.tile([P, 1], f32, tag="nvy")
            nc.vector.tensor_scalar(y[:].bitcast(i32), a[:].bitcast(i32),
                                    one_i[:], None,
                                    op0=ALU.logical_shift_right)
            nc.vector.tensor_tensor(y[:].bitcast(i32), magic[:],
                                    y[:].bitcast(i32), op=ALU.subtract)
            uu = lsc.tile([P, 1], f32, tag="nvu")
            # one Newton step: ~0.17% max rel error on rstd, well inside
            # the tolerance; a second step would double the serial DVE chain
            nc.vector.tensor_tensor(uu[:], y[:], y[:], op=ALU.mult)
            nc.vector.tensor_tensor(uu[:], uu[:], a[:], op=ALU.mult)
            nc.vector.tensor_scalar(uu[:], uu[:], -0.5, 1.5,
                                    op0=ALU.mult, op1=ALU.add)
            nc.vector.tensor_tensor(y[:], y[:], uu[:], op=ALU.mult)
            return y

        def ln_chunk(s, wkey, outT=None, qc=None, out_dram=None):
            """stats+normalize s [P,D] bf16; write transposed to outT or
            f32 rows to out_dram."""
            bns = lsc.tile([P, 2, 6], f32, tag="bns")
            nc.vector.bn_stats(bns[:, 0, :], s[:, 0:512])
            nc.vector.bn_stats(bns[:, 1, :], s[:, 512:D])
            mv = lsc.tile([P, 2], f32, tag="mv")
            nc.vector.bn_aggr(mv[:], bns[:])
            rstd = newton_rstd(mv[:, 1:2])
            if out_dram is not None:
                o = outp.tile([P, D], f32, tag="out")
                nc.vector.tensor_scalar(o[:], s[:], mv[:, 0:1], rstd[:],
                                        op0=ALU.subtract, op1=ALU.mult)
                if not ln_trivial:
                    nc.vector.tensor_tensor(o[:], o[:], ln_bc[wkey + "_w"][:],
                                            op=ALU.mult)
                    nc.vector.tensor_tensor(o[:], o[:], ln_bc[wkey + "_b"][:],
                                            op=ALU.add)
                nc.sync.dma_start(out_dram[qc * P: (qc + 1) * P, :], o[:])
            else:
                nrm = lns.tile([P, D], bf16, tag="nrm")
                nc.vector.tensor_scalar(nrm[:], s[:], mv[:, 0:1], rstd[:],
                                        op0=ALU.subtract, op1=ALU.mult)
                if not ln_trivial:
                    nc.vector.tensor_tensor(nrm[:], nrm[:], ln_bc[wkey + "_w"][:],
                                            op=ALU.mult)
                    nc.vector.tensor_tensor(nrm[:], nrm[:], ln_bc[wkey + "_b"][:],
                                            op=ALU.add)
                tr = ps_mm.tile([P, D], bf16, tag="mm", name="lntr")
                for dt in range(DT):
                    nc.tensor.transpose(tr[:, dt * P: (dt + 1) * P],
                                        nrm[:, dt * P: (dt + 1) * P], ident16[:])
                nc.vector.tensor_copy(
                    outT[:, :, qc * P: (qc + 1) * P],
                    tr[:].rearrange("p (dt c) -> p dt c", dt=DT),
                )

        def oproj_ln_units(att_sb, w_sb, resid_dram, wkey, outT):
            units = []
            for qc in range(QC):

                def u(qc=qc):
                    xq = rsd.tile([P, D], bf16, tag="xq")
                    nc.sync.dma_start(xq[:], resid_dram[qc * P: (qc + 1) * P, :])
                    s = lns.tile([P, D], bf16, tag="lns")
                    for half in range(2):
                        ps = ps_mm.tile([P, 512], f32, tag="mm")
                        for t4 in range(4):
                            nc.tensor.matmul(
                                ps[:],
                                att_sb[:, 2 * t4: 2 * t4 + 2, qc * P: (qc + 1) * P],
                                w_sb[:, 2 * t4: 2 * t4 + 2,
                                     half * 512: (half + 1) * 512],
                                start=(t4 == 0), stop=(t4 == 3), perf_mode=DR,
                            )
                        nc.vector.scalar_tensor_tensor(
                            s[:, half * 512: (half + 1) * 512], ps[:],
                            1.0 / (O_SCALE * WO_SCALE),
                            xq[:, half * 512: (half + 1) * 512],
                            op0=ALU.mult, op1=ALU.add)
                    ln_chunk(s, wkey, outT=outT, qc=qc)

                units.append(u)
            return units

        # ---------------- fusion MLP ----------------
        def fus1_units(n0):
            units = []
            for dt in range(DT):

                def u(dt=dt, n0=n0):
                    wt = f1pool.tile([P, 2 * DT, P], bf16, tag="f1")
                    nc.sync.dma_start(wt[:], w_d["f1"][dt])
                    ps = ps_mm.tile([P, 512], f32, tag="mm")
                    for kt in range(DT):
                        nc.tensor.matmul(
                            ps[:], wt[:, kt, :], fusedT["t"][:, kt, n0: n0 + 512],
                            start=(kt == 0), stop=False,
                        )
                    for kt in range(DT):
                        nc.tensor.matmul(
                            ps[:], wt[:, DT + kt, :],
                            fusedT["f"][:, kt, n0: n0 + 512],
                            start=False, stop=(kt == DT - 1),
                        )
                    nc.scalar.activation(
                        hT[:, dt, n0: n0 + 512], ps[:], AF.Gelu,
                        bias=bf1_col[:, dt: dt + 1],
                    )

                units.append(u)
            return units

        def fus2_ln_units(w2_sb):
            units = []
            for qc in range(QC):

                def u(qc=qc):
                    s = lns.tile([P, D], bf16, tag="lns")
                    for half in range(2):
                        ps = ps_mm.tile([P, 512], f32, tag="mm")
                        for dt in range(DT):
                            nc.tensor.matmul(
                                ps[:],
                                hT[:, dt, qc * P: (qc + 1) * P],
                                w2_sb[:, dt, half * 512: (half + 1) * 512],
                                start=(dt == 0), stop=(dt == DT - 1),
                            )
                        nc.vector.tensor_tensor(
                            s[:, half * 512: (half + 1) * 512], ps[:],
                            b2_bc[:, half * 512: (half + 1) * 512], op=ALU.add)
                    ln_chunk(s, "lnu", qc=qc, out_dram=out_d)

                units.append(u)
            return units

        def run_interleaved(primary, filler):
            k = 0
            for i, u in enumerate(primary):
                u()
                want = (i + 1) * len(filler) // len(primary)
                while k < want:
                    filler[k]()
                    k += 1
            while k < len(filler):
                filler[k]()
                k += 1

        def attn_stream(qt_sb, kd, vd, att_dst, order, pat):
            """flat quanta stream over units with fin deferred one unit."""
            stream = []
            prev_fin = None
            for qi, hp in order:
                qs, fin = attn_units(qt_sb, kd, vd, att_dst, qi, hp, pat)
                stream += qs[:2]
                if prev_fin is not None:
                    stream.append(prev_fin)
                stream += qs[2:]
                prev_fin = fin
            stream.append(prev_fin)
            return stream

        # ------------------------------------------------------------------
        # program
        # ------------------------------------------------------------------
        # Phase 1: Kf/Vf/g_f (full T from xfT) then Qt (xtT blocks 0-1)
        xf_load = [x_loader(xfT_d, bi) for bi in range(4)]
        xf_load[0]()
        w_kf = lw8("kf")
        w_vf = lw8("vf")
        vg_f = const.tile([P, DT, NH], fp8, name="vgf")
        nc.sync.dma_start(vg_f[:], vg_d["f"][:, :, :])
        vg_t = const.tile([P, DT, NH], fp8, name="vgt")
        nc.sync.dma_start(vg_t[:], vg_d["t"][:, :, :])
        # queue the remaining x-block DMAs before the w_qt load so the
        # blocks aren't stuck behind its 1MB transfer
        for bi in range(1, 4):
            xf_load[bi]()
        # w_qt rides in the (phase-3) w2pool slot: it doesn't have to wait
        # for a wpool slot, so Qt-proj (and then attention-1) start earlier
        w_qt = w2pool.tile([P, DT, D], fp8, tag="w16", name="w_qt")
        nc.sync.dma_start(w_qt[:, 0:4, :], w_d["qt"][:, 0:4, :])
        nc.sync.dma_start(w_qt[:, 4:8, :], w_d["qt"][:, 4:8, :])
        xt_load = [x_loader(xtT_d, bi) for bi in range(4)]
        xt_load[0]()
        xt_load[1]()
        qt_byblk = [qk_proj_units(w_qt, xt_load[bi], bi * 512,
                                  q_sink(qT["t"], bi * 512)) for bi in range(2)]
        for bi in range(4):
            n0 = bi * 512
            gx = xf_load[bi]
            ku = qk_proj_units(w_kf, gx, n0, k_sink(k_dr["f"], n0))
            gu = g_units(vg_f, gx, n0, g_sb["f"])
            vu = v_units(w_vf, gx, n0, g_sb["f"], v_dr["f"])
            run_interleaved(ku, gu + vu)
        # hg-major Qt emission right after the blocks: w_qt is already
        # resident (w2pool), so these only wait on their x tiles
        for j in range(8):
            qt_byblk[0][j]()
            qt_byblk[1][j]()

        # Phase 2: attn-1 || Kt/Vt/g_t + Qf
        # blocks 0-1 of xtT reuse phase-1 cached tiles (their readers are all
        # emitted before the xs slot cycles again); blocks 2-3 and the Qf xf
        # re-reads get fresh loaders.
        w_kt = lw8("kt")
        w_vt = lw8("vt")
        xt_load2 = [xt_load[0], xt_load[1],
                    x_loader(xtT_d, 2), x_loader(xtT_d, 3)]
        xf_load2 = [x_loader(xfT_d, 0), x_loader(xfT_d, 1)]
        fillers = []
        for bi in range(4):
            n0 = bi * 512
            gx = xt_load2[bi]
            fillers += qk_proj_units(w_kt, gx, n0, k_sink(k_dr["t"], n0))
            fillers += g_units(vg_t, gx, n0, g_sb["t"])
            fillers += v_units(w_vt, gx, n0, g_sb["t"], v_dr["t"])
        w_qf = lw8("qf")
        for bi in range(2):
            n0 = bi * 512
            fillers += qk_proj_units(w_qf, xf_load2[bi], n0, q_sink(qT["f"], n0))
        prim1 = attn_stream(qT["t"], k_dr["f"], v_dr["f"], attnT["t"],
                            [(qi, hp) for qi in range(2) for hp in range(8)],
                            EXP_PATTERN1)
        # issue unit 0's kS/vS DMAs before the interleave so the first QK
        # isn't waiting on the load latency
        prim1[0]()
        run_interleaved(prim1[1:], fillers)

        # Phase 3: attn-2 || O-proj(t)+LN_t, then late: oproj_f qt0 + fus blk0
        w_ot = lw8("ot")
        w_of = lw8("of")
        w_f2 = w2pool.tile([P, DT, D], bf16, tag="w16")
        nc.sync.dma_start(w_f2[:], w_d["f2"][:, :, :])
        oln_t = oproj_ln_units(attnT["t"], w_ot, xtq_d, "lnt", fusedT["t"])
        oln_f = oproj_ln_units(attnT["f"], w_of, xfq_d, "lnf", fusedT["f"])
        f1_0 = fus1_units(0)
        f1_1 = fus1_units(512)
        f2u = fus2_ln_units(w_f2)

        def blob(us):
            def u():
                for x in us:
                    x()

            return u

        # attn-2 processes qt1 FIRST so the qt1 half of the fusion pipeline
        # (oproj_f qc4-7, fus1 blk1, fus2 qc4-7) overlaps the qt0 attention
        # units; only qt0's short chain remains as the tail. f1 gelu blobs
        # keep the ACT table set from thrashing mid-attention.
        noop = lambda: None
        prim2 = attn_stream(qT["f"], k_dr["t"], v_dr["t"], attnT["f"],
                            [(qi, hp) for qi in (1, 0) for hp in range(8)],
                            EXP_PATTERN2)
        half = len(prim2) // 2
        run_interleaved(prim2[:half], list(oln_t))
        run_interleaved(prim2[half:],
                        list(oln_f[4:]) + [noop, blob(f1_1), noop,
                                           blob(f2u[4:6]), blob(f2u[6:8]),
                                           noop])

        # Phase 4 tail: qt0's chain
        for u in oln_f[:4]:
            u()
        for u in f1_0:
            u()
        for u in f2u[:4]:
            u()

    nc.compile()
    return nc


# ---------------------------------------------------------------------------
# host side
# ---------------------------------------------------------------------------
_CACHE = {}


def _get_nc(ln_trivial=True):
    key = f"nc{ln_trivial}"
    if key not in _CACHE:
        _CACHE[key] = _build_nc(ln_trivial)
    return _CACHE[key]


def _qk_perm():
    idx = np.empty(D, np.int64)
    for tile in range(DT):
        hg, dh = tile // 2, tile % 2
        p = np.arange(P)
        head = 4 * hg + p // 32
        d = 32 * dh + p % 32
        idx[tile * P: (tile + 1) * P] = 64 * head + d
    return idx


def _make_in_maps(inputs):
    import ml_dtypes

    F8 = ml_dtypes.float8_e4m3fn

    def wshuf(w, dt_):
        w = np.asarray(w, np.float32)
        nkt = w.shape[0] // P
        return np.ascontiguousarray(
            w.reshape(nkt, P, w.shape[1]).transpose(1, 0, 2)).astype(dt_)

    t = np.asarray(inputs["temporal_tokens"], np.float32)
    f = np.asarray(inputs["feature_tokens"], np.float32)
    perm = _qk_perm()

    shared = {}
    for n in ["qt", "kf", "qf", "kt"]:
        shared[f"w_{n}"] = wshuf(np.asarray(inputs[f"{n}_w"], np.float32)[:, perm], F8)
    for n in ["vf", "vt"]:
        shared[f"w_{n}"] = wshuf(inputs[f"{n}_w"], F8)
    for n in ["ot", "of"]:
        shared[f"w_{n}"] = wshuf(np.asarray(inputs[f"{n}_w"], np.float32) * WO_SCALE, F8)
    f1 = np.asarray(inputs["fus1_w"], np.float32)  # [2D, D]
    # [dt, 128(din-part), 2DT(kt), 128(dout)] per dout-tile
    f1r = f1.reshape(2 * DT, P, DT, P).transpose(2, 1, 0, 3)
    shared["w_f1"] = np.ascontiguousarray(f1r).astype(ml_dtypes.bfloat16)
    shared["w_f2"] = wshuf(inputs["fus2_w"], ml_dtypes.bfloat16)
    kfw = np.asarray(inputs["kf_w"], np.float32)
    ktw = np.asarray(inputs["kt_w"], np.float32)
    qtb = np.asarray(inputs["qt_b"], np.float32)
    qfb = np.asarray(inputs["qf_b"], np.float32)
    vgf = np.stack([kfw[:, 64 * h: 64 * h + 64] @ qtb[64 * h: 64 * h + 64]
                    for h in range(NH)], axis=1) * VG_SCALE
    vgt = np.stack([ktw[:, 64 * h: 64 * h + 64] @ qfb[64 * h: 64 * h + 64]
                    for h in range(NH)], axis=1) * VG_SCALE
    shared["vg_f"] = wshuf(vgf, F8)
    shared["vg_t"] = wshuf(vgt, F8)
    shared["b_f1"] = np.ascontiguousarray(
        np.asarray(inputs["fus1_b"], np.float32).reshape(DT, P).T)
    shared["b2row"] = np.ascontiguousarray(
        np.asarray(inputs["fus2_b"], np.float32).reshape(1, D))

    ln_trivial = all(
        np.all(np.asarray(inputs[k + "_w"]) == 1) and
        np.all(np.asarray(inputs[k + "_b"]) == 0)
        for k in ["ln_t", "ln_f", "ln_fus"])
    if not ln_trivial:
        for src, dst in [("ln_t_w", "lnt_w"), ("ln_t_b", "lnt_b"),
                         ("ln_f_w", "lnf_w"), ("ln_f_b", "lnf_b"),
                         ("ln_fus_w", "lnu_w"), ("ln_fus_b", "lnu_b")]:
            shared[dst] = np.ascontiguousarray(
                np.asarray(inputs[src], np.float32).reshape(1, D))

    rt = (np.asarray(inputs["ot_b"], np.float32)
          + np.asarray(inputs["vf_b"], np.float32) @ np.asarray(inputs["ot_w"], np.float32))
    rf = (np.asarray(inputs["of_b"], np.float32)
          + np.asarray(inputs["vt_b"], np.float32) @ np.asarray(inputs["of_w"], np.float32))

    def xshuf(xT):
        return np.ascontiguousarray(
            xT.reshape(DT, P, T // 512, 512).transpose(2, 1, 0, 3)).astype(F8)

    in_maps = []
    for c in range(8):
        b, half = divmod(c, 2)
        r0 = half * TQ
        xt, xf = t[b], f[b]
        pr = np.concatenate([np.arange(r0, T), np.arange(0, r0)])
        m = dict(shared)
        m["xtT"] = xshuf(xt[pr].T)
        m["xfT"] = xshuf(xf[pr].T)
        m["xtq"] = np.ascontiguousarray(xt[r0: r0 + TQ] + rt).astype(
            ml_dtypes.bfloat16)
        m["xfq"] = np.ascontiguousarray(xf[r0: r0 + TQ] + rf).astype(
            ml_dtypes.bfloat16)
        in_maps.append(m)
    return in_maps, ln_trivial


def kernel(**inputs):
    try:
        import jax

        jax.config.update("jax_compilation_cache_dir", "/tmp/jaxcache")
        jax.config.update("jax_persistent_cache_min_entry_size_bytes", -1)
        jax.config.update("jax_persistent_cache_min_compile_time_secs", 0.0)
    except Exception:
        pass
    from concourse.bass_utils import run_bass_kernel_spmd

    in_maps, ln_trivial = _make_in_maps(inputs)
    nc = _get_nc(ln_trivial)
    res = run_bass_kernel_spmd(nc, in_maps, list(range(8)))
    out = np.empty((4, T, D), np.float32)
    for c in range(8):
        b, half = divmod(c, 2)
        out[b, half * TQ: (half + 1) * TQ] = res.results[c]["out"]
    return out


# revision 6
# speedup vs baseline: 1.0218x; 1.0017x over previous
"""CoAttentionFusion TRN2 kernel v2 (8 cores SPMD, fp8 DoubleRow + 2-engine exp).

Per core c: batch b=c//2, query-half h=c%2 (1024 q rows); K/V over full T=2048
recomputed per pair (collectives cost more than the 109us of PE they save).

Key techniques vs the bf16 baseline:
- All attention-path matmuls in fp8e4m3 with DoubleRow perf mode (2 k-tiles
  per instruction, 0.5 cycles/row): K/V/Q projections, QK^T (2x32 contraction
  pairs), P@V (P^T stationary -> token-major O), O-projection.
- Q/K weights column-permuted on host so each head's 64 dims land as
  [32 partitions x 2 halves] for the DoubleRow QK layout.
- Q/K biases removed from the matmuls: K-bias is softmax-invariant; Q-bias
  becomes a per-key factor g_k = exp((x_kv @ (Wk@bq))/8) folded into V' rows
  and the denominator column of V'.
- O-proj bias and V-bias@W_o folded into the f32 residual on host.
- exp split between ACT (true exp->fp8) and DVE (Schraudolph uint8 bit-trick
  -> fp8e4m3) per EXP_PATTERN; probabilities consumed as fp8.
- Token-major O-proj output feeds LayerNorm directly (no LN in-transpose);
  LN rstd via Newton rsqrt on DVE (avoids ACT table thrashing with exp).
- fusion MLP stays bf16 (fp8 there fails the tolerance).
"""

import numpy as np

P = 128
D = 1024
T = 2048
TQ = 1024
NH = 16
HD = 64
DT = 8
KT = 16
QC = 8
EPS = 1e-5
LOG2E = 1.4426950408889634
O_SCALE = 32.0
WO_SCALE = 16.0
VG_SCALE = 64.0

# exp engine per (g8, head) slot within a unit: 'A' = ACT true exp,
# 'D' = DVE Schraudolph. Alternating keeps both engines fed. attn-1 runs
# with the projection sinks on ACT (9A/7D); attn-2 has the LN work on DVE
# (11A/5D).
EXP_PATTERN1 = "ADADADAADADADADA"
EXP_PATTERN2 = "ADAADADAADAADAAA" "ADAADADAADAADAAD"

_WQK = ["qt", "kf", "qf", "kt"]


def _build_nc(ln_trivial):
    import concourse.bass as bass
    import concourse.tile as tile
    from concourse import bacc, mybir
    from concourse.masks import make_identity
    from contextlib import ExitStack

    f32 = mybir.dt.float32
    bf16 = mybir.dt.bfloat16
    fp8 = mybir.dt.float8e4
    u8 = mybir.dt.uint8
    i32 = mybir.dt.int32
    AF = mybir.ActivationFunctionType
    ALU = mybir.AluOpType
    DR = mybir.MatmulPerfMode.DoubleRow

    nc = bacc.Bacc("TRN2", target_bir_lowering=False, debug=False, num_devices=8)

    # ---------------- DRAM I/O ----------------
    xtT_d = nc.dram_tensor("xtT", [T // 512, P, DT, 512], fp8, kind="ExternalInput")
    xfT_d = nc.dram_tensor("xfT", [T // 512, P, DT, 512], fp8, kind="ExternalInput")
    xtq_d = nc.dram_tensor("xtq", [TQ, D], bf16, kind="ExternalInput")
    xfq_d = nc.dram_tensor("xfq", [TQ, D], bf16, kind="ExternalInput")
    w_d = {}
    for n in ["qt", "kf", "vf", "qf", "kt", "vt", "ot", "of"]:
        w_d[n] = nc.dram_tensor(f"w_{n}", [P, DT, D], fp8, kind="ExternalInput")
    w_d["f1"] = nc.dram_tensor("w_f1", [DT, P, 2 * DT, P], bf16, kind="ExternalInput")
    w_d["f2"] = nc.dram_tensor("w_f2", [P, DT, D], bf16, kind="ExternalInput")
    vg_d = {"f": nc.dram_tensor("vg_f", [P, DT, NH], fp8, kind="ExternalInput"),
            "t": nc.dram_tensor("vg_t", [P, DT, NH], fp8, kind="ExternalInput")}
    bf1_d = nc.dram_tensor("b_f1", [P, DT], f32, kind="ExternalInput")
    b2_d = nc.dram_tensor("b2row", [1, D], f32, kind="ExternalInput")
    ln_d = {}
    if not ln_trivial:
        for n in ["lnt_w", "lnt_b", "lnf_w", "lnf_b", "lnu_w", "lnu_b"]:
            ln_d[n] = nc.dram_tensor(n, [1, D], f32, kind="ExternalInput")
    out_d = nc.dram_tensor("out", [TQ, D], f32, kind="ExternalOutput")

    with tile.TileContext(nc) as tc, ExitStack() as ctx:
        const = ctx.enter_context(tc.tile_pool(name="const", bufs=1))
        res = ctx.enter_context(tc.tile_pool(name="res", bufs=1))
        wpool = ctx.enter_context(tc.tile_pool(name="wpool", bufs=2))
        w2pool = ctx.enter_context(tc.tile_pool(name="w2pool", bufs=1))
        f1pool = ctx.enter_context(tc.tile_pool(name="f1pool", bufs=2))
        xs = ctx.enter_context(tc.tile_pool(name="xs", bufs=3))
        kvp = ctx.enter_context(tc.tile_pool(name="kvp", bufs=2))
        vsp = ctx.enter_context(tc.tile_pool(name="vsp", bufs=2))
        ppool = ctx.enter_context(tc.tile_pool(name="ppool", bufs=2))
        otokp = ctx.enter_context(tc.tile_pool(name="otokp", bufs=2))
        gpool = ctx.enter_context(tc.tile_pool(name="gpool", bufs=1))
        stg = ctx.enter_context(tc.tile_pool(name="stg", bufs=3))
        lns = ctx.enter_context(tc.tile_pool(name="lns", bufs=3))
        lsc = ctx.enter_context(tc.tile_pool(name="lsc", bufs=4))
        rowp = ctx.enter_context(tc.tile_pool(name="rowp", bufs=1))
        rsd = ctx.enter_context(tc.tile_pool(name="rsd", bufs=2))
        outp = ctx.enter_context(tc.tile_pool(name="outp", bufs=1))
        dram = ctx.enter_context(tc.tile_pool(name="dram", bufs=1, space="DRAM"))
        ps_qk = ctx.enter_context(tc.tile_pool(name="ps_qk", bufs=2, space="PSUM"))
        ps_pv = ctx.enter_context(tc.tile_pool(name="ps_pv", bufs=2, space="PSUM"))
        ps_mm = ctx.enter_context(tc.tile_pool(name="ps_mm", bufs=2, space="PSUM"))

        ident16 = const.tile([P, P], bf16, name="ident16")
        make_identity(nc, ident16[:])
        eps_t = const.tile([P, 1], f32, name="eps")
        nc.gpsimd.memset(eps_t[:], EPS)
        magic = const.tile([P, 1], i32, name="magic")
        nc.gpsimd.memset(magic[:], 0x5F3759DF)
        one_i = const.tile([P, 1], i32, name="one_i")
        nc.gpsimd.memset(one_i[:], 1)

        def row_bcast(dram_t, tag, dt_=f32):
            r = rowp.tile([1, D], f32, tag="row")
            nc.sync.dma_start(r[:], dram_t)
            if dt_ is not f32:
                rr = rowp.tile([1, D], dt_, tag="rowc")
                nc.vector.tensor_copy(rr[:], r[:])
                r = rr
            b = const.tile([P, D], dt_, name=tag)
            nc.gpsimd.partition_broadcast(b[:], r[:])
            return b

        b2_bc = row_bcast(b2_d[:, :], "b2bc")
        ln_bc = {}
        if not ln_trivial:
            for n in ["lnt_w", "lnt_b", "lnf_w", "lnf_b", "lnu_w", "lnu_b"]:
                ln_bc[n] = row_bcast(ln_d[n][:, :], n)
        bf1_col = const.tile([P, DT], f32, name="bf1")
        nc.sync.dma_start(bf1_col[:], bf1_d[:, :])

        def lw8(name):
            # two DMAs: first half unblocks the first matmuls, and each DMA
            # costs ~625ns of HWDGE issue time so fewer is better
            t = wpool.tile([P, DT, D], fp8, tag="w8")
            nc.sync.dma_start(t[:, 0:4, :], w_d[name][:, 0:4, :])
            nc.sync.dma_start(t[:, 4:8, :], w_d[name][:, 4:8, :])
            return t

        # DRAM scratch
        k_dr = {"f": dram.tile([4, P, 2, T], fp8, name="kf_dr"),
                "t": dram.tile([4, P, 2, T], fp8, name="kt_dr")}
        v_dr = {"f": dram.tile([NH, P, KT, HD + 1], fp8, name="vf_dr"),
                "t": dram.tile([NH, P, KT, HD + 1], fp8, name="vt_dr")}

        # resident activations
        qT = {"t": res.tile([P, 4, 2, TQ], fp8, name="qT_t"),
              "f": res.tile([P, 4, 2, TQ], fp8, name="qT_f")}
        attnT = {"t": res.tile([P, DT, TQ], fp8, name="attnT_t"),
                 "f": res.tile([P, DT, TQ], fp8, name="attnT_f")}
        fusedT = {"t": res.tile([P, DT, TQ], bf16, name="fusedT_t"),
                  "f": res.tile([P, DT, TQ], bf16, name="fusedT_f")}
        hT = res.tile([P, DT, TQ], bf16, name="hT")
        # spilled fusedT_t-half partial sums of fus1 block 0 (tail shortener)
        h_t0 = res.tile([P, DT, 512], bf16, name="h_t0")
        g_sb = {"f": gpool.tile([P, KT, NH], f32, name="g_f"),
                "t": gpool.tile([P, KT, NH], f32, name="g_t")}

        def x_loader(x_dram, bi):
            blk = {}

            def get():
                if "x" not in blk:
                    xb = xs.tile([P, DT, 512], fp8, tag="xs")
                    nc.sync.dma_start(xb[:], x_dram[bi])
                    blk["x"] = xb
                return blk["x"]

            return get

        # ---------------- unit builders ----------------
        def qk_proj_units(w_sb, get_x, n0, sink):
            units = []
            for hg in range(4):
                for dh in range(2):

                    def u(hg=hg, dh=dh):
                        ps = ps_mm.tile([P, 512], f32, tag="mm")
                        xb = get_x()
                        cs = (2 * hg + dh) * P
                        for t4 in range(4):
                            nc.tensor.matmul(
                                ps[:],
                                w_sb[:, 2 * t4: 2 * t4 + 2, cs: cs + P],
                                xb[:, 2 * t4: 2 * t4 + 2, :],
                                start=(t4 == 0), stop=(t4 == 3), perf_mode=DR,
                            )
                        sink(hg, dh, ps)

                    units.append(u)
            return units

        def k_sink(kd, n0):
            # batch the two dh halves of one hg into a single DMA (each DMA
            # costs ~625ns of HWDGE issue time)
            stage = {}

            def sink(hg, dh, ps):
                if hg not in stage:
                    stage[hg] = stg.tile([P, 2, 512], fp8, tag="k8", name="k8")
                nc.scalar.activation(stage[hg][:, dh, :], ps[:], AF.Identity)
                if dh == 1:
                    nc.sync.dma_start(kd[hg][:, :, n0: n0 + 512], stage[hg][:])

            return sink

        def q_sink(qdst, n0):
            # DVE copy: attention phases are ACT-walled, startup is balanced
            def sink(hg, dh, ps):
                nc.vector.tensor_copy(qdst[:, hg, dh, n0: n0 + 512], ps[:])

            return sink

        def g_units(vg_sb, get_x, n0, g_t):
            units = []
            for tci in range(4):

                def u(tci=tci):
                    ps = ps_mm.tile([P, 512], f32, tag="mm")
                    xb = get_x()
                    for t4 in range(4):
                        nc.tensor.matmul(
                            ps[:, 0:NH],
                            xb[:, 2 * t4: 2 * t4 + 2, tci * P: (tci + 1) * P],
                            vg_sb[:, 2 * t4: 2 * t4 + 2, :],
                            start=(t4 == 0), stop=(t4 == 3), perf_mode=DR,
                        )
                    kti = (n0 + tci * P) // P
                    nc.scalar.activation(g_t[:, kti, :], ps[:, 0:NH], AF.Exp,
                                         scale=1.0 / (8.0 * VG_SCALE))

                units.append(u)
            return units

        def v_units(w_sb, get_x, n0, g_t, vd):
            units = []
            vstage = {}
            for tci in range(4):
                for half in range(2):

                    def u(tci=tci, half=half):
                        ps = ps_mm.tile([P, 512], f32, tag="mm")
                        xb = get_x()
                        for t4 in range(4):
                            nc.tensor.matmul(
                                ps[:],
                                xb[:, 2 * t4: 2 * t4 + 2, tci * P: (tci + 1) * P],
                                w_sb[:, 2 * t4: 2 * t4 + 2,
                                     half * 512: (half + 1) * 512],
                                start=(t4 == 0), stop=(t4 == 3), perf_mode=DR,
                            )
                        kti = (n0 + tci * P) // P
                        if tci not in vstage:
                            vstage[tci] = stg.tile([P, NH, HD + 1], fp8,
                                                   tag="v8", name="v8")
                        s = vstage[tci]
                        gb = g_t[:, kti, half * 8:(half + 1) * 8]
                        nc.vector.tensor_tensor(
                            s[:, half * 8: (half + 1) * 8, 0:HD],
                            ps[:].rearrange("p (h e) -> p h e", h=8),
                            gb.unsqueeze(-1).broadcast_to([P, 8, HD]),
                            op=ALU.mult,
                        )
                        nc.vector.tensor_copy(
                            s[:, half * 8: (half + 1) * 8, HD: HD + 1],
                            gb.unsqueeze(-1))
                        if half == 1:
                            nc.sync.dma_start(
                                vd.rearrange("h p kt e -> p h kt e")[:, :, kti, :],
                                s[:],
                            )

                    units.append(u)
            return units

        # ---------------- attention ----------------
        exp_ctr = [0]

        def attn_units(qt_sb, kd, vd, att_dst, qi, hp, pat):
            """returns list of quanta closures for unit (qi, hp)."""
            state = {}
            h0, h1 = 2 * hp, 2 * hp + 1
            hg = hp // 2

            def get_ks():
                if "ks" not in state:
                    ks = kvp.tile([P, 2, T], fp8, tag="ks")
                    # halves: QK groups 0-3 only need tokens 0-1023, so the
                    # first half unblocks as soon as x-blocks 0-1 are sunk
                    nc.sync.dma_start(ks[:, :, 0:TQ], kd[hg][:, :, 0:TQ])
                    nc.sync.dma_start(ks[:, :, TQ:T], kd[hg][:, :, TQ:T])
                    state["ks"] = ks
                return state["ks"]

            def get_vs(hi):
                if "vs" not in state:
                    v = vsp.tile([P, 2, KT, HD + 1], fp8, tag="vs", name="vs")
                    nc.sync.dma_start(
                        v[:], vd.rearrange("h p kt e -> p h kt e")
                        [:, 2 * hp: 2 * hp + 2, :, :])
                    state["vs"] = v
                return state["vs"][:, hi]

            def get_pt(hi):
                key = f"pt{hi}"
                if key not in state:
                    state[key] = ppool.tile([P, 8, 2, 512], fp8, tag="pt",
                                            name=f"pt{hi}")
                return state[key]

            def qk_quantum(hi, g8lo, g8hi):
                def u():
                    ks = get_ks()
                    h = 2 * hp + hi
                    base = 32 * (h % 4)
                    for g8 in range(g8lo, g8hi):
                        ps = ps_qk.tile([P, 2, 512], f32, tag="qk")
                        for j in range(2):
                            kt = 2 * g8 + j
                            nc.tensor.matmul(
                                ps[:, j, :],
                                ks[base: base + 32, :, kt * P: (kt + 1) * P],
                                qt_sb[base: base + 32, h // 4, :,
                                      qi * 512: (qi + 1) * 512],
                                start=True, stop=True, perf_mode=DR,
                                tile_position=(base, 0),
                            )
                        pt = get_pt(hi)
                        eng = pat[exp_ctr[0] % len(pat)]
                        exp_ctr[0] += 1
                        if eng == "A":
                            nc.scalar.activation(pt[:, g8, :, :], ps[:],
                                                 AF.Exp, scale=0.125)
                        else:
                            nc.vector.tensor_scalar(
                                pt[:, g8, :, :].bitcast(u8), ps[:],
                                LOG2E, 56.0, op0=ALU.mult, op1=ALU.add)

                return u

            def pv_quantum(hi):
                def u():
                    vs = get_vs(hi)
                    pt = get_pt(hi)
                    otok = state["otok"]
                    for qc in range(4):
                        ps = ps_pv.tile([P, 512], f32, tag="pv")
                        for g8 in range(8):
                            nc.tensor.matmul(
                                ps[:, 0: HD + 1],
                                pt[:, g8, :, qc * P: (qc + 1) * P],
                                vs[:, 2 * g8: 2 * g8 + 2, :],
                                start=(g8 == 0), stop=(g8 == 7), perf_mode=DR,
                            )
                        inv = lsc.tile([P, 1], f32, tag="inv")
                        nc.vector.reciprocal(inv[:], ps[:, HD: HD + 1])
                        nc.vector.tensor_scalar(
                            otok[:, qc, hi * HD: (hi + 1) * HD],
                            ps[:, 0:HD], inv[:], O_SCALE,
                            op0=ALU.mult, op1=ALU.mult)

                return u

            def fin_quantum():
                def u():
                    otok = state["otok"]
                    tr = ps_mm.tile([P, 4, P], bf16, tag="mm", name="otr")
                    for qc in range(4):
                        nc.tensor.transpose(tr[:, qc, :], otok[:, qc, :],
                                            ident16[:])
                    nc.scalar.activation(
                        att_dst[:, hp, qi * 512: (qi + 1) * 512]
                        .rearrange("p (a b) -> p a b", a=4),
                        tr[:], AF.Identity,
                    )

                return u

            def start_quantum():
                def u():
                    state["otok"] = otokp.tile([P, 4, P], bf16, tag="otok",
                                               name="otok")
                    get_ks()
                    get_vs(0)
                    get_vs(1)

                return u

            # head-major: PV of head 0 overlaps head 1's exps, halving the
            # exp->PV convoy on the DVE queue and freeing pt slots earlier.
            # fin is returned separately so the caller can defer it one unit
            # (its deps are stale by then -> no ACT-queue stall).
            return ([start_quantum(), qk_quantum(0, 0, 4), qk_quantum(0, 4, 8),
                     pv_quantum(0), qk_quantum(1, 0, 4), qk_quantum(1, 4, 8),
                     pv_quantum(1)], fin_quantum())

        # ---------------- O-proj + LN ----------------
        def newton_rstd(var_ap):
            """rstd [P,1] f32 from var (+eps) via bit-trick + 2 Newton steps."""
            a = lsc.tile([P, 1], f32, tag="nva")
            nc.vector.tensor_scalar_add(a[:], var_ap, eps_t[:])
            y = lsc.tile([P, 1], f32, tag="nvy")
            nc.vector.tensor_scalar(y[:].bitcast(i32), a[:].bitcast(i32),
                                    one_i[:], None,
                                    op0=ALU.logical_shift_right)
            nc.vector.tensor_tensor(y[:].bitcast(i32), magic[:],
                                    y[:].bitcast(i32), op=ALU.subtract)
            uu = lsc.tile([P, 1], f32, tag="nvu")
            # one Newton step: ~0.17% max rel error on rstd, well inside
            # the tolerance; a second step would double the serial DVE chain
            nc.vector.tensor_tensor(uu[:], y[:], y[:], op=ALU.mult)
            nc.vector.tensor_tensor(uu[:], uu[:], a[:], op=ALU.mult)
            nc.vector.tensor_scalar(uu[:], uu[:], -0.5, 1.5,
                                    op0=ALU.mult, op1=ALU.add)
            nc.vector.tensor_tensor(y[:], y[:], uu[:], op=ALU.mult)
            return y

        def ln_chunk(s, wkey, outT=None, qc=None, out_dram=None):
            """stats+normalize s [P,D] bf16; write transposed to outT or
            f32 rows to out_dram."""
            bns = lsc.tile([P, 2, 6], f32, tag="bns")
            nc.vector.bn_stats(bns[:, 0, :], s[:, 0:512])
            nc.vector.bn_stats(bns[:, 1, :], s[:, 512:D])
            mv = lsc.tile([P, 2], f32, tag="mv")
            nc.vector.bn_aggr(mv[:], bns[:])
            rstd = newton_rstd(mv[:, 1:2])
            if out_dram is not None:
                o = outp.tile([P, D], f32, tag="out")
                nc.vector.tensor_scalar(o[:], s[:], mv[:, 0:1], rstd[:],
                                        op0=ALU.subtract, op1=ALU.mult)
                if not ln_trivial:
                    nc.vector.tensor_tensor(o[:], o[:], ln_bc[wkey + "_w"][:],
                                            op=ALU.mult)
                    nc.vector.tensor_tensor(o[:], o[:], ln_bc[wkey + "_b"][:],
                                            op=ALU.add)
                nc.sync.dma_start(out_dram[qc * P: (qc + 1) * P, :], o[:])
            else:
                nrm = lns.tile([P, D], bf16, tag="nrm")
                nc.vector.tensor_scalar(nrm[:], s[:], mv[:, 0:1], rstd[:],
                                        op0=ALU.subtract, op1=ALU.mult)
                if not ln_trivial:
                    nc.vector.tensor_tensor(nrm[:], nrm[:], ln_bc[wkey + "_w"][:],
                                            op=ALU.mult)
                    nc.vector.tensor_tensor(nrm[:], nrm[:], ln_bc[wkey + "_b"][:],
                                            op=ALU.add)
                tr = ps_mm.tile([P, D], bf16, tag="mm", name="lntr")
                for dt in range(DT):
                    nc.tensor.transpose(tr[:, dt * P: (dt + 1) * P],
                                        nrm[:, dt * P: (dt + 1) * P], ident16[:])
                nc.vector.tensor_copy(
                    outT[:, :, qc * P: (qc + 1) * P],
                    tr[:].rearrange("p (dt c) -> p dt c", dt=DT),
                )

        def oproj_ln_units(att_sb, w_sb, resid_dram, wkey, outT):
            units = []
            for qc in range(QC):

                def u(qc=qc):
                    xq = rsd.tile([P, D], bf16, tag="xq")
                    nc.sync.dma_start(xq[:], resid_dram[qc * P: (qc + 1) * P, :])
                    s = lns.tile([P, D], bf16, tag="lns")
                    for half in range(2):
                        ps = ps_mm.tile([P, 512], f32, tag="mm")
                        for t4 in range(4):
                            nc.tensor.matmul(
                                ps[:],
                                att_sb[:, 2 * t4: 2 * t4 + 2, qc * P: (qc + 1) * P],
                                w_sb[:, 2 * t4: 2 * t4 + 2,
                                     half * 512: (half + 1) * 512],
                                start=(t4 == 0), stop=(t4 == 3), perf_mode=DR,
                            )
                        nc.vector.scalar_tensor_tensor(
                            s[:, half * 512: (half + 1) * 512], ps[:],
                            1.0 / (O_SCALE * WO_SCALE),
                            xq[:, half * 512: (half + 1) * 512],
                            op0=ALU.mult, op1=ALU.add)
                    ln_chunk(s, wkey, outT=outT, qc=qc)

                units.append(u)
            return units

        # ---------------- fusion MLP ----------------
        def fus1t_units():
            """block-0 fusedT_t half of the fus1 contraction, spilled to
            SBUF mid-attn-2 so only the fusedT_f half gates the tail."""
            units = []
            for dt in range(DT):

                def u(dt=dt):
                    wt = f1pool.tile([P, 2 * DT, P], bf16, tag="f1")
                    nc.sync.dma_start(wt[:], w_d["f1"][dt])
                    ps = ps_mm.tile([P, 512], f32, tag="mm")
                    for kt in range(DT):
                        nc.tensor.matmul(
                            ps[:], wt[:, kt, :], fusedT["t"][:, kt, 0:512],
                            start=(kt == 0), stop=(kt == DT - 1),
                        )
                    nc.scalar.activation(h_t0[:, dt, :], ps[:], AF.Identity)

                units.append(u)
            return units

        def fus1f_units():
            units = []
            for dt in range(DT):

                def u(dt=dt):
                    wt = f1pool.tile([P, 2 * DT, P], bf16, tag="f1")
                    nc.sync.dma_start(wt[:], w_d["f1"][dt])
                    ps = ps_mm.tile([P, 512], f32, tag="mm")
                    for kt in range(DT):
                        nc.tensor.matmul(
                            ps[:], wt[:, DT + kt, :],
                            fusedT["f"][:, kt, 0:512],
                            start=(kt == 0), stop=(kt == DT - 1),
                        )
                    s = stg.tile([P, 512], bf16, tag="v8", name="f1s")
                    nc.vector.tensor_tensor(s[:], ps[:], h_t0[:, dt, :],
                                            op=ALU.add)
                    nc.scalar.activation(
                        hT[:, dt, 0:512], s[:], AF.Gelu,
                        bias=bf1_col[:, dt: dt + 1],
                    )

                units.append(u)
            return units

        def fus1_units(n0):
            units = []
            for dt in range(DT):

                def u(dt=dt, n0=n0):
                    wt = f1pool.tile([P, 2 * DT, P], bf16, tag="f1")
                    nc.sync.dma_start(wt[:], w_d["f1"][dt])
                    ps = ps_mm.tile([P, 512], f32, tag="mm")
                    for kt in range(DT):
                        nc.tensor.matmul(
                            ps[:], wt[:, kt, :], fusedT["t"][:, kt, n0: n0 + 512],
                            start=(kt == 0), stop=False,
                        )
                    for kt in range(DT):
                        nc.tensor.matmul(
                            ps[:], wt[:, DT + kt, :],
                            fusedT["f"][:, kt, n0: n0 + 512],
                            start=False, stop=(kt == DT - 1),
                        )
                    nc.scalar.activation(
                        hT[:, dt, n0: n0 + 512], ps[:], AF.Gelu,
                        bias=bf1_col[:, dt: dt + 1],
                    )

                units.append(u)
            return units

        def fus2_ln_units(w2_sb):
            units = []
            for qc in range(QC):

                def u(qc=qc):
                    s = lns.tile([P, D], bf16, tag="lns")
                    for half in range(2):
                        ps = ps_mm.tile([P, 512], f32, tag="mm")
                        for dt in range(DT):
                            nc.tensor.matmul(
                                ps[:],
                                hT[:, dt, qc * P: (qc + 1) * P],
                                w2_sb[:, dt, half * 512: (half + 1) * 512],
                                start=(dt == 0), stop=(dt == DT - 1),
                            )
                        nc.vector.tensor_tensor(
                            s[:, half * 512: (half + 1) * 512], ps[:],
                            b2_bc[:, half * 512: (half + 1) * 512], op=ALU.add)
                    ln_chunk(s, "lnu", qc=qc, out_dram=out_d)

                units.append(u)
            return units

        def run_interleaved(primary, filler):
            k = 0
            for i, u in enumerate(primary):
                u()
                want = (i + 1) * len(filler) // len(primary)
                while k < want:
                    filler[k]()
                    k += 1
            while k < len(filler):
                filler[k]()
                k += 1

        def attn_stream(qt_sb, kd, vd, att_dst, order, pat):
            """flat quanta stream over units with fin deferred one unit."""
            stream = []
            prev_fin = None
            for qi, hp in order:
                qs, fin = attn_units(qt_sb, kd, vd, att_dst, qi, hp, pat)
                stream += qs[:2]
                if prev_fin is not None:
                    stream.append(prev_fin)
                stream += qs[2:]
                prev_fin = fin
            stream.append(prev_fin)
            return stream

        # ------------------------------------------------------------------
        # program
        # ------------------------------------------------------------------
        # Phase 1: Kf/Vf/g_f (full T from xfT) then Qt (xtT blocks 0-1)
        xf_load = [x_loader(xfT_d, bi) for bi in range(4)]
        xf_load[0]()
        w_kf = lw8("kf")
        w_vf = lw8("vf")
        vg_f = const.tile([P, DT, NH], fp8, name="vgf")
        nc.sync.dma_start(vg_f[:], vg_d["f"][:, :, :])
        vg_t = const.tile([P, DT, NH], fp8, name="vgt")
        nc.sync.dma_start(vg_t[:], vg_d["t"][:, :, :])
        # queue the remaining x-block DMAs before the w_qt load so the
        # blocks aren't stuck behind its 1MB transfer
        for bi in range(1, 4):
            xf_load[bi]()
        # w_qt rides in the (phase-3) w2pool slot: it doesn't have to wait
        # for a wpool slot, so Qt-proj (and then attention-1) start earlier
        w_qt = w2pool.tile([P, DT, D], fp8, tag="w16", name="w_qt")
        nc.sync.dma_start(w_qt[:, 0:4, :], w_d["qt"][:, 0:4, :])
        nc.sync.dma_start(w_qt[:, 4:8, :], w_d["qt"][:, 4:8, :])
        xt_load = [x_loader(xtT_d, bi) for bi in range(4)]
        xt_load[0]()
        xt_load[1]()
        qt_byblk = [qk_proj_units(w_qt, xt_load[bi], bi * 512,
                                  q_sink(qT["t"], bi * 512)) for bi in range(2)]
        for bi in range(4):
            n0 = bi * 512
            gx = xf_load[bi]
            ku = qk_proj_units(w_kf, gx, n0, k_sink(k_dr["f"], n0))
            gu = g_units(vg_f, gx, n0, g_sb["f"])
            vu = v_units(w_vf, gx, n0, g_sb["f"], v_dr["f"])
            run_interleaved(ku, gu + vu)
        # hg-major Qt emission right after the blocks: w_qt is already
        # resident (w2pool), so these only wait on their x tiles
        for j in range(8):
            qt_byblk[0][j]()
            qt_byblk[1][j]()

        # Phase 2: attn-1 || Kt/Vt/g_t + Qf
        # blocks 0-1 of xtT reuse phase-1 cached tiles (their readers are all
        # emitted before the xs slot cycles again); blocks 2-3 and the Qf xf
        # re-reads get fresh loaders.
        w_kt = lw8("kt")
        w_vt = lw8("vt")
        xt_load2 = [xt_load[0], xt_load[1],
                    x_loader(xtT_d, 2), x_loader(xtT_d, 3)]
        xf_load2 = [x_loader(xfT_d, 0), x_loader(xfT_d, 1)]
        fillers = []
        for bi in range(4):
            n0 = bi * 512
            gx = xt_load2[bi]
            fillers += qk_proj_units(w_kt, gx, n0, k_sink(k_dr["t"], n0))
            fillers += g_units(vg_t, gx, n0, g_sb["t"])
            fillers += v_units(w_vt, gx, n0, g_sb["t"], v_dr["t"])
        w_qf = lw8("qf")
        for bi in range(2):
            n0 = bi * 512
            fillers += qk_proj_units(w_qf, xf_load2[bi], n0, q_sink(qT["f"], n0))
        prim1 = attn_stream(qT["t"], k_dr["f"], v_dr["f"], attnT["t"],
                            [(qi, hp) for qi in range(2) for hp in range(8)],
                            EXP_PATTERN1)
        # issue unit 0's kS/vS DMAs before the interleave so the first QK
        # isn't waiting on the load latency
        prim1[0]()
        run_interleaved(prim1[1:], fillers)

        # Phase 3: attn-2 || O-proj(t)+LN_t, then late: oproj_f qt0 + fus blk0
        w_ot = lw8("ot")
        w_of = lw8("of")
        w_f2 = w2pool.tile([P, DT, D], bf16, tag="w16")
        nc.sync.dma_start(w_f2[:], w_d["f2"][:, :, :])
        oln_t = oproj_ln_units(attnT["t"], w_ot, xtq_d, "lnt", fusedT["t"])
        oln_f = oproj_ln_units(attnT["f"], w_of, xfq_d, "lnf", fusedT["f"])
        f1t_0 = fus1t_units()
        f1f_0 = fus1f_units()
        f1_1 = fus1_units(512)
        f2u = fus2_ln_units(w_f2)

        def blob(us):
            def u():
                for x in us:
                    x()

            return u

        # attn-2 processes qt1 FIRST so the qt1 half of the fusion pipeline
        # (oproj_f qc4-7, fus1 blk1, fus2 qc4-7) overlaps the qt0 attention
        # units; only qt0's short chain remains as the tail. f1 gelu blobs
        # keep the ACT table set from thrashing mid-attention.
        noop = lambda: None
        prim2 = attn_stream(qT["f"], k_dr["t"], v_dr["t"], attnT["f"],
                            [(qi, hp) for qi in (1, 0) for hp in range(8)],
                            EXP_PATTERN2)
        half = len(prim2) // 2
        run_interleaved(prim2[:half], list(oln_t))
        run_interleaved(prim2[half:],
                        [blob(f1t_0)] + list(oln_f[4:])
                        + [noop, blob(f1_1), noop,
                           blob(f2u[4:6]), blob(f2u[6:8]), noop])

        # Phase 4 tail: qt0's chain (fus1's fusedT_t half was pre-computed
        # mid-attn-2 into h_t0, so only the fusedT_f half runs here)
        for u in oln_f[:4]:
            u()
        for u in f1f_0:
            u()
        for u in f2u[:4]:
            u()

    nc.compile()
    return nc


# ---------------------------------------------------------------------------
# host side
# ---------------------------------------------------------------------------
_CACHE = {}


def _get_nc(ln_trivial=True):
    key = f"nc{ln_trivial}"
    if key not in _CACHE:
        _CACHE[key] = _build_nc(ln_trivial)
    return _CACHE[key]


def _qk_perm():
    idx = np.empty(D, np.int64)
    for tile in range(DT):
        hg, dh = tile // 2, tile % 2
        p = np.arange(P)
        head = 4 * hg + p // 32
        d = 32 * dh + p % 32
        idx[tile * P: (tile + 1) * P] = 64 * head + d
    return idx


def _make_in_maps(inputs):
    import ml_dtypes

    F8 = ml_dtypes.float8_e4m3fn

    def wshuf(w, dt_):
        w = np.asarray(w, np.float32)
        nkt = w.shape[0] // P
        return np.ascontiguousarray(
            w.reshape(nkt, P, w.shape[1]).transpose(1, 0, 2)).astype(dt_)

    t = np.asarray(inputs["temporal_tokens"], np.float32)
    f = np.asarray(inputs["feature_tokens"], np.float32)
    perm = _qk_perm()

    shared = {}
    for n in ["qt", "kf", "qf", "kt"]:
        shared[f"w_{n}"] = wshuf(np.asarray(inputs[f"{n}_w"], np.float32)[:, perm], F8)
    for n in ["vf", "vt"]:
        shared[f"w_{n}"] = wshuf(inputs[f"{n}_w"], F8)
    for n in ["ot", "of"]:
        shared[f"w_{n}"] = wshuf(np.asarray(inputs[f"{n}_w"], np.float32) * WO_SCALE, F8)
    f1 = np.asarray(inputs["fus1_w"], np.float32)  # [2D, D]
    # [dt, 128(din-part), 2DT(kt), 128(dout)] per dout-tile
    f1r = f1.reshape(2 * DT, P, DT, P).transpose(2, 1, 0, 3)
    shared["w_f1"] = np.ascontiguousarray(f1r).astype(ml_dtypes.bfloat16)
    shared["w_f2"] = wshuf(inputs["fus2_w"], ml_dtypes.bfloat16)
    kfw = np.asarray(inputs["kf_w"], np.float32)
    ktw = np.asarray(inputs["kt_w"], np.float32)
    qtb = np.asarray(inputs["qt_b"], np.float32)
    qfb = np.asarray(inputs["qf_b"], np.float32)
    vgf = np.stack([kfw[:, 64 * h: 64 * h + 64] @ qtb[64 * h: 64 * h + 64]
                    for h in range(NH)], axis=1) * VG_SCALE
    vgt = np.stack([ktw[:, 64 * h: 64 * h + 64] @ qfb[64 * h: 64 * h + 64]
                    for h in range(NH)], axis=1) * VG_SCALE
    shared["vg_f"] = wshuf(vgf, F8)
    shared["vg_t"] = wshuf(vgt, F8)
    shared["b_f1"] = np.ascontiguousarray(
        np.asarray(inputs["fus1_b"], np.float32).reshape(DT, P).T)
    shared["b2row"] = np.ascontiguousarray(
        np.asarray(inputs["fus2_b"], np.float32).reshape(1, D))

    ln_trivial = all(
        np.all(np.asarray(inputs[k + "_w"]) == 1) and
        np.all(np.asarray(inputs[k + "_b"]) == 0)
        for k in ["ln_t", "ln_f", "ln_fus"])
    if not ln_trivial:
        for src, dst in [("ln_t_w", "lnt_w"), ("ln_t_b", "lnt_b"),
                         ("ln_f_w", "lnf_w"), ("ln_f_b", "lnf_b"),
                         ("ln_fus_w", "lnu_w"), ("ln_fus_b", "lnu_b")]:
            shared[dst] = np.ascontiguousarray(
                np.asarray(inputs[src], np.float32).reshape(1, D))

    rt = (np.asarray(inputs["ot_b"], np.float32)
          + np.asarray(inputs["vf_b"], np.float32) @ np.asarray(inputs["ot_w"], np.float32))
    rf = (np.asarray(inputs["of_b"], np.float32)
          + np.asarray(inputs["vt_b"], np.float32) @ np.asarray(inputs["of_w"], np.float32))

    def xshuf(xT):
        return np.ascontiguousarray(
            xT.reshape(DT, P, T // 512, 512).transpose(2, 1, 0, 3)).astype(F8)

    in_maps = []
    for c in range(8):
        b, half = divmod(c, 2)
        r0 = half * TQ
        xt, xf = t[b], f[b]
        pr = np.concatenate([np.arange(r0, T), np.arange(0, r0)])
        m = dict(shared)
        m["xtT"] = xshuf(xt[pr].T)
        m["xfT"] = xshuf(xf[pr].T)
        m["xtq"] = np.ascontiguousarray(xt[r0: r0 + TQ] + rt).astype(
            ml_dtypes.bfloat16)
        m["xfq"] = np.ascontiguousarray(xf[r0: r0 + TQ] + rf).astype(
            ml_dtypes.bfloat16)
        in_maps.append(m)
    return in_maps, ln_trivial


def kernel(**inputs):
    try:
        import jax

        jax.config.update("jax_compilation_cache_dir", "/tmp/jaxcache")
        jax.config.update("jax_persistent_cache_min_entry_size_bytes", -1)
        jax.config.update("jax_persistent_cache_min_compile_time_secs", 0.0)
    except Exception:
        pass
    from concourse.bass_utils import run_bass_kernel_spmd

    in_maps, ln_trivial = _make_in_maps(inputs)
    nc = _get_nc(ln_trivial)
    res = run_bass_kernel_spmd(nc, in_maps, list(range(8)))
    out = np.empty((4, T, D), np.float32)
    for c in range(8):
        b, half = divmod(c, 2)
        out[b, half * TQ: (half + 1) * TQ] = res.results[c]["out"]
    return out
